# revision 30
# baseline (speedup 1.0000x reference)
"""Dihedral2Coord Trainium2 kernel.

Math: the reference applies K sequential dihedral-set steps; step k rotates
a suffix of the atom chain rigidly about the current J-K bond.  Every step's
transform is a proper rigid motion that moves all four pivot atoms of every
later step together, so the dihedral measured at application time equals the
dihedral of the ORIGINAL coordinates (dihedrals are invariant under rigid
motion).  Step k's rotation, expressed in original coordinates, is therefore
a fixed affine A_k computable from the original positions alone, and the
cumulative transform is the prefix product C_k = A_0 @ A_1 @ ... @ A_k.
The kernel:
  A) computes all K per-conformer Rodrigues affines in parallel,
  B) prefix-composes them with a blocked scan,
  C) applies C_{km(m)} to each atom run, where km(m) counts the steps whose
     mask includes atom m (verified prefix-structured on host).

Sharding: pure data parallelism over conformers N across 8 cores (SPMD).
"""

import sys

import numpy as np

try:
    import concourse.bass as bass
except ImportError:  # path in the grading container
    sys.path.insert(0, "/opt/trn_rl_repo")
    import concourse.bass as bass

import concourse.tile as tile
from concourse import mybir
from concourse.bass_utils import run_bass_kernel_spmd

f32 = mybir.dt.float32
i32 = mybir.dt.int32
Alu = mybir.AluOpType
Act = mybir.ActivationFunctionType

NCORES = 8
P = 128
TWO_PI = float(2.0 * np.pi)
_HALF_PI = float(np.pi / 2)

_WAIT_CAP = 1  # this walrus build rejects >1 sync-wait per instruction


def _register_const(nc, value, dtype=f32):
    """Register an activation-bias constant.  Written on the Activation
    engine from the framework's const-1.0 AP (ordered by Bass.__init__'s
    barrier); later ACT reads are same-engine program-ordered, so no extra
    barrier is needed."""
    if (dtype, value) in nc.const_aps.aps:
        return
    t = nc.alloc_sbuf_tensor(f"const-{dtype.name}-{value}", [128, 1], dtype)
    one = nc.const_aps.aps[(f32, 1.0)]
    nc.scalar.activation(t.ap(), one, Act.Identity, bias=0.0, scale=float(value))
    nc.const_aps.aps[(dtype, value)] = t.ap()


def _split_multi_waits(nc):
    """Split every instruction carrying >cap sync-waits into single-wait
    NoOps (same engine, immediately before, same block).  Waits are monotone
    semaphore conditions so this preserves semantics exactly."""
    n = 0
    for func in nc.m.functions:
        for bb in func.blocks:
            old = list(bb.instructions)
            if not any(
                i.sync_info is not None and len(i.sync_info.on_wait) > _WAIT_CAP
                for i in old
            ):
                continue
            new = []
            for inst in old:
                si = inst.sync_info
                if si is not None and len(si.on_wait) > _WAIT_CAP:
                    waits = list(si.on_wait)
                    head, tail = waits[:-_WAIT_CAP], waits[-_WAIT_CAP:]
                    for j in range(0, len(head), _WAIT_CAP):
                        n += 1
                        new.append(
                            mybir.InstNoOp(
                                name=f"{inst.name}_ws{j}",
                                engine=inst.engine,
                                sync_info=mybir.SyncInfo(
                                    on_wait=list(head[j : j + _WAIT_CAP]), on_update=[]
                                ),
                                bass_nofuse=True,
                            )
                        )
                    try:
                        si.on_wait[:] = tail
                    except TypeError:
                        inst.sync_info = mybir.SyncInfo(
                            on_wait=tail, on_update=list(si.on_update)
                        )
                new.append(inst)
            try:
                bb.instructions[:] = new
            except TypeError:
                bb.instructions = new
    return n


def _ap(base, offset_elems, dims):
    """Free-dim AP view into an SBUF tile AP `base` (partition dim kept).
    dims: list of [step, count] in elements of the tile's free space."""
    return bass.AP(
        tensor=base.tensor,
        offset=base.offset + offset_elems,
        ap=[list(base.ap[0])] + [list(d) for d in dims],
    )


def _dram_ap(t, offset, dims):
    return bass.AP(tensor=t.tensor, offset=offset, ap=[list(d) for d in dims])


def _analyse_mask(angles, move_mask):
    """Host-side structural analysis. Returns (km, runs): km[m] is the last
    step applied to atom m (-1 = never moved); runs are (start, len, k)."""
    K, M = move_mask.shape
    km = move_mask.astype(np.int64).sum(0) - 1
    kk = np.arange(K)[:, None]
    if not (move_mask == (kk <= km[None, :])).all():
        raise NotImplementedError("move_mask is not prefix-structured per atom")
    for k in range(K):
        for a in angles[k]:
            if not move_mask[:k, a].all():
                raise NotImplementedError("pivot atoms not rigidly co-moved")
    runs = []
    m = 0
    while m < M:
        j = m
        while j + 1 < M and km[j + 1] == km[m]:
            j += 1
        if km[m] >= 0:
            runs.append((m, j - m + 1, int(km[m])))
        m = j + 1
    return km, runs


def _build(angles, move_mask, NL, K, M):
    """Build the Bass module for one core handling NL conformers."""
    G = NL // P
    assert NL == G * P
    GK = G * K
    L = 8               # within-block scan length
    assert K % L == 0
    B = K // L          # blocks per conformer-group
    NB = G * B          # blocks over the flattened (g,k) axis

    angles = np.asarray(angles)
    arange_quads = bool((angles == np.arange(K * 4).reshape(K, 4)).all())
    km, runs = _analyse_mask(angles, move_mask)

    nc = bass.Bass()
    for cval in (1024.0, 1024.25, 1024.0 * TWO_PI, 1024.0 * TWO_PI + _HALF_PI):
        _register_const(nc, float(cval))
    SP = min(int(angles.max()) + 1, M)   # pivot region boundary
    # vin and the pivot-region planes travel in ONE array/DMA so only one
    # DMA first-byte latency sits ahead of stage A
    catA = nc.declare_dram_parameter("catA", [P, G * K + 3 * G * SP], f32,
                                     isOutput=False)
    posB = (nc.declare_dram_parameter("posB", [P, 3, G, M - SP], f32,
                                      isOutput=False) if SP < M else None)
    outT = nc.declare_dram_parameter("outT", [P, 3, G, M], f32, isOutput=True)

    with tile.TileContext(nc) as tc:
        with tc.tile_pool(name="main", bufs=1) as pool:
            # ---- SBUF tensors ----
            # pos planes split at SP so stage A only waits on the pivot DMA
            CATA = pool.tile([P, G * K + 3 * G * SP], f32)
            PLB = pool.tile([P, 3, G, M - SP], f32, name="PLB") if SP < M else None
            OUTA = pool.tile([P, 3, G, SP], f32)
            OUTB = pool.tile([P, 3, G, M - SP], f32, name="OUTB") if SP < M else None
            # packed r-vectors / normals with duplicated xy components so a
            # +1/+2 component rotation is a plain offset (cross-product trick)
            RV = pool.tile([P, 3, 5, G, K], f32)  # (rIJ,rJK,rKL) x (x,y,z,x,y)
            NN = pool.tile([P, 3, 5, G, K], f32)  # (nIJK,nJKL,m) x (x,y,z,x,y)
            TA = pool.tile([P, 2, 3, G, K], f32)
            TB = pool.tile([P, 2, 3, G, K], f32)
            AT = pool.tile([P, 12, G, K], f32)   # A_k; q=4i+j, strides q:GK, g:K, k:1
            CT = pool.tile([P, 12, GK], f32)     # full prefixes
            NBP = NB + 4   # 4 pad columns so Hillis j<s lanes read in-bounds
            PT = pool.tile([P, 12, NBP], f32)    # block products / prefixes
            PT2 = pool.tile([P, 12, NBP], f32)   # Hillis ping-pong buffer
            PTB = pool.tile([P, G, 12, (K // 8) * 7], f32)  # prefixes bcast over t
            ACC = pool.tile([P, 12 * max(GK, 64)], f32)
            AC2 = pool.tile([P, 12 * max(GK, 64)], f32)
            AC3 = pool.tile([P, 12 * max(GK, 64)], f32)

            cata = CATA[:, :]
            vv = _ap(cata, 0, [[K, G], [1, K]])
            pla = _ap(cata, GK, [])
            plb = PLB[:, :, :, :] if PLB is not None else None
            outa = OUTA[:, :, :, :]
            outb = OUTB[:, :, :, :] if OUTB is not None else None

            def pl_view(m0, ln, _unused=None):
                """(base_ap, local column offset, group stride, comp stride)
                for columns [m0, m0+ln) — must not cross the SP boundary."""
                if m0 < SP:
                    assert m0 + ln <= SP
                    return pla, m0, SP, G * SP
                return plb, m0 - SP, M - SP, G * (M - SP)

            def out_view(m0, ln):
                if m0 < SP:
                    assert m0 + ln <= SP
                    return outa, m0, SP, G * SP
                return outb, m0 - SP, M - SP, G * (M - SP)
            rv = RV[:, :, :, :, :]
            nn = NN[:, :, :, :, :]
            t1v = TA[:, :, :, :, :]
            t2v = TB[:, :, :, :, :]
            at = AT[:, :, :, :]
            ct = CT[:, :, :]
            pt = _ap(PT[:, :, :], 4, [[NBP, 12], [1, NB]])
            pt2 = _ap(PT2[:, :, :], 4, [[NBP, 12], [1, NB]])
            ptb = PTB[:, :, :, :]
            acc = ACC[:, :]
            ac2 = AC2[:, :]
            ac3 = AC3[:, :]

            RVv, RVc = 5 * GK, GK   # RV strides: vec, comp
            NVv = 5 * GK

            # ---- DMA in ----
            # All on the sync ring, in priority order: vin (tiny, unblocks
            # the ACT sin chain), pivot region (unblocks stage A), rest.
            # Host arrays are partition-major so each partition row is one
            # contiguous multi-KB descriptor.
            row = G * K + 3 * G * SP
            nc.sync.dma_start(
                out=_ap(cata, 0, [[1, row]]),
                in_=_dram_ap(catA[:, :], 0, [[row, P], [1, row]]),
            )
            if PLB is not None:
                nc.sync.dma_start(
                    out=_ap(plb, 0, [[1, 3 * G * (M - SP)]]),
                    in_=_dram_ap(posB[:, :, :, :], 0,
                                 [[3 * G * (M - SP), P], [1, 3 * G * (M - SP)]]),
                )
            # Hillis pad columns must hold finite values (they feed the
            # patched lanes); zero them before the block-prefix scan
            nc.gpsimd.memset(_ap(PT[:, :, :], 0, [[NBP, 12], [1, 4]]), 0.0)
            nc.gpsimd.memset(_ap(PT2[:, :, :], 0, [[NBP, 12], [1, 4]]), 0.0)

            # ---- helpers ----
            tmp_idx = [0]

            def T(dt=f32):
                tmp_idx[0] += 1
                return pool.tile([P, G, K], dt, name=f"tmp{tmp_idx[0]}")

            def mul(a, b):
                o = T(); nc.vector.tensor_mul(o, a, b); return o

            def add(a, b):
                o = T(); nc.vector.tensor_add(o, a, b); return o

            def sub(a, b):
                o = T(); nc.vector.tensor_sub(o, a, b); return o

            def aff(a, scale, bias):
                o = T()
                nc.scalar.activation(o, a, Act.Identity, bias=bias, scale=scale)
                return o

            def activ(a, fn):
                o = T(); nc.scalar.activation(o, a, fn); return o

            def dot3v(a_base, a_off, a_cs, b_base, b_off, b_cs, eng=None):
                """dot over xyz comps via one mul + one innermost-reduce.
                a/b given as (tile_ap, elem offset, comp stride); both must
                have gk contiguous (stride 1)."""
                tmp_idx[0] += 1
                dp = pool.tile([P, GK, 3], f32, name=f"dp{tmp_idx[0]}")[:, :, :]
                (eng or nc.vector).tensor_mul(
                    dp,
                    _ap(a_base, a_off, [[1, GK], [a_cs, 3]]),
                    _ap(b_base, b_off, [[1, GK], [b_cs, 3]]),
                )
                o = T()
                nc.vector.tensor_reduce(
                    _ap(o, 0, [[1, GK]]), dp, mybir.AxisListType.X, Alu.add)
                return o

            # ---- pivot sources ----
            if not arange_quads:
                PIV = pool.tile([P, 3, G, 4, K], f32)
                pv = PIV[:, :, :, :, :]
                for k in range(K):
                    for q in range(4):
                        nc.vector.tensor_copy(
                            _ap(pv, q * K + k, [[G * 4 * K, 3], [4 * K, G]]),
                            _ap(pla, int(angles[k, q]),
                                [[G * SP, 3], [SP, G]]),
                        )

            def piv_ap(c, q):
                if arange_quads:
                    return _ap(pla, c * G * SP + q, [[SP, G], [4, K]])
                return _ap(pv, c * G * 4 * K + q * K, [[4 * K, G], [1, K]])

            pJ = [piv_ap(c, 1) for c in range(3)]

            def _ap_cat3(_pj):
                # the three pJ views share a regular comp stride; rebuild as
                # one 3-dim AP [c][g][k]
                if arange_quads:
                    return _ap(pla, 1, [[G * SP, 3], [SP, G], [4, K]])
                return _ap(pv, K, [[G * 4 * K, 3], [4 * K, G], [1, K]])

            # ---- stage A: packed r-vectors and cross products ----
            # r-vectors need a 4th AP dim for g, so that one op stays per-g
            # (DVE codegen caps free dims at 3); the rest fuse over (g,k)
            for g in range(G):
                if arange_quads:
                    in1 = _ap(pla, g * SP + 1, [[1, 3], [G * SP, 3], [4, K]])
                    in0 = _ap(pla, g * SP + 0, [[1, 3], [G * SP, 3], [4, K]])
                else:
                    in1 = _ap(pv, g * 4 * K + K, [[K, 3], [G * 4 * K, 3], [1, K]])
                    in0 = _ap(pv, g * 4 * K + 0, [[K, 3], [G * 4 * K, 3], [1, K]])
                nc.vector.tensor_sub(
                    _ap(rv, g * K, [[RVv, 3], [RVc, 3], [1, K]]), in1, in0)
            # duplicate comps x,y into slots 3,4
            nc.vector.tensor_copy(
                _ap(rv, 3 * RVc, [[RVv, 3], [RVc, 2], [1, GK]]),
                _ap(rv, 0, [[RVv, 3], [RVc, 2], [1, GK]]))
            # nIJK, nJKL = cross(A=[rIJ,rJK], B=[rJK,rKL]) via comp offsets
            nc.vector.tensor_mul(
                _ap(t1v, 0, [[3 * GK, 2], [GK, 3], [1, GK]]),
                _ap(rv, RVc, [[RVv, 2], [RVc, 3], [1, GK]]),
                _ap(rv, RVv + 2 * RVc, [[RVv, 2], [RVc, 3], [1, GK]]))
            nc.vector.tensor_mul(
                _ap(t2v, 0, [[3 * GK, 2], [GK, 3], [1, GK]]),
                _ap(rv, 2 * RVc, [[RVv, 2], [RVc, 3], [1, GK]]),
                _ap(rv, RVv + RVc, [[RVv, 2], [RVc, 3], [1, GK]]))
            nc.vector.tensor_sub(
                _ap(nn, 0, [[NVv, 2], [GK, 3], [1, GK]]),
                _ap(t1v, 0, [[3 * GK, 2], [GK, 3], [1, GK]]),
                _ap(t2v, 0, [[3 * GK, 2], [GK, 3], [1, GK]]))
            nc.vector.tensor_copy(
                _ap(nn, 3 * GK, [[NVv, 2], [GK, 2], [1, GK]]),
                _ap(nn, 0, [[NVv, 2], [GK, 2], [1, GK]]))
            # m = nIJK x rJK -> NN vec slot 2
            nc.vector.tensor_mul(
                _ap(t1v, 0, [[GK, 3], [1, GK]]),
                _ap(nn, GK, [[GK, 3], [1, GK]]),
                _ap(rv, RVv + 2 * RVc, [[RVc, 3], [1, GK]]))
            nc.vector.tensor_mul(
                _ap(t2v, 0, [[GK, 3], [1, GK]]),
                _ap(nn, 2 * GK, [[GK, 3], [1, GK]]),
                _ap(rv, RVv + RVc, [[RVc, 3], [1, GK]]))
            nc.vector.tensor_sub(
                _ap(nn, 2 * NVv, [[GK, 3], [1, GK]]),
                _ap(t1v, 0, [[GK, 3], [1, GK]]),
                _ap(t2v, 0, [[GK, 3], [1, GK]]))

            # compact pJ copy — only needs PLA, so emit it early to keep
            # the vector engine busy across the stage A -> B boundary
            PJC = pool.tile([P, 3, G, K], f32)
            pjc = PJC[:, :, :, :]
            nc.vector.tensor_copy(_ap(pjc, 0, [[GK, 3], [K, G], [1, K]]),
                                  _ap_cat3(pJ))

            def rvec(v, c):
                return _ap(rv, v * RVv + c * RVc, [[K, G], [1, K]])

            def nvec(v, c):
                return _ap(nn, v * NVv + c * GK, [[K, G], [1, K]])

            rJK = [rvec(1, c) for c in range(3)]
            rjk_off = RVv                      # RV vec 1, comp stride RVc
            m_off = 2 * NVv                    # m lives in NN vec 2

            # paired dot products: one mul+reduce covers two dots that share
            # a left operand; results land adjacently for fused downstream ops
            DOTS = pool.tile([P, 4, GK], f32)  # rows: x0, l1^2, y0, lm^2
            DP4 = pool.tile([P, 2, GK, 3], f32)
            dots = DOTS[:, :, :]
            dp4 = DP4[:, :, :, :]
            # {x0, l1^2} = nIJK . (nJKL, nIJK)
            nc.vector.tensor_mul(
                dp4,
                _ap(nn, 0, [[0, 2], [1, GK], [GK, 3]]),
                _ap(nn, NVv, [[-NVv, 2], [1, GK], [GK, 3]]))
            nc.vector.tensor_reduce(
                _ap(dots, 0, [[GK, 2], [1, GK]]), dp4,
                mybir.AxisListType.X, Alu.add)
            # {y0, lm^2} = m . (nJKL, m)
            nc.vector.tensor_mul(
                dp4,
                _ap(nn, m_off, [[0, 2], [1, GK], [GK, 3]]),
                _ap(nn, NVv, [[NVv, 2], [1, GK], [GK, 3]]))
            nc.vector.tensor_reduce(
                _ap(dots, 2 * GK, [[GK, 2], [1, GK]]), dp4,
                mybir.AxisListType.X, Alu.add)
            jks = dot3v(rv, rjk_off, RVc, rv, rjk_off, RVc)
            L1LM = pool.tile([P, 2, GK], f32)  # (l1, lm)
            l1lm = L1LM[:, :, :]
            nc.scalar.activation(
                _ap(l1lm, 0, [[GK, 2], [1, GK]]),
                _ap(dots, GK, [[2 * GK, 2], [1, GK]]), Act.Sqrt)
            XY = pool.tile([P, 2, GK], f32)    # (x1, y1) = (x0*lm, y0*l1)
            xy = XY[:, :, :]
            nc.vector.tensor_mul(
                xy,
                _ap(dots, 0, [[2 * GK, 2], [1, GK]]),
                _ap(l1lm, GK, [[-GK, 2], [1, GK]]))
            SQ = pool.tile([P, 2, GK], f32)
            sq = SQ[:, :, :]
            nc.vector.tensor_mul(sq, xy, xy)
            hs = T()
            nc.vector.tensor_add(_ap(hs[:, :, :], 0, [[1, GK]]),
                                 _ap(sq, 0, [[1, GK]]),
                                 _ap(sq, GK, [[1, GK]]))
            hr = T(); nc.vector.reciprocal(hr, hs)
            rh = activ(hr, Act.Sqrt)            # 1/hypot
            CS = pool.tile([P, 2, GK], f32)     # (ccur, scur)
            cs_ = CS[:, :, :]
            nc.vector.tensor_mul(
                cs_, xy, _ap(rh[:, :, :], 0, [[0, 2], [1, GK]]))
            jkr = T(); nc.vector.reciprocal(jkr, jks)
            jrs = activ(jkr, Act.Sqrt)          # 1/|rJK|
            AXT = pool.tile([P, 3, G, K], f32)
            axt = AXT[:, :, :, :]
            nc.vector.tensor_mul(
                _ap(axt, 0, [[GK, 3], [1, GK]]),
                _ap(rv, rjk_off, [[RVc, 3], [1, GK]]),
                _ap(jrs[:, :, :], 0, [[0, 3], [1, GK]]),
            )
            ax = [_ap(axt, c * GK, [[K, G], [1, K]]) for c in range(3)]

            # sin/cos of targets with range reduction (Sin table ok |x|<~3.55)
            def reduced_sin(shift_quarter, extra):
                q = aff(vv, 1.0 / TWO_PI, 1024.0 + shift_quarter)
                qi = T(i32)
                nc.vector.tensor_copy(qi, q)     # f32->i32 rounds to nearest
                qf = T()
                nc.vector.tensor_copy(qf, qi)
                t = aff(qf, -TWO_PI, 1024.0 * TWO_PI + extra)
                return activ(add(vv, t), Act.Sin)

            sv = reduced_sin(0.0, 0.0)
            cv = reduced_sin(0.25, _HALF_PI)

            PC1 = pool.tile([P, 2, GK], f32)   # cv * (ccur, scur)
            PC2 = pool.tile([P, 2, GK], f32)   # sv * (ccur, scur)
            pc1 = PC1[:, :, :]
            pc2 = PC2[:, :, :]
            nc.vector.tensor_mul(pc1, cs_, _ap(cv[:, :, :], 0, [[0, 2], [1, GK]]))
            nc.vector.tensor_mul(pc2, cs_, _ap(sv[:, :, :], 0, [[0, 2], [1, GK]]))
            c_ = T()
            s_ = T()
            nc.vector.tensor_add(_ap(c_[:, :, :], 0, [[1, GK]]),
                                 _ap(pc1, 0, [[1, GK]]), _ap(pc2, GK, [[1, GK]]))
            nc.vector.tensor_sub(_ap(s_[:, :, :], 0, [[1, GK]]),
                                 _ap(pc2, 0, [[1, GK]]), _ap(pc1, GK, [[1, GK]]))
            t1_ = T()
            nc.vector.tensor_scalar(t1_, c_, -1.0, 1.0, Alu.mult, Alu.add)  # 1-cos

            TAX = pool.tile([P, 3, G, K], f32)
            SAX = pool.tile([P, 3, G, K], f32)
            UD = pool.tile([P, 3, G, K], f32)
            OD = pool.tile([P, 2, G, K], f32)
            taxv = TAX[:, :, :, :]
            saxv = SAX[:, :, :, :]
            udv = UD[:, :, :, :]
            odv = OD[:, :, :, :]
            d3 = [[GK, 3], [1, GK]]
            bc3 = [[0, 3], [1, GK]]
            nc.vector.tensor_mul(_ap(taxv, 0, d3), _ap(axt, 0, d3),
                                 _ap(t1_[:, :, :], 0, bc3))
            nc.vector.tensor_mul(_ap(saxv, 0, d3), _ap(axt, 0, d3),
                                 _ap(s_[:, :, :], 0, bc3))
            nc.vector.tensor_mul(_ap(udv, 0, d3), _ap(taxv, 0, d3),
                                 _ap(axt, 0, d3))

            def aq(q):
                return _ap(at, q * GK, [[K, G], [1, K]])

            # diagonal: q = 0,5,10 -> stride 5*GK
            nc.vector.tensor_add(
                _ap(at, 0, [[5 * GK, 3], [1, GK]]),
                _ap(udv, 0, d3),
                _ap(c_[:, :, :], 0, bc3),
            )
            # off-diagonal products: txy,txz = tax0*(ax1,ax2); tyz = tax1*ax2
            nc.vector.tensor_mul(
                _ap(odv, 0, [[GK, 2], [1, GK]]),
                _ap(axt, GK, [[GK, 2], [1, GK]]),
                _ap(taxv, 0, [[0, 2], [1, GK]]),
            )
            tyz = T()
            nc.vector.tensor_mul(tyz, _ap(taxv, GK, [[K, G], [1, K]]),
                                 _ap(axt, 2 * GK, [[K, G], [1, K]]))
            txy = _ap(odv, 0, [[K, G], [1, K]])
            txz = _ap(odv, GK, [[K, G], [1, K]])
            sax = [_ap(saxv, c * GK, [[K, G], [1, K]]) for c in range(3)]
            nc.vector.tensor_sub(aq(1), txy, sax[2])
            nc.vector.tensor_add(aq(4), txy, sax[2])
            nc.vector.tensor_add(aq(2), txz, sax[1])
            nc.vector.tensor_sub(aq(8), txz, sax[1])
            nc.vector.tensor_sub(aq(6), tyz, sax[0])
            nc.vector.tensor_add(aq(9), tyz, sax[0])

            # b = pJ - R @ pJ : batched products, reduce, sub (pjc hoisted)
            BP = pool.tile([P, 3, GK, 3], f32)
            bp = BP[:, :, :, :]
            nc.vector.tensor_mul(
                bp,
                _ap(at, 0, [[4 * GK, 3], [1, GK], [GK, 3]]),
                _ap(pjc, 0, [[0, 3], [1, GK], [GK, 3]]),
            )
            RPJ = pool.tile([P, 3, G, K], f32)
            rpj = RPJ[:, :, :, :]
            nc.vector.tensor_reduce(
                _ap(rpj, 0, [[GK, 3], [1, GK]]), bp,
                mybir.AxisListType.X, Alu.add)
            nc.vector.tensor_sub(
                _ap(at, 3 * GK, [[4 * GK, 3], [1, GK]]),
                _ap(pjc, 0, [[GK, 3], [1, GK]]),
                _ap(rpj, 0, [[GK, 3], [1, GK]]),
            )

            # ---- stage B: blocked prefix composition ----
            at_flat = _ap(at, 0, [[GK, 12], [1, GK]])

            def compose(dst, dq, dbd, doff, left, lq, lbd, loff,
                        right, rq, rbd, roff):
                """dst[i,j,*] = sum_m left[i,m,*]*right[m,j,*]; dst[i,3,*] +=
                left[i,3,*].  *bd = batch [step,count] dims (equal counts)."""
                counts = [d[1] for d in dbd]
                assert [d[1] for d in lbd] == counts
                assert [d[1] for d in rbd] == counts
                nb = 1
                for cnt in counts:
                    nb *= cnt
                abd = []
                stp = 1
                for cnt in reversed(counts):
                    abd.insert(0, [stp, cnt])
                    stp *= cnt

                def accv(base):
                    return _ap(base, 0, [[4 * nb, 3], [nb, 4]] + abd)

                dstv = _ap(dst, doff, [[4 * dq, 3], [dq, 4]] + dbd)

                def dmul(tgt, mrow):
                    nc.vector.tensor_mul(
                        accv(tgt),
                        _ap(right, roff + 4 * mrow * rq,
                            [[0, 3], [rq, 4]] + rbd),
                        _ap(left, loff + mrow * lq,
                            [[4 * lq, 3], [0, 4]] + lbd),
                    )

                dmul(acc, 0)
                dmul(ac2, 1)
                nc.vector.tensor_add(accv(acc), accv(acc), accv(ac2))
                dmul(ac2, 2)
                nc.vector.tensor_add(dstv, accv(acc), accv(ac2))
                bias_d = _ap(dst, doff + 3 * dq, [[4 * dq, 3]] + dbd)
                nc.vector.tensor_add(
                    bias_d, bias_d,
                    _ap(left, loff + 3 * lq, [[4 * lq, 3]] + lbd),
                )

            # within-block scan, in place: A[:, t] <- A[:, t-1] o A[:, t]
            # (the 3 muls read the slot before the final add overwrites it)
            for t in range(1, L):
                compose(at_flat, GK, [[L, NB]], t,
                        at_flat, GK, [[L, NB]], t - 1,
                        at_flat, GK, [[L, NB]], t)
            # block products
            nc.vector.tensor_copy(
                _ap(pt, 0, [[NBP, 12], [1, NB]]),
                _ap(at_flat, L - 1, [[GK, 12], [L, NB]]),
            )
            # block-prefix scan: Hillis-Steele over the flattened (g,b) axis.
            # Lanes j%B < s read the neighbour's tail (garbage) and are
            # patched by the trailing copy before the buffers swap.
            src_pt, dst_pt = pt, pt2
            s = 1
            while s < B:
                compose(dst_pt, NBP, [[1, NB]], 0,
                        src_pt, NBP, [[1, NB]], -s,
                        src_pt, NBP, [[1, NB]], 0)
                nc.vector.tensor_copy(
                    _ap(dst_pt, 0, [[NBP, 12], [B, G], [1, s]]),
                    _ap(src_pt, 0, [[NBP, 12], [B, G], [1, s]]))
                src_pt, dst_pt = dst_pt, src_pt
                s *= 2
            ptf = src_pt

            # ---- stage C ----
            def dma_out_cols(a0, ln, ring):
                # split ranges crossing the SP tile boundary
                if a0 < SP and a0 + ln > SP:
                    dma_out_cols(a0, SP - a0, ring)
                    dma_out_cols(SP, a0 + ln - SP, ring)
                    return
                base, mloc, gs, cs = out_view(a0, ln)
                nc.scalar.dma_start(
                    out=_dram_ap(outT[:, :, :, :], a0,
                                 [[3 * G * M, P], [G * M, 3], [M, G], [1, ln]]),
                    in_=_ap(base, mloc, [[cs, 3], [gs, G], [1, ln]]),
                )

            def apply_single_from(coef, coefq, coefoff, m0, length):
                """out[:, :, m0:m0+length] = R@p + b with per-(partition,g)
                scalar coefficients from `coef` (q stride coefq, g stride
                coefoff).  Muls on ACT (per-partition scale), adds on DVE."""
                if m0 < SP and m0 + length > SP:
                    apply_single_from(coef, coefq, coefoff, m0, SP - m0)
                    apply_single_from(coef, coefq, coefoff, SP, m0 + length - SP)
                    return
                plbase, mloc, gs, cs = pl_view(m0, length, None)
                obase, omloc, ogs, ocs = out_view(m0, length)
                tmp_idx[0] += 1
                prod = [[pool.tile([P, G * length], f32,
                                   name=f"prod{tmp_idx[0]}_{i}_{cc}")[:, :]
                         for cc in range(3)] for i in range(3)]
                for i in range(3):
                    for cc in range(3):
                        for g in range(G):
                            nc.scalar.activation(
                                _ap(prod[i][cc], g * length, [[1, length]]),
                                _ap(plbase, cc * cs + g * gs + mloc,
                                    [[1, length]]),
                                Act.Identity,
                                scale=_ap(coef, (4 * i + cc) * coefq
                                          + g * coefoff, [[1, 1]]),
                            )
                for i in range(3):
                    d_t = [[length, G], [1, length]]
                    s1 = _ap(prod[i][0], 0, d_t)
                    nc.vector.tensor_add(s1, s1, _ap(prod[i][1], 0, d_t))
                    nc.vector.tensor_add(s1, s1, _ap(prod[i][2], 0, d_t))
                    for g in range(G):
                        nc.vector.tensor_scalar(
                            _ap(obase, i * ocs + g * ogs + omloc, [[1, length]]),
                            _ap(prod[i][0], g * length, [[1, length]]),
                            _ap(coef, (4 * i + 3) * coefq + g * coefoff, [[1, 1]]),
                            None, Alu.add,
                        )

            pt_last = bass.AP(tensor=ptf.tensor, offset=ptf.offset + (B - 1),
                              ap=list(ptf.ap))

            def apply_runs(starts, length, ks):
                nr = len(starts)
                if nr == 1 and ks[0] == K - 1:
                    # chain-last prefix == last block product: ready right
                    # after the block-prefix scan, before distribute.
                    apply_single_from(pt_last, NBP, B, starts[0], length)
                    return
                if nr == 1:
                    base = bass.AP(tensor=ct.tensor, offset=ct.offset + ks[0],
                                   ap=list(ct.ap))
                    apply_single_from(base, GK, K, starts[0], length)
                    return
                sm = starts[1] - starts[0]
                sk = ks[1] - ks[0]
                m0, k0 = starts[0], ks[0]
                span = max(starts) + length - m0
                plbase, mloc, gs, cs = pl_view(m0, span, None)
                obase, omloc, ogs, ocs = out_view(m0, span)
                d_pl = [[gs, G], [sm, nr], [1, length]]
                d_out = [[ogs, G], [sm, nr], [1, length]]
                d_c = [[K, G], [sk, nr], [0, length]]
                d_acc = [[nr * length, G], [length, nr], [1, length]]
                for i in range(3):
                    for cc in range(2):
                        tgt = acc if cc == 0 else ac2
                        nc.vector.tensor_mul(
                            _ap(tgt, 0, d_acc),
                            _ap(plbase, cc * cs + mloc, d_pl),
                            _ap(ct, (4 * i + cc) * GK + k0, d_c),
                        )
                    nc.vector.tensor_add(
                        _ap(acc, 0, d_acc), _ap(acc, 0, d_acc), _ap(ac2, 0, d_acc)
                    )
                    nc.vector.tensor_mul(
                        _ap(ac2, 0, d_acc),
                        _ap(plbase, 2 * cs + mloc, d_pl),
                        _ap(ct, (4 * i + 2) * GK + k0, d_c),
                    )
                    nc.vector.tensor_add(
                        _ap(acc, 0, d_acc), _ap(acc, 0, d_acc),
                        _ap(ac2, 0, d_acc),
                    )
                    nc.vector.tensor_add(
                        _ap(obase, i * ocs + omloc, d_out),
                        _ap(acc, 0, d_acc),
                        _ap(ct, (4 * i + 3) * GK + k0, d_c),
                    )

            def emit_distribute():
                # distribute: block 0 copies, blocks b>=1 get P[b-1] @ W
                nk = (B - 1) * L
                nc.vector.tensor_copy(
                    _ap(ct, 0, [[GK, 12], [K, G], [1, L]]),
                    _ap(at_flat, 0, [[GK, 12], [K, G], [1, L]]),
                )
                # broadcast block prefixes over t so g fuses into 3-dim APs:
                # PTB[g][q][jb*L + t] = ptf[q][g*B + jb]
                for g in range(G):
                    nc.vector.tensor_copy(
                        _ap(ptb, g * 12 * nk, [[nk, 12], [L, B - 1], [1, L]]),
                        _ap(ptf, g * B, [[NBP, 12], [1, B - 1], [0, L]]))
                d_w = [[GK, 4], [K, G], [1, nk]]
                d_a = [[G * nk, 4], [nk, G], [1, nk]]
                accs = (acc, ac2, ac3)
                # all 9 cross products first (pure reads of W and P), then
                # the combines
                for m in range(3):
                    for i in range(3):
                        nc.vector.tensor_mul(
                            _ap(accs[m], i * 4 * G * nk, d_a),
                            _ap(at_flat, 4 * m * GK + L, d_w),
                            _ap(ptb, (4 * i + m) * nk,
                                [[0, 4], [12 * nk, G], [1, nk]]),
                        )
                for i in range(3):
                    o = i * 4 * G * nk
                    nc.vector.tensor_add(
                        _ap(acc, o, d_a), _ap(acc, o, d_a), _ap(ac2, o, d_a))
                    nc.vector.tensor_add(
                        _ap(ct, 4 * i * GK + L, d_w),
                        _ap(acc, o, d_a), _ap(ac3, o, d_a))
                    nc.vector.tensor_add(
                        _ap(ct, (4 * i + 3) * GK + L, [[K, G], [1, nk]]),
                        _ap(ct, (4 * i + 3) * GK + L, [[K, G], [1, nk]]),
                        _ap(ptb, (4 * i + 3) * nk, [[12 * nk, G], [1, nk]]),
                    )

            # unmoved atoms: copy + DMA as soon as PL lands
            unmoved = [m for m in range(M) if km[m] < 0]
            u0 = 0
            while u0 < len(unmoved):
                u1 = u0
                while u1 + 1 < len(unmoved) and unmoved[u1 + 1] == unmoved[u1] + 1:
                    u1 += 1
                a0, ln = unmoved[u0], u1 - u0 + 1
                assert a0 + ln <= SP or a0 >= SP
                ubase, umloc, ugs, ucs = pl_view(a0, ln, None)
                uobase, uomloc, uogs, uocs = out_view(a0, ln)
                nc.vector.tensor_copy(
                    _ap(uobase, uomloc, [[uocs, 3], [uogs, G], [1, ln]]),
                    _ap(ubase, umloc, [[ucs, 3], [ugs, G], [1, ln]]),
                )
                dma_out_cols(a0, ln, 0)
                u0 = u1 + 1

            # classes: chain-last single-run first (overlaps distribute)
            by_len = {}
            for (m0, ln, k) in runs:
                by_len.setdefault(ln, []).append((m0, k))
            classes = sorted(
                by_len.items(),
                key=lambda kv: 0 if (len(kv[1]) == 1 and kv[1][0][1] == K - 1)
                else 1)
            emitted_distribute = False
            ring = 1
            for ln, rs in classes:
                starts = [r[0] for r in rs]
                ks = [r[1] for r in rs]
                nr = len(rs)
                chain_last_single = nr == 1 and ks[0] == K - 1
                if not chain_last_single and not emitted_distribute:
                    emit_distribute()
                    emitted_distribute = True
                regular = nr <= 2 or (
                    all(starts[r] == starts[0] + r * (starts[1] - starts[0])
                        for r in range(nr))
                    and all(ks[r] == ks[0] + r * (ks[1] - ks[0])
                            for r in range(nr))
                )
                if regular and nr >= 4:
                    # skewed halves: the later chunk is smaller so the final
                    # exposed output DMA is short
                    h = (nr * 3) // 4
                    apply_runs(starts[:h], ln, ks[:h])
                    lo = min(starts[:h]); hi = max(s + ln for s in starts[:h])
                    dma_out_cols(lo, hi - lo, ring); ring ^= 1
                    apply_runs(starts[h:], ln, ks[h:])
                    lo = min(starts[h:]); hi = max(s + ln for s in starts[h:])
                    dma_out_cols(lo, hi - lo, ring); ring ^= 1
                    continue
                if regular:
                    apply_runs(starts, ln, ks)
                else:
                    for (m0, k) in rs:
                        apply_runs([m0], ln, [k])
                lo = min(starts)
                hi = max(s + ln for s in starts)
                dma_out_cols(lo, hi - lo, ring)
                ring ^= 1

    _split_multi_waits(nc)
    return nc


_BUILD_CACHE = {}


def make_in_maps(input, pos, angles):
    input = np.asarray(input, dtype=np.float32)
    pos = np.asarray(pos, dtype=np.float32)
    N, K = input.shape
    M = pos.shape[1]
    NL = N // NCORES
    G = NL // P
    SP = min(int(np.asarray(angles).max()) + 1, M)
    in_maps = []
    for c in range(NCORES):
        sl = slice(c * NL, (c + 1) * NL)
        # (NL, M, 3) -> (P, 3, G, M): partition-major so each partition row
        # is one contiguous DMA descriptor
        pm = pos[sl].reshape(G, P, M, 3).transpose(1, 3, 0, 2)
        vrows = input[sl].reshape(G, P, K).transpose(1, 0, 2).reshape(P, G * K)
        arows = pm[:, :, :, :SP].reshape(P, 3 * G * SP)
        im = {"catA": np.ascontiguousarray(
            np.concatenate([vrows, arows], axis=1))}
        if SP < M:
            im["posB"] = np.ascontiguousarray(pm[:, :, :, SP:])
        in_maps.append(im)
    return in_maps


def kernel(input, pos, angles, move_mask):
    input = np.ascontiguousarray(np.asarray(input, dtype=np.float32))
    pos = np.ascontiguousarray(np.asarray(pos, dtype=np.float32))
    angles = np.asarray(angles)
    move_mask = np.asarray(move_mask).astype(bool)

    N, K = input.shape
    _, M, three = pos.shape
    assert three == 3
    assert N % (NCORES * P) == 0
    NL = N // NCORES

    key = (N, K, M, angles.tobytes(), move_mask.tobytes())
    nc = _BUILD_CACHE.get(key)
    if nc is None:
        nc = _build(angles, move_mask, NL, K, M)
        _BUILD_CACHE[key] = nc

    in_maps = make_in_maps(input, pos, angles)

    # the axon-proxied NRT occasionally wedges transiently
    # (NRT_EXEC_UNIT_UNRECOVERABLE); one retry recovers it
    try:
        res = run_bass_kernel_spmd(nc, in_maps, list(range(NCORES)))
    except Exception:
        res = run_bass_kernel_spmd(nc, in_maps, list(range(NCORES)))

    out = np.empty((N, M, 3), dtype=np.float32)
    for c in range(NCORES):
        sl = slice(c * NL, (c + 1) * NL)
        o = res.results[c]["outT"]           # (P, 3, G, M)
        out[sl] = o.transpose(2, 0, 3, 1).reshape(NL, M, 3)
    return out



# revision 48
# speedup vs baseline: 1.0447x; 1.0447x over previous
"""Dihedral2Coord Trainium2 kernel.

Math: the reference applies K sequential dihedral-set steps; step k rotates
a suffix of the atom chain rigidly about the current J-K bond.  Every step's
transform is a proper rigid motion that moves all four pivot atoms of every
later step together, so the dihedral measured at application time equals the
dihedral of the ORIGINAL coordinates (dihedrals are invariant under rigid
motion).  Step k's rotation, expressed in original coordinates, is therefore
a fixed affine A_k computable from the original positions alone, and the
cumulative transform is the prefix product C_k = A_0 @ A_1 @ ... @ A_k.
The kernel:
  A) computes all K per-conformer Rodrigues affines in parallel,
  B) prefix-composes them with a blocked scan,
  C) applies C_{km(m)} to each atom run, where km(m) counts the steps whose
     mask includes atom m (verified prefix-structured on host).

Sharding: pure data parallelism over conformers N across 8 cores (SPMD).
"""

import sys

import numpy as np

try:
    import concourse.bass as bass
except ImportError:  # path in the grading container
    sys.path.insert(0, "/opt/trn_rl_repo")
    import concourse.bass as bass

import concourse.tile as tile
from concourse import mybir
from concourse.bass_utils import run_bass_kernel_spmd

f32 = mybir.dt.float32
i32 = mybir.dt.int32
Alu = mybir.AluOpType
Act = mybir.ActivationFunctionType

NCORES = 8
P = 128
TWO_PI = float(2.0 * np.pi)
_HALF_PI = float(np.pi / 2)

_WAIT_CAP = 1  # this walrus build rejects >1 sync-wait per instruction


def _register_const(nc, value, dtype=f32):
    """Register an activation-bias constant.  Written on the Activation
    engine from the framework's const-1.0 AP (ordered by Bass.__init__'s
    barrier); later ACT reads are same-engine program-ordered, so no extra
    barrier is needed."""
    if (dtype, value) in nc.const_aps.aps:
        return
    t = nc.alloc_sbuf_tensor(f"const-{dtype.name}-{value}", [128, 1], dtype)
    one = nc.const_aps.aps[(f32, 1.0)]
    nc.scalar.activation(t.ap(), one, Act.Identity, bias=0.0, scale=float(value))
    nc.const_aps.aps[(dtype, value)] = t.ap()


def _split_multi_waits(nc):
    """Split every instruction carrying >cap sync-waits into single-wait
    NoOps (same engine, immediately before, same block).  Waits are monotone
    semaphore conditions so this preserves semantics exactly."""
    n = 0
    for func in nc.m.functions:
        for bb in func.blocks:
            old = list(bb.instructions)
            if not any(
                i.sync_info is not None and len(i.sync_info.on_wait) > _WAIT_CAP
                for i in old
            ):
                continue
            new = []
            for inst in old:
                si = inst.sync_info
                if si is not None and len(si.on_wait) > _WAIT_CAP:
                    waits = list(si.on_wait)
                    head, tail = waits[:-_WAIT_CAP], waits[-_WAIT_CAP:]
                    for j in range(0, len(head), _WAIT_CAP):
                        n += 1
                        new.append(
                            mybir.InstNoOp(
                                name=f"{inst.name}_ws{j}",
                                engine=inst.engine,
                                sync_info=mybir.SyncInfo(
                                    on_wait=list(head[j : j + _WAIT_CAP]), on_update=[]
                                ),
                                bass_nofuse=True,
                            )
                        )
                    try:
                        si.on_wait[:] = tail
                    except TypeError:
                        inst.sync_info = mybir.SyncInfo(
                            on_wait=tail, on_update=list(si.on_update)
                        )
                new.append(inst)
            try:
                bb.instructions[:] = new
            except TypeError:
                bb.instructions = new
    return n


def _ap(base, offset_elems, dims):
    """Free-dim AP view into an SBUF tile AP `base` (partition dim kept).
    dims: list of [step, count] in elements of the tile's free space."""
    return bass.AP(
        tensor=base.tensor,
        offset=base.offset + offset_elems,
        ap=[list(base.ap[0])] + [list(d) for d in dims],
    )


def _dram_ap(t, offset, dims):
    return bass.AP(tensor=t.tensor, offset=offset, ap=[list(d) for d in dims])


def _analyse_mask(angles, move_mask):
    """Host-side structural analysis. Returns (km, runs): km[m] is the last
    step applied to atom m (-1 = never moved); runs are (start, len, k)."""
    K, M = move_mask.shape
    km = move_mask.astype(np.int64).sum(0) - 1
    kk = np.arange(K)[:, None]
    if not (move_mask == (kk <= km[None, :])).all():
        raise NotImplementedError("move_mask is not prefix-structured per atom")
    for k in range(K):
        for a in angles[k]:
            if not move_mask[:k, a].all():
                raise NotImplementedError("pivot atoms not rigidly co-moved")
    runs = []
    m = 0
    while m < M:
        j = m
        while j + 1 < M and km[j + 1] == km[m]:
            j += 1
        if km[m] >= 0:
            runs.append((m, j - m + 1, int(km[m])))
        m = j + 1
    return km, runs


def _seg_bounds(angles, move_mask, M):
    """(SP, SPB): pivot region [0, SP); B-tile starts at SPB <= SP so no
    run/unmoved segment crosses a tile boundary (columns [SPB, SP) are
    duplicated in both tiles)."""
    km, runs = _analyse_mask(angles, move_mask)
    SP = min(int(np.asarray(angles).max()) + 1, M)
    segs = [(m0, ln) for (m0, ln, _k) in runs]
    m = 0
    while m < M:
        if km[m] < 0:
            j = m
            while j + 1 < M and km[j + 1] < 0:
                j += 1
            segs.append((m, j - m + 1))
            m = j + 1
        else:
            m += 1
    SPB = SP
    for (m0, ln) in segs:
        if m0 < SP < m0 + ln:
            SPB = min(SPB, m0)
    return SP, SPB


def _build(angles, move_mask, NL, K, M):
    """Build the Bass module for one core handling NL conformers."""
    G = NL // P
    assert NL == G * P
    GK = G * K
    L = 8               # within-block scan length
    assert K % L == 0
    B = K // L          # blocks per conformer-group
    NB = G * B          # blocks over the flattened (g,k) axis

    angles = np.asarray(angles)
    arange_quads = bool((angles == np.arange(K * 4).reshape(K, 4)).all())
    km, runs = _analyse_mask(angles, move_mask)

    nc = bass.Bass()
    for cval in (1024.0, 1024.25, 1024.0 * TWO_PI, 1024.0 * TWO_PI + _HALF_PI):
        _register_const(nc, float(cval))
    SP, SPB = _seg_bounds(angles, move_mask, M)
    MB = M - SPB        # B-tile width
    vinD = nc.declare_dram_parameter("vin", [P, G * K], f32, isOutput=False)
    pivA = nc.declare_dram_parameter("pivA", [P, 3 * G * SP], f32,
                                     isOutput=False)
    # quad-permuted pivot planes: pivP[c][q][g][k] = pos[., 4k+q, c] so the
    # r-vector subtraction reads contiguously (innermost (g,k))
    pivPd = (nc.declare_dram_parameter("pivP", [P, 12 * G * K], f32,
                                       isOutput=False) if arange_quads
             else None)
    posB = (nc.declare_dram_parameter("posB", [P, 3, G, MB], f32,
                                      isOutput=False) if SPB < M else None)
    outT = nc.declare_dram_parameter("outT", [P, 3, G, M], f32, isOutput=True)

    with tile.TileContext(nc) as tc:
        with tc.tile_pool(name="main", bufs=1) as pool:
            # ---- SBUF tensors ----
            # separate tiles per DMA so consumers wait only on what they need
            VIN = pool.tile([P, G * K], f32)
            PLA = pool.tile([P, 3 * G * SP], f32)
            PIVP = (pool.tile([P, 3, 4, GK], f32, name="PIVP")
                    if arange_quads else None)
            PLB = pool.tile([P, 3, G, MB], f32, name="PLB") if SPB < M else None
            OUTA = pool.tile([P, 3, G, SP], f32)
            OUTB = pool.tile([P, 3, G, MB], f32, name="OUTB") if SPB < M else None
            # packed r-vectors / normals with duplicated xy components so a
            # +1/+2 component rotation is a plain offset (cross-product trick)
            RV = pool.tile([P, 3, 5, G, K], f32)  # (rIJ,rJK,rKL) x (x,y,z,x,y)
            NN = pool.tile([P, 3, 5, G, K], f32)  # (nIJK,nJKL,m) x (x,y,z,x,y)
            TA = pool.tile([P, 2, 3, G, K], f32)
            TB = pool.tile([P, 2, 3, G, K], f32)
            AT = pool.tile([P, 12, G, K], f32)   # A_k; q=4i+j, strides q:GK, g:K, k:1
            CT = pool.tile([P, 12, GK], f32)     # full prefixes
            NBP = NB + 4   # 4 pad columns so Hillis j<s lanes read in-bounds
            PT = pool.tile([P, 12, NBP], f32)    # block products / prefixes
            PT2 = pool.tile([P, 12, NBP], f32)   # Hillis ping-pong buffer
            PTB = pool.tile([P, G, 12, (K // 8) * 7], f32)  # prefixes bcast over t
            ACC = pool.tile([P, 12 * max(GK, 64)], f32)
            AC2 = pool.tile([P, 12 * max(GK, 64)], f32)
            AC3 = pool.tile([P, 12 * max(GK, 64)], f32)

            vv = _ap(VIN[:, :], 0, [[K, G], [1, K]])
            pla = _ap(PLA[:, :], 0, [])
            pivp = PIVP[:, :, :, :] if PIVP is not None else None
            plb = PLB[:, :, :, :] if PLB is not None else None
            outa = OUTA[:, :, :, :]
            outb = OUTB[:, :, :, :] if OUTB is not None else None

            def pl_view(m0, ln, _unused=None):
                """(base_ap, local column offset, group stride, comp stride)
                for columns [m0, m0+ln): B tile from SPB, else A tile."""
                if m0 >= SPB:
                    return plb, m0 - SPB, MB, G * MB
                assert m0 + ln <= SP
                return pla, m0, SP, G * SP

            def out_view(m0, ln):
                if m0 >= SPB:
                    return outb, m0 - SPB, MB, G * MB
                assert m0 + ln <= SP
                return outa, m0, SP, G * SP
            rv = RV[:, :, :, :, :]
            nn = NN[:, :, :, :, :]
            t1v = TA[:, :, :, :, :]
            t2v = TB[:, :, :, :, :]
            at = AT[:, :, :, :]
            ct = CT[:, :, :]
            pt = _ap(PT[:, :, :], 4, [[NBP, 12], [1, NB]])
            pt2 = _ap(PT2[:, :, :], 4, [[NBP, 12], [1, NB]])
            ptb = PTB[:, :, :, :]
            acc = ACC[:, :]
            ac2 = AC2[:, :]
            ac3 = AC3[:, :]

            RVv, RVc = 5 * GK, GK   # RV strides: vec, comp
            NVv = 5 * GK

            # ---- DMA in ----
            # All on the sync ring, in priority order: pivP (gates stage A),
            # vin (gates the ACT sin chain), pivA (gates pJ copy + A-apply),
            # posB (gates the B-tile apply, late).  Host arrays are
            # partition-major so each row is one contiguous descriptor.
            if pivp is not None:
                nc.sync.dma_start(
                    out=_ap(pivp, 0, [[1, 12 * GK]]),
                    in_=_dram_ap(pivPd[:, :], 0, [[12 * GK, P], [1, 12 * GK]]),
                )
            nc.sync.dma_start(
                out=_ap(vv, 0, [[1, GK]]),
                in_=_dram_ap(vinD[:, :], 0, [[GK, P], [1, GK]]),
            )
            nc.sync.dma_start(
                out=_ap(pla, 0, [[1, 3 * G * SP]]),
                in_=_dram_ap(pivA[:, :], 0, [[3 * G * SP, P], [1, 3 * G * SP]]),
            )
            if PLB is not None:
                nc.sync.dma_start(
                    out=_ap(plb, 0, [[1, 3 * G * MB]]),
                    in_=_dram_ap(posB[:, :, :, :], 0,
                                 [[3 * G * MB, P], [1, 3 * G * MB]]),
                )
            # Hillis pad columns must hold finite values (they feed the
            # patched lanes); zero them before the block-prefix scan
            nc.gpsimd.memset(_ap(PT[:, :, :], 0, [[NBP, 12], [1, 4]]), 0.0)
            nc.gpsimd.memset(_ap(PT2[:, :, :], 0, [[NBP, 12], [1, 4]]), 0.0)

            # ---- helpers ----
            tmp_idx = [0]

            def T(dt=f32):
                tmp_idx[0] += 1
                return pool.tile([P, G, K], dt, name=f"tmp{tmp_idx[0]}")

            def mul(a, b):
                o = T(); nc.vector.tensor_mul(o, a, b); return o

            def add(a, b):
                o = T(); nc.vector.tensor_add(o, a, b); return o

            def sub(a, b):
                o = T(); nc.vector.tensor_sub(o, a, b); return o

            def aff(a, scale, bias):
                o = T()
                nc.scalar.activation(o, a, Act.Identity, bias=bias, scale=scale)
                return o

            def activ(a, fn):
                o = T(); nc.scalar.activation(o, a, fn); return o

            def dot3v(a_base, a_off, a_cs, b_base, b_off, b_cs, eng=None):
                """dot over xyz comps via one mul + one innermost-reduce.
                a/b given as (tile_ap, elem offset, comp stride); both must
                have gk contiguous (stride 1)."""
                tmp_idx[0] += 1
                dp = pool.tile([P, GK, 3], f32, name=f"dp{tmp_idx[0]}")[:, :, :]
                (eng or nc.vector).tensor_mul(
                    dp,
                    _ap(a_base, a_off, [[1, GK], [a_cs, 3]]),
                    _ap(b_base, b_off, [[1, GK], [b_cs, 3]]),
                )
                o = T()
                nc.vector.tensor_reduce(
                    _ap(o, 0, [[1, GK]]), dp, mybir.AxisListType.X, Alu.add)
                return o

            # ---- pivot sources ----
            if not arange_quads:
                PIV = pool.tile([P, 3, G, 4, K], f32)
                pv = PIV[:, :, :, :, :]
                for k in range(K):
                    for q in range(4):
                        nc.vector.tensor_copy(
                            _ap(pv, q * K + k, [[G * 4 * K, 3], [4 * K, G]]),
                            _ap(pla, int(angles[k, q]),
                                [[G * SP, 3], [SP, G]]),
                        )

            def piv_ap(c, q):
                if arange_quads:
                    return _ap(pivp, c * 4 * GK + q * GK, [[K, G], [1, K]])
                return _ap(pv, c * G * 4 * K + q * K, [[4 * K, G], [1, K]])

            pJ = [piv_ap(c, 1) for c in range(3)]

            def _ap_cat3(_pj):
                # the three pJ views share a regular comp stride; rebuild as
                # one 3-dim AP [c][g][k]
                if arange_quads:
                    return _ap(pivp, GK, [[4 * GK, 3], [1, GK]])
                return _ap(pv, K, [[G * 4 * K, 3], [4 * K, G], [1, K]])

            # ---- stage A: packed r-vectors and cross products ----
            if arange_quads:
                # quad-permuted pivots: v-dim is the q axis, (g,k) contiguous
                nc.vector.tensor_sub(
                    _ap(rv, 0, [[RVv, 3], [RVc, 3], [1, GK]]),
                    _ap(pivp, GK, [[GK, 3], [4 * GK, 3], [1, GK]]),
                    _ap(pivp, 0, [[GK, 3], [4 * GK, 3], [1, GK]]))
            else:
                for g in range(G):
                    in1 = _ap(pv, g * 4 * K + K,
                              [[K, 3], [G * 4 * K, 3], [1, K]])
                    in0 = _ap(pv, g * 4 * K + 0,
                              [[K, 3], [G * 4 * K, 3], [1, K]])
                    nc.vector.tensor_sub(
                        _ap(rv, g * K, [[RVv, 3], [RVc, 3], [1, K]]), in1, in0)
            # duplicate comps x,y into slots 3,4
            nc.vector.tensor_copy(
                _ap(rv, 3 * RVc, [[RVv, 3], [RVc, 2], [1, GK]]),
                _ap(rv, 0, [[RVv, 3], [RVc, 2], [1, GK]]))
            # nIJK, nJKL = cross(A=[rIJ,rJK], B=[rJK,rKL]) via comp offsets
            nc.vector.tensor_mul(
                _ap(t1v, 0, [[3 * GK, 2], [GK, 3], [1, GK]]),
                _ap(rv, RVc, [[RVv, 2], [RVc, 3], [1, GK]]),
                _ap(rv, RVv + 2 * RVc, [[RVv, 2], [RVc, 3], [1, GK]]))
            nc.vector.tensor_mul(
                _ap(t2v, 0, [[3 * GK, 2], [GK, 3], [1, GK]]),
                _ap(rv, 2 * RVc, [[RVv, 2], [RVc, 3], [1, GK]]),
                _ap(rv, RVv + RVc, [[RVv, 2], [RVc, 3], [1, GK]]))
            nc.vector.tensor_sub(
                _ap(nn, 0, [[NVv, 2], [GK, 3], [1, GK]]),
                _ap(t1v, 0, [[3 * GK, 2], [GK, 3], [1, GK]]),
                _ap(t2v, 0, [[3 * GK, 2], [GK, 3], [1, GK]]))
            nc.vector.tensor_copy(
                _ap(nn, 3 * GK, [[NVv, 2], [GK, 2], [1, GK]]),
                _ap(nn, 0, [[NVv, 2], [GK, 2], [1, GK]]))
            # m = nIJK x rJK -> NN vec slot 2
            nc.vector.tensor_mul(
                _ap(t1v, 0, [[GK, 3], [1, GK]]),
                _ap(nn, GK, [[GK, 3], [1, GK]]),
                _ap(rv, RVv + 2 * RVc, [[RVc, 3], [1, GK]]))
            nc.vector.tensor_mul(
                _ap(t2v, 0, [[GK, 3], [1, GK]]),
                _ap(nn, 2 * GK, [[GK, 3], [1, GK]]),
                _ap(rv, RVv + RVc, [[RVc, 3], [1, GK]]))
            nc.vector.tensor_sub(
                _ap(nn, 2 * NVv, [[GK, 3], [1, GK]]),
                _ap(t1v, 0, [[GK, 3], [1, GK]]),
                _ap(t2v, 0, [[GK, 3], [1, GK]]))

            # compact pJ copy — only needs PLA, so emit it early to keep
            # the vector engine busy across the stage A -> B boundary
            PJC = pool.tile([P, 3, G, K], f32)
            pjc = PJC[:, :, :, :]
            nc.vector.tensor_copy(_ap(pjc, 0, [[GK, 3], [K, G], [1, K]]),
                                  _ap_cat3(pJ))

            def rvec(v, c):
                return _ap(rv, v * RVv + c * RVc, [[K, G], [1, K]])

            def nvec(v, c):
                return _ap(nn, v * NVv + c * GK, [[K, G], [1, K]])

            rJK = [rvec(1, c) for c in range(3)]
            rjk_off = RVv                      # RV vec 1, comp stride RVc
            m_off = 2 * NVv                    # m lives in NN vec 2

            # paired dot products: one mul+reduce covers two dots that share
            # a left operand; results land adjacently for fused downstream ops
            DOTS = pool.tile([P, 4, GK], f32)  # rows: x0, l1^2, y0, lm^2
            DP4 = pool.tile([P, 2, GK, 3], f32)
            dots = DOTS[:, :, :]
            dp4 = DP4[:, :, :, :]
            # {x0, l1^2} = nIJK . (nJKL, nIJK)
            nc.vector.tensor_mul(
                dp4,
                _ap(nn, 0, [[0, 2], [1, GK], [GK, 3]]),
                _ap(nn, NVv, [[-NVv, 2], [1, GK], [GK, 3]]))
            nc.vector.tensor_reduce(
                _ap(dots, 0, [[GK, 2], [1, GK]]), dp4,
                mybir.AxisListType.X, Alu.add)
            # {y0, lm^2} = m . (nJKL, m)
            nc.vector.tensor_mul(
                dp4,
                _ap(nn, m_off, [[0, 2], [1, GK], [GK, 3]]),
                _ap(nn, NVv, [[NVv, 2], [1, GK], [GK, 3]]))
            nc.vector.tensor_reduce(
                _ap(dots, 2 * GK, [[GK, 2], [1, GK]]), dp4,
                mybir.AxisListType.X, Alu.add)
            jks = dot3v(rv, rjk_off, RVc, rv, rjk_off, RVc)
            L1LM = pool.tile([P, 2, GK], f32)  # (l1, lm)
            l1lm = L1LM[:, :, :]
            nc.scalar.activation(
                _ap(l1lm, 0, [[GK, 2], [1, GK]]),
                _ap(dots, GK, [[2 * GK, 2], [1, GK]]), Act.Sqrt)
            XY = pool.tile([P, 2, GK], f32)    # (x1, y1) = (x0*lm, y0*l1)
            xy = XY[:, :, :]
            nc.vector.tensor_mul(
                xy,
                _ap(dots, 0, [[2 * GK, 2], [1, GK]]),
                _ap(l1lm, GK, [[-GK, 2], [1, GK]]))
            SQ = pool.tile([P, 2, GK], f32)
            sq = SQ[:, :, :]
            nc.vector.tensor_mul(sq, xy, xy)
            hs = T()
            nc.vector.tensor_add(_ap(hs[:, :, :], 0, [[1, GK]]),
                                 _ap(sq, 0, [[1, GK]]),
                                 _ap(sq, GK, [[1, GK]]))
            hr = T(); nc.vector.reciprocal(hr, hs)
            rh = activ(hr, Act.Sqrt)            # 1/hypot
            CS = pool.tile([P, 2, GK], f32)     # (ccur, scur)
            cs_ = CS[:, :, :]
            nc.vector.tensor_mul(
                cs_, xy, _ap(rh[:, :, :], 0, [[0, 2], [1, GK]]))
            jkr = T(); nc.vector.reciprocal(jkr, jks)
            jrs = activ(jkr, Act.Sqrt)          # 1/|rJK|
            AXT = pool.tile([P, 3, G, K], f32)
            axt = AXT[:, :, :, :]
            nc.vector.tensor_mul(
                _ap(axt, 0, [[GK, 3], [1, GK]]),
                _ap(rv, rjk_off, [[RVc, 3], [1, GK]]),
                _ap(jrs[:, :, :], 0, [[0, 3], [1, GK]]),
            )
            ax = [_ap(axt, c * GK, [[K, G], [1, K]]) for c in range(3)]

            # sin/cos of targets with range reduction (Sin table ok |x|<~3.55)
            def reduced_sin(shift_quarter, extra):
                q = aff(vv, 1.0 / TWO_PI, 1024.0 + shift_quarter)
                qi = T(i32)
                nc.vector.tensor_copy(qi, q)     # f32->i32 rounds to nearest
                qf = T()
                nc.vector.tensor_copy(qf, qi)
                t = aff(qf, -TWO_PI, 1024.0 * TWO_PI + extra)
                return activ(add(vv, t), Act.Sin)

            sv = reduced_sin(0.0, 0.0)
            cv = reduced_sin(0.25, _HALF_PI)

            PC1 = pool.tile([P, 2, GK], f32)   # cv * (ccur, scur)
            PC2 = pool.tile([P, 2, GK], f32)   # sv * (ccur, scur)
            pc1 = PC1[:, :, :]
            pc2 = PC2[:, :, :]
            nc.vector.tensor_mul(pc1, cs_, _ap(cv[:, :, :], 0, [[0, 2], [1, GK]]))
            nc.vector.tensor_mul(pc2, cs_, _ap(sv[:, :, :], 0, [[0, 2], [1, GK]]))
            c_ = T()
            s_ = T()
            nc.vector.tensor_add(_ap(c_[:, :, :], 0, [[1, GK]]),
                                 _ap(pc1, 0, [[1, GK]]), _ap(pc2, GK, [[1, GK]]))
            nc.vector.tensor_sub(_ap(s_[:, :, :], 0, [[1, GK]]),
                                 _ap(pc2, 0, [[1, GK]]), _ap(pc1, GK, [[1, GK]]))
            t1_ = T()
            nc.vector.tensor_scalar(t1_, c_, -1.0, 1.0, Alu.mult, Alu.add)  # 1-cos

            TAX = pool.tile([P, 3, G, K], f32)
            SAX = pool.tile([P, 3, G, K], f32)
            UD = pool.tile([P, 3, G, K], f32)
            OD = pool.tile([P, 2, G, K], f32)
            taxv = TAX[:, :, :, :]
            saxv = SAX[:, :, :, :]
            udv = UD[:, :, :, :]
            odv = OD[:, :, :, :]
            d3 = [[GK, 3], [1, GK]]
            bc3 = [[0, 3], [1, GK]]
            nc.vector.tensor_mul(_ap(taxv, 0, d3), _ap(axt, 0, d3),
                                 _ap(t1_[:, :, :], 0, bc3))
            nc.vector.tensor_mul(_ap(saxv, 0, d3), _ap(axt, 0, d3),
                                 _ap(s_[:, :, :], 0, bc3))
            nc.vector.tensor_mul(_ap(udv, 0, d3), _ap(taxv, 0, d3),
                                 _ap(axt, 0, d3))

            def aq(q):
                return _ap(at, q * GK, [[K, G], [1, K]])

            # diagonal: q = 0,5,10 -> stride 5*GK
            nc.vector.tensor_add(
                _ap(at, 0, [[5 * GK, 3], [1, GK]]),
                _ap(udv, 0, d3),
                _ap(c_[:, :, :], 0, bc3),
            )
            # off-diagonal products: txy,txz = tax0*(ax1,ax2); tyz = tax1*ax2
            nc.vector.tensor_mul(
                _ap(odv, 0, [[GK, 2], [1, GK]]),
                _ap(axt, GK, [[GK, 2], [1, GK]]),
                _ap(taxv, 0, [[0, 2], [1, GK]]),
            )
            tyz = T()
            nc.vector.tensor_mul(tyz, _ap(taxv, GK, [[K, G], [1, K]]),
                                 _ap(axt, 2 * GK, [[K, G], [1, K]]))
            txy = _ap(odv, 0, [[K, G], [1, K]])
            txz = _ap(odv, GK, [[K, G], [1, K]])
            sax = [_ap(saxv, c * GK, [[K, G], [1, K]]) for c in range(3)]
            nc.vector.tensor_sub(aq(1), txy, sax[2])
            nc.vector.tensor_add(aq(4), txy, sax[2])
            nc.vector.tensor_add(aq(2), txz, sax[1])
            nc.vector.tensor_sub(aq(8), txz, sax[1])
            nc.vector.tensor_sub(aq(6), tyz, sax[0])
            nc.vector.tensor_add(aq(9), tyz, sax[0])

            # b = pJ - R @ pJ : batched products, reduce, sub (pjc hoisted)
            BP = pool.tile([P, 3, GK, 3], f32)
            bp = BP[:, :, :, :]
            nc.vector.tensor_mul(
                bp,
                _ap(at, 0, [[4 * GK, 3], [1, GK], [GK, 3]]),
                _ap(pjc, 0, [[0, 3], [1, GK], [GK, 3]]),
            )
            RPJ = pool.tile([P, 3, G, K], f32)
            rpj = RPJ[:, :, :, :]
            nc.vector.tensor_reduce(
                _ap(rpj, 0, [[GK, 3], [1, GK]]), bp,
                mybir.AxisListType.X, Alu.add)
            nc.vector.tensor_sub(
                _ap(at, 3 * GK, [[4 * GK, 3], [1, GK]]),
                _ap(pjc, 0, [[GK, 3], [1, GK]]),
                _ap(rpj, 0, [[GK, 3], [1, GK]]),
            )

            # ---- stage B: blocked prefix composition ----
            at_flat = _ap(at, 0, [[GK, 12], [1, GK]])

            def compose(dst, dq, dbd, doff, left, lq, lbd, loff,
                        right, rq, rbd, roff):
                """dst[i,j,*] = sum_m left[i,m,*]*right[m,j,*]; dst[i,3,*] +=
                left[i,3,*].  *bd = batch [step,count] dims (equal counts)."""
                counts = [d[1] for d in dbd]
                assert [d[1] for d in lbd] == counts
                assert [d[1] for d in rbd] == counts
                nb = 1
                for cnt in counts:
                    nb *= cnt
                abd = []
                stp = 1
                for cnt in reversed(counts):
                    abd.insert(0, [stp, cnt])
                    stp *= cnt

                def accv(base):
                    return _ap(base, 0, [[4 * nb, 3], [nb, 4]] + abd)

                dstv = _ap(dst, doff, [[4 * dq, 3], [dq, 4]] + dbd)

                def dmul(tgt, mrow):
                    nc.vector.tensor_mul(
                        accv(tgt),
                        _ap(right, roff + 4 * mrow * rq,
                            [[0, 3], [rq, 4]] + rbd),
                        _ap(left, loff + mrow * lq,
                            [[4 * lq, 3], [0, 4]] + lbd),
                    )

                dmul(acc, 0)
                dmul(ac2, 1)
                nc.vector.tensor_add(accv(acc), accv(acc), accv(ac2))
                dmul(ac2, 2)
                nc.vector.tensor_add(dstv, accv(acc), accv(ac2))
                bias_d = _ap(dst, doff + 3 * dq, [[4 * dq, 3]] + dbd)
                nc.vector.tensor_add(
                    bias_d, bias_d,
                    _ap(left, loff + 3 * lq, [[4 * lq, 3]] + lbd),
                )

            # within-block scan, in place: A[:, t] <- A[:, t-1] o A[:, t]
            # (the 3 muls read the slot before the final add overwrites it)
            for t in range(1, L):
                compose(at_flat, GK, [[L, NB]], t,
                        at_flat, GK, [[L, NB]], t - 1,
                        at_flat, GK, [[L, NB]], t)
            # block products
            nc.vector.tensor_copy(
                _ap(pt, 0, [[NBP, 12], [1, NB]]),
                _ap(at_flat, L - 1, [[GK, 12], [L, NB]]),
            )
            # block-prefix scan: Hillis-Steele over the flattened (g,b) axis.
            # Lanes j%B < s read the neighbour's tail (garbage) and are
            # patched by the trailing copy before the buffers swap.
            src_pt, dst_pt = pt, pt2
            s = 1
            while s < B:
                compose(dst_pt, NBP, [[1, NB]], 0,
                        src_pt, NBP, [[1, NB]], -s,
                        src_pt, NBP, [[1, NB]], 0)
                nc.vector.tensor_copy(
                    _ap(dst_pt, 0, [[NBP, 12], [B, G], [1, s]]),
                    _ap(src_pt, 0, [[NBP, 12], [B, G], [1, s]]))
                src_pt, dst_pt = dst_pt, src_pt
                s *= 2
            ptf = src_pt

            # ---- stage C ----
            def dma_out_cols(a0, ln, ring):
                base, mloc, gs, cs = out_view(a0, ln)
                nc.scalar.dma_start(
                    out=_dram_ap(outT[:, :, :, :], a0,
                                 [[3 * G * M, P], [G * M, 3], [M, G], [1, ln]]),
                    in_=_ap(base, mloc, [[cs, 3], [gs, G], [1, ln]]),
                )

            def apply_single_from(coef, coefq, coefoff, m0, length):
                """out[:, :, m0:m0+length] = R@p + b with per-(partition,g)
                scalar coefficients from `coef` (q stride coefq, g stride
                coefoff).  Muls on ACT (per-partition scale), adds on DVE."""
                plbase, mloc, gs, cs = pl_view(m0, length, None)
                obase, omloc, ogs, ocs = out_view(m0, length)
                tmp_idx[0] += 1
                prod = [[pool.tile([P, G * length], f32,
                                   name=f"prod{tmp_idx[0]}_{i}_{cc}")[:, :]
                         for cc in range(3)] for i in range(3)]
                for i in range(3):
                    for cc in range(3):
                        for g in range(G):
                            nc.scalar.activation(
                                _ap(prod[i][cc], g * length, [[1, length]]),
                                _ap(plbase, cc * cs + g * gs + mloc,
                                    [[1, length]]),
                                Act.Identity,
                                scale=_ap(coef, (4 * i + cc) * coefq
                                          + g * coefoff, [[1, 1]]),
                            )
                for i in range(3):
                    d_t = [[length, G], [1, length]]
                    s1 = _ap(prod[i][0], 0, d_t)
                    nc.vector.tensor_add(s1, s1, _ap(prod[i][1], 0, d_t))
                    nc.vector.tensor_add(s1, s1, _ap(prod[i][2], 0, d_t))
                    for g in range(G):
                        # + translation via the ACT bias port (keeps DVE free)
                        nc.scalar.activation(
                            _ap(obase, i * ocs + g * ogs + omloc, [[1, length]]),
                            _ap(prod[i][0], g * length, [[1, length]]),
                            Act.Identity,
                            bias=_ap(coef, (4 * i + 3) * coefq + g * coefoff,
                                     [[1, 1]]),
                            scale=1.0,
                        )

            pt_last = bass.AP(tensor=ptf.tensor, offset=ptf.offset + (B - 1),
                              ap=list(ptf.ap))

            def apply_runs(starts, length, ks):
                nr = len(starts)
                if nr == 1 and ks[0] == K - 1:
                    # chain-last prefix == last block product: ready right
                    # after the block-prefix scan, before distribute.
                    apply_single_from(pt_last, NBP, B, starts[0], length)
                    return
                if nr == 1:
                    base = bass.AP(tensor=ct.tensor, offset=ct.offset + ks[0],
                                   ap=list(ct.ap))
                    apply_single_from(base, GK, K, starts[0], length)
                    return
                sm = starts[1] - starts[0]
                sk = ks[1] - ks[0]
                m0, k0 = starts[0], ks[0]
                span = max(starts) + length - m0
                plbase, mloc, gs, cs = pl_view(m0, span, None)
                obase, omloc, ogs, ocs = out_view(m0, span)
                d_pl = [[gs, G], [sm, nr], [1, length]]
                d_out = [[ogs, G], [sm, nr], [1, length]]
                d_c = [[K, G], [sk, nr], [0, length]]
                d_acc = [[nr * length, G], [length, nr], [1, length]]
                for i in range(3):
                    for cc in range(2):
                        tgt = acc if cc == 0 else ac2
                        nc.vector.tensor_mul(
                            _ap(tgt, 0, d_acc),
                            _ap(plbase, cc * cs + mloc, d_pl),
                            _ap(ct, (4 * i + cc) * GK + k0, d_c),
                        )
                    nc.vector.tensor_add(
                        _ap(acc, 0, d_acc), _ap(acc, 0, d_acc), _ap(ac2, 0, d_acc)
                    )
                    nc.vector.tensor_mul(
                        _ap(ac2, 0, d_acc),
                        _ap(plbase, 2 * cs + mloc, d_pl),
                        _ap(ct, (4 * i + 2) * GK + k0, d_c),
                    )
                    nc.vector.tensor_add(
                        _ap(acc, 0, d_acc), _ap(acc, 0, d_acc),
                        _ap(ac2, 0, d_acc),
                    )
                    nc.vector.tensor_add(
                        _ap(obase, i * ocs + omloc, d_out),
                        _ap(acc, 0, d_acc),
                        _ap(ct, (4 * i + 3) * GK + k0, d_c),
                    )

            def emit_distribute():
                # distribute: block 0 copies, blocks b>=1 get P[b-1] @ W
                nk = (B - 1) * L
                nc.vector.tensor_copy(
                    _ap(ct, 0, [[GK, 12], [K, G], [1, L]]),
                    _ap(at_flat, 0, [[GK, 12], [K, G], [1, L]]),
                )
                # broadcast block prefixes over t so g fuses into 3-dim APs:
                # PTB[g][q][jb*L + t] = ptf[q][g*B + jb]
                for g in range(G):
                    nc.vector.tensor_copy(
                        _ap(ptb, g * 12 * nk, [[nk, 12], [L, B - 1], [1, L]]),
                        _ap(ptf, g * B, [[NBP, 12], [1, B - 1], [0, L]]))
                d_w = [[GK, 4], [K, G], [1, nk]]
                d_a = [[G * nk, 4], [nk, G], [1, nk]]
                accs = (acc, ac2, ac3)
                # all 9 cross products first (pure reads of W and P), then
                # the combines
                for m in range(3):
                    for i in range(3):
                        nc.vector.tensor_mul(
                            _ap(accs[m], i * 4 * G * nk, d_a),
                            _ap(at_flat, 4 * m * GK + L, d_w),
                            _ap(ptb, (4 * i + m) * nk,
                                [[0, 4], [12 * nk, G], [1, nk]]),
                        )
                for i in range(3):
                    o = i * 4 * G * nk
                    nc.vector.tensor_add(
                        _ap(acc, o, d_a), _ap(acc, o, d_a), _ap(ac2, o, d_a))
                    nc.vector.tensor_add(
                        _ap(ct, 4 * i * GK + L, d_w),
                        _ap(acc, o, d_a), _ap(ac3, o, d_a))
                    nc.vector.tensor_add(
                        _ap(ct, (4 * i + 3) * GK + L, [[K, G], [1, nk]]),
                        _ap(ct, (4 * i + 3) * GK + L, [[K, G], [1, nk]]),
                        _ap(ptb, (4 * i + 3) * nk, [[12 * nk, G], [1, nk]]),
                    )

            # unmoved atoms: copy + DMA as soon as PL lands
            unmoved = [m for m in range(M) if km[m] < 0]
            u0 = 0
            while u0 < len(unmoved):
                u1 = u0
                while u1 + 1 < len(unmoved) and unmoved[u1 + 1] == unmoved[u1] + 1:
                    u1 += 1
                a0, ln = unmoved[u0], u1 - u0 + 1
                assert a0 + ln <= SP or a0 >= SPB
                ubase, umloc, ugs, ucs = pl_view(a0, ln, None)
                uobase, uomloc, uogs, uocs = out_view(a0, ln)
                nc.vector.tensor_copy(
                    _ap(uobase, uomloc, [[uocs, 3], [uogs, G], [1, ln]]),
                    _ap(ubase, umloc, [[ucs, 3], [ugs, G], [1, ln]]),
                )
                dma_out_cols(a0, ln, 0)
                u0 = u1 + 1

            # classes: chain-last single-run first (overlaps distribute)
            by_len = {}
            for (m0, ln, k) in runs:
                by_len.setdefault(ln, []).append((m0, k))
            classes = sorted(
                by_len.items(),
                key=lambda kv: 0 if (len(kv[1]) == 1 and kv[1][0][1] == K - 1)
                else 1)
            emitted_distribute = False
            ring = 1
            for ln, rs in classes:
                starts = [r[0] for r in rs]
                ks = [r[1] for r in rs]
                nr = len(rs)
                chain_last_single = nr == 1 and ks[0] == K - 1
                if not chain_last_single and not emitted_distribute:
                    emit_distribute()
                    emitted_distribute = True
                regular = nr <= 2 or (
                    all(starts[r] == starts[0] + r * (starts[1] - starts[0])
                        for r in range(nr))
                    and all(ks[r] == ks[0] + r * (ks[1] - ks[0])
                            for r in range(nr))
                )
                if regular and nr >= 4:
                    # skewed halves: the later chunk is smaller so the final
                    # exposed output DMA is short
                    h = (nr * 3) // 4
                    apply_runs(starts[:h], ln, ks[:h])
                    lo = min(starts[:h]); hi = max(s + ln for s in starts[:h])
                    dma_out_cols(lo, hi - lo, ring); ring ^= 1
                    apply_runs(starts[h:], ln, ks[h:])
                    lo = min(starts[h:]); hi = max(s + ln for s in starts[h:])
                    dma_out_cols(lo, hi - lo, ring); ring ^= 1
                    continue
                if regular:
                    apply_runs(starts, ln, ks)
                else:
                    for (m0, k) in rs:
                        apply_runs([m0], ln, [k])
                lo = min(starts)
                hi = max(s + ln for s in starts)
                dma_out_cols(lo, hi - lo, ring)
                ring ^= 1

    _split_multi_waits(nc)
    return nc


_BUILD_CACHE = {}


def make_in_maps(input, pos, angles, move_mask):
    input = np.asarray(input, dtype=np.float32)
    pos = np.asarray(pos, dtype=np.float32)
    angles = np.asarray(angles)
    N, K = input.shape
    M = pos.shape[1]
    NL = N // NCORES
    G = NL // P
    SP, SPB = _seg_bounds(angles, np.asarray(move_mask).astype(bool), M)
    arange_quads = bool((angles == np.arange(K * 4).reshape(K, 4)).all())
    in_maps = []
    for c in range(NCORES):
        sl = slice(c * NL, (c + 1) * NL)
        # (NL, M, 3) -> (P, 3, G, M): partition-major so each partition row
        # is one contiguous DMA descriptor
        pm = pos[sl].reshape(G, P, M, 3).transpose(1, 3, 0, 2)
        vrows = input[sl].reshape(G, P, K).transpose(1, 0, 2).reshape(P, G * K)
        im = {"vin": np.ascontiguousarray(vrows),
              "pivA": np.ascontiguousarray(
                  pm[:, :, :, :SP].reshape(P, 3 * G * SP))}
        if arange_quads:
            # pivP[p][c][q][g][k] = pm[p][c][g][4k+q]
            pp = pm[:, :, :, :4 * K].reshape(P, 3, G, K, 4)
            im["pivP"] = np.ascontiguousarray(
                pp.transpose(0, 1, 4, 2, 3).reshape(P, 12 * G * K))
        if SPB < M:
            im["posB"] = np.ascontiguousarray(pm[:, :, :, SPB:])
        in_maps.append(im)
    return in_maps


def kernel(input, pos, angles, move_mask):
    input = np.ascontiguousarray(np.asarray(input, dtype=np.float32))
    pos = np.ascontiguousarray(np.asarray(pos, dtype=np.float32))
    angles = np.asarray(angles)
    move_mask = np.asarray(move_mask).astype(bool)

    N, K = input.shape
    _, M, three = pos.shape
    assert three == 3
    assert N % (NCORES * P) == 0
    NL = N // NCORES

    key = (N, K, M, angles.tobytes(), move_mask.tobytes())
    nc = _BUILD_CACHE.get(key)
    if nc is None:
        nc = _build(angles, move_mask, NL, K, M)
        _BUILD_CACHE[key] = nc

    in_maps = make_in_maps(input, pos, angles, move_mask)

    # the axon-proxied NRT occasionally wedges transiently
    # (NRT_EXEC_UNIT_UNRECOVERABLE); one retry recovers it
    try:
        res = run_bass_kernel_spmd(nc, in_maps, list(range(NCORES)))
    except Exception:
        res = run_bass_kernel_spmd(nc, in_maps, list(range(NCORES)))

    out = np.empty((N, M, 3), dtype=np.float32)
    for c in range(NCORES):
        sl = slice(c * NL, (c + 1) * NL)
        o = res.results[c]["outT"]           # (P, 3, G, M)
        out[sl] = o.transpose(2, 0, 3, 1).reshape(NL, M, 3)
    return out



# revision 52
# speedup vs baseline: 1.0475x; 1.0027x over previous
"""Dihedral2Coord Trainium2 kernel.

Math: the reference applies K sequential dihedral-set steps; step k rotates
a suffix of the atom chain rigidly about the current J-K bond.  Every step's
transform is a proper rigid motion that moves all four pivot atoms of every
later step together, so the dihedral measured at application time equals the
dihedral of the ORIGINAL coordinates (dihedrals are invariant under rigid
motion).  Step k's rotation, expressed in original coordinates, is therefore
a fixed affine A_k computable from the original positions alone, and the
cumulative transform is the prefix product C_k = A_0 @ A_1 @ ... @ A_k.
The kernel:
  A) computes all K per-conformer Rodrigues affines in parallel,
  B) prefix-composes them with a blocked scan,
  C) applies C_{km(m)} to each atom run, where km(m) counts the steps whose
     mask includes atom m (verified prefix-structured on host).

Sharding: pure data parallelism over conformers N across 8 cores (SPMD).
"""

import sys

import numpy as np

try:
    import concourse.bass as bass
except ImportError:  # path in the grading container
    sys.path.insert(0, "/opt/trn_rl_repo")
    import concourse.bass as bass

import concourse.tile as tile
from concourse import mybir
from concourse.bass_utils import run_bass_kernel_spmd

f32 = mybir.dt.float32
i32 = mybir.dt.int32
Alu = mybir.AluOpType
Act = mybir.ActivationFunctionType

NCORES = 8
P = 128
TWO_PI = float(2.0 * np.pi)
_HALF_PI = float(np.pi / 2)

_WAIT_CAP = 1  # this walrus build rejects >1 sync-wait per instruction


def _register_const(nc, value, dtype=f32):
    """Register an activation-bias constant.  Written on the Activation
    engine from the framework's const-1.0 AP (ordered by Bass.__init__'s
    barrier); later ACT reads are same-engine program-ordered, so no extra
    barrier is needed."""
    if (dtype, value) in nc.const_aps.aps:
        return
    t = nc.alloc_sbuf_tensor(f"const-{dtype.name}-{value}", [128, 1], dtype)
    one = nc.const_aps.aps[(f32, 1.0)]
    nc.scalar.activation(t.ap(), one, Act.Identity, bias=0.0, scale=float(value))
    nc.const_aps.aps[(dtype, value)] = t.ap()


def _split_multi_waits(nc):
    """Split every instruction carrying >cap sync-waits into single-wait
    NoOps (same engine, immediately before, same block).  Waits are monotone
    semaphore conditions so this preserves semantics exactly."""
    n = 0
    for func in nc.m.functions:
        for bb in func.blocks:
            old = list(bb.instructions)
            if not any(
                i.sync_info is not None and len(i.sync_info.on_wait) > _WAIT_CAP
                for i in old
            ):
                continue
            new = []
            for inst in old:
                si = inst.sync_info
                if si is not None and len(si.on_wait) > _WAIT_CAP:
                    waits = list(si.on_wait)
                    head, tail = waits[:-_WAIT_CAP], waits[-_WAIT_CAP:]
                    for j in range(0, len(head), _WAIT_CAP):
                        n += 1
                        new.append(
                            mybir.InstNoOp(
                                name=f"{inst.name}_ws{j}",
                                engine=inst.engine,
                                sync_info=mybir.SyncInfo(
                                    on_wait=list(head[j : j + _WAIT_CAP]), on_update=[]
                                ),
                                bass_nofuse=True,
                            )
                        )
                    try:
                        si.on_wait[:] = tail
                    except TypeError:
                        inst.sync_info = mybir.SyncInfo(
                            on_wait=tail, on_update=list(si.on_update)
                        )
                new.append(inst)
            try:
                bb.instructions[:] = new
            except TypeError:
                bb.instructions = new
    return n


def _ap(base, offset_elems, dims):
    """Free-dim AP view into an SBUF tile AP `base` (partition dim kept).
    dims: list of [step, count] in elements of the tile's free space."""
    return bass.AP(
        tensor=base.tensor,
        offset=base.offset + offset_elems,
        ap=[list(base.ap[0])] + [list(d) for d in dims],
    )


def _dram_ap(t, offset, dims):
    return bass.AP(tensor=t.tensor, offset=offset, ap=[list(d) for d in dims])


def _analyse_mask(angles, move_mask):
    """Host-side structural analysis. Returns (km, runs): km[m] is the last
    step applied to atom m (-1 = never moved); runs are (start, len, k)."""
    K, M = move_mask.shape
    km = move_mask.astype(np.int64).sum(0) - 1
    kk = np.arange(K)[:, None]
    if not (move_mask == (kk <= km[None, :])).all():
        raise NotImplementedError("move_mask is not prefix-structured per atom")
    for k in range(K):
        for a in angles[k]:
            if not move_mask[:k, a].all():
                raise NotImplementedError("pivot atoms not rigidly co-moved")
    runs = []
    m = 0
    while m < M:
        j = m
        while j + 1 < M and km[j + 1] == km[m]:
            j += 1
        if km[m] >= 0:
            runs.append((m, j - m + 1, int(km[m])))
        m = j + 1
    return km, runs


def _seg_bounds(angles, move_mask, M):
    """(SP, SPB): pivot region [0, SP); B-tile starts at SPB <= SP so no
    run/unmoved segment crosses a tile boundary (columns [SPB, SP) are
    duplicated in both tiles)."""
    km, runs = _analyse_mask(angles, move_mask)
    SP = min(int(np.asarray(angles).max()) + 1, M)
    segs = [(m0, ln) for (m0, ln, _k) in runs]
    m = 0
    while m < M:
        if km[m] < 0:
            j = m
            while j + 1 < M and km[j + 1] < 0:
                j += 1
            segs.append((m, j - m + 1))
            m = j + 1
        else:
            m += 1
    SPB = SP
    for (m0, ln) in segs:
        if m0 < SP < m0 + ln:
            SPB = min(SPB, m0)
    return SP, SPB


def _build(angles, move_mask, NL, K, M):
    """Build the Bass module for one core handling NL conformers."""
    G = NL // P
    assert NL == G * P
    GK = G * K
    L = 8               # within-block scan length
    assert K % L == 0
    B = K // L          # blocks per conformer-group
    NB = G * B          # blocks over the flattened (g,k) axis

    angles = np.asarray(angles)
    arange_quads = bool((angles == np.arange(K * 4).reshape(K, 4)).all())
    km, runs = _analyse_mask(angles, move_mask)

    nc = bass.Bass()
    for cval in (1024.0, 1024.25, 1024.0 * TWO_PI, 1024.0 * TWO_PI + _HALF_PI):
        _register_const(nc, float(cval))
    SP, SPB = _seg_bounds(angles, move_mask, M)
    MB = M - SPB        # B-tile width
    vinD = nc.declare_dram_parameter("vin", [P, G * K], f32, isOutput=False)
    pivA = nc.declare_dram_parameter("pivA", [P, 3 * G * SP], f32,
                                     isOutput=False)
    # quad-permuted pivot planes: pivP[c][q][g][k] = pos[., 4k+q, c] so the
    # r-vector subtraction reads contiguously (innermost (g,k))
    pivPd = (nc.declare_dram_parameter("pivP", [P, 12 * G * K], f32,
                                       isOutput=False) if arange_quads
             else None)
    posB = (nc.declare_dram_parameter("posB", [P, 3, G, MB], f32,
                                      isOutput=False) if SPB < M else None)
    outT = nc.declare_dram_parameter("outT", [P, 3, G, M], f32, isOutput=True)

    with tile.TileContext(nc) as tc:
        with tc.tile_pool(name="main", bufs=1) as pool:
            # ---- SBUF tensors ----
            # separate tiles per DMA so consumers wait only on what they need
            VIN = pool.tile([P, G * K], f32)
            PLA = pool.tile([P, 3 * G * SP], f32)
            PIVP = (pool.tile([P, 3, 4, GK], f32, name="PIVP")
                    if arange_quads else None)
            PLB = pool.tile([P, 3, G, MB], f32, name="PLB") if SPB < M else None
            OUTA = pool.tile([P, 3, G, SP], f32)
            OUTB = pool.tile([P, 3, G, MB], f32, name="OUTB") if SPB < M else None
            # packed r-vectors / normals with duplicated xy components so a
            # +1/+2 component rotation is a plain offset (cross-product trick)
            RV = pool.tile([P, 3, 5, G, K], f32)  # (rIJ,rJK,rKL) x (x,y,z,x,y)
            NN = pool.tile([P, 3, 5, G, K], f32)  # (nIJK,nJKL,m) x (x,y,z,x,y)
            TA = pool.tile([P, 2, 3, G, K], f32)
            TB = pool.tile([P, 2, 3, G, K], f32)
            AT = pool.tile([P, 12, G, K], f32)   # A_k; q=4i+j, strides q:GK, g:K, k:1
            CT = pool.tile([P, 12, GK], f32)     # full prefixes
            NBP = NB + 4   # 4 pad columns so Hillis j<s lanes read in-bounds
            PT = pool.tile([P, 12, NBP], f32)    # block products / prefixes
            PT2 = pool.tile([P, 12, NBP], f32)   # Hillis ping-pong buffer
            PTB = pool.tile([P, G, 12, (K // 8) * 7], f32)  # prefixes bcast over t
            ACC = pool.tile([P, 12 * max(GK, 64)], f32)
            AC2 = pool.tile([P, 12 * max(GK, 64)], f32)
            AC3 = pool.tile([P, 12 * max(GK, 64)], f32)

            vv = _ap(VIN[:, :], 0, [[K, G], [1, K]])
            pla = _ap(PLA[:, :], 0, [])
            pivp = PIVP[:, :, :, :] if PIVP is not None else None
            plb = PLB[:, :, :, :] if PLB is not None else None
            outa = OUTA[:, :, :, :]
            outb = OUTB[:, :, :, :] if OUTB is not None else None

            def pl_view(m0, ln, _unused=None):
                """(base_ap, local column offset, group stride, comp stride)
                for columns [m0, m0+ln): B tile from SPB, else A tile."""
                if m0 >= SPB:
                    return plb, m0 - SPB, MB, G * MB
                assert m0 + ln <= SP
                return pla, m0, SP, G * SP

            def out_view(m0, ln):
                if m0 >= SPB:
                    return outb, m0 - SPB, MB, G * MB
                assert m0 + ln <= SP
                return outa, m0, SP, G * SP
            rv = RV[:, :, :, :, :]
            nn = NN[:, :, :, :, :]
            t1v = TA[:, :, :, :, :]
            t2v = TB[:, :, :, :, :]
            at = AT[:, :, :, :]
            ct = CT[:, :, :]
            pt = _ap(PT[:, :, :], 4, [[NBP, 12], [1, NB]])
            pt2 = _ap(PT2[:, :, :], 4, [[NBP, 12], [1, NB]])
            ptb = PTB[:, :, :, :]
            acc = ACC[:, :]
            ac2 = AC2[:, :]
            ac3 = AC3[:, :]

            RVv, RVc = 5 * GK, GK   # RV strides: vec, comp
            NVv = 5 * GK

            # ---- DMA in ----
            # All on the sync ring, in priority order: pivP (gates stage A),
            # vin (gates the ACT sin chain), pivA (gates pJ copy + A-apply),
            # posB (gates the B-tile apply, late).  Host arrays are
            # partition-major so each row is one contiguous descriptor.
            if pivp is not None:
                nc.sync.dma_start(
                    out=_ap(pivp, 0, [[1, 12 * GK]]),
                    in_=_dram_ap(pivPd[:, :], 0, [[12 * GK, P], [1, 12 * GK]]),
                )
            nc.sync.dma_start(
                out=_ap(vv, 0, [[1, GK]]),
                in_=_dram_ap(vinD[:, :], 0, [[GK, P], [1, GK]]),
            )
            nc.sync.dma_start(
                out=_ap(pla, 0, [[1, 3 * G * SP]]),
                in_=_dram_ap(pivA[:, :], 0, [[3 * G * SP, P], [1, 3 * G * SP]]),
            )
            if PLB is not None:
                nc.sync.dma_start(
                    out=_ap(plb, 0, [[1, 3 * G * MB]]),
                    in_=_dram_ap(posB[:, :, :, :], 0,
                                 [[3 * G * MB, P], [1, 3 * G * MB]]),
                )
            # Hillis pad columns must hold finite values (they feed the
            # patched lanes); zero them before the block-prefix scan
            nc.gpsimd.memset(_ap(PT[:, :, :], 0, [[NBP, 12], [1, 4]]), 0.0)
            nc.gpsimd.memset(_ap(PT2[:, :, :], 0, [[NBP, 12], [1, 4]]), 0.0)

            # ---- helpers ----
            tmp_idx = [0]

            def T(dt=f32):
                tmp_idx[0] += 1
                return pool.tile([P, G, K], dt, name=f"tmp{tmp_idx[0]}")

            def mul(a, b):
                o = T(); nc.vector.tensor_mul(o, a, b); return o

            def add(a, b):
                o = T(); nc.vector.tensor_add(o, a, b); return o

            def sub(a, b):
                o = T(); nc.vector.tensor_sub(o, a, b); return o

            def aff(a, scale, bias):
                o = T()
                nc.scalar.activation(o, a, Act.Identity, bias=bias, scale=scale)
                return o

            def activ(a, fn):
                o = T(); nc.scalar.activation(o, a, fn); return o

            def dot3v(a_base, a_off, a_cs, b_base, b_off, b_cs, eng=None):
                """dot over xyz comps via one mul + one innermost-reduce.
                a/b given as (tile_ap, elem offset, comp stride); both must
                have gk contiguous (stride 1)."""
                tmp_idx[0] += 1
                dp = pool.tile([P, GK, 3], f32, name=f"dp{tmp_idx[0]}")[:, :, :]
                (eng or nc.vector).tensor_mul(
                    dp,
                    _ap(a_base, a_off, [[1, GK], [a_cs, 3]]),
                    _ap(b_base, b_off, [[1, GK], [b_cs, 3]]),
                )
                o = T()
                nc.vector.tensor_reduce(
                    _ap(o, 0, [[1, GK]]), dp, mybir.AxisListType.X, Alu.add)
                return o

            # ---- pivot sources ----
            if not arange_quads:
                PIV = pool.tile([P, 3, G, 4, K], f32)
                pv = PIV[:, :, :, :, :]
                for k in range(K):
                    for q in range(4):
                        nc.vector.tensor_copy(
                            _ap(pv, q * K + k, [[G * 4 * K, 3], [4 * K, G]]),
                            _ap(pla, int(angles[k, q]),
                                [[G * SP, 3], [SP, G]]),
                        )

            def piv_ap(c, q):
                if arange_quads:
                    return _ap(pivp, c * 4 * GK + q * GK, [[K, G], [1, K]])
                return _ap(pv, c * G * 4 * K + q * K, [[4 * K, G], [1, K]])

            pJ = [piv_ap(c, 1) for c in range(3)]

            def _ap_cat3(_pj):
                # the three pJ views share a regular comp stride; rebuild as
                # one 3-dim AP [c][g][k]
                if arange_quads:
                    return _ap(pivp, GK, [[4 * GK, 3], [1, GK]])
                return _ap(pv, K, [[G * 4 * K, 3], [4 * K, G], [1, K]])

            # ---- stage A: packed r-vectors and cross products ----
            if arange_quads:
                # quad-permuted pivots: v-dim is the q axis, (g,k) contiguous
                nc.vector.tensor_sub(
                    _ap(rv, 0, [[RVv, 3], [RVc, 3], [1, GK]]),
                    _ap(pivp, GK, [[GK, 3], [4 * GK, 3], [1, GK]]),
                    _ap(pivp, 0, [[GK, 3], [4 * GK, 3], [1, GK]]))
            else:
                for g in range(G):
                    in1 = _ap(pv, g * 4 * K + K,
                              [[K, 3], [G * 4 * K, 3], [1, K]])
                    in0 = _ap(pv, g * 4 * K + 0,
                              [[K, 3], [G * 4 * K, 3], [1, K]])
                    nc.vector.tensor_sub(
                        _ap(rv, g * K, [[RVv, 3], [RVc, 3], [1, K]]), in1, in0)
            # duplicate comps x,y into slots 3,4
            nc.vector.tensor_copy(
                _ap(rv, 3 * RVc, [[RVv, 3], [RVc, 2], [1, GK]]),
                _ap(rv, 0, [[RVv, 3], [RVc, 2], [1, GK]]))
            # nIJK, nJKL = cross(A=[rIJ,rJK], B=[rJK,rKL]) via comp offsets
            nc.vector.tensor_mul(
                _ap(t1v, 0, [[3 * GK, 2], [GK, 3], [1, GK]]),
                _ap(rv, RVc, [[RVv, 2], [RVc, 3], [1, GK]]),
                _ap(rv, RVv + 2 * RVc, [[RVv, 2], [RVc, 3], [1, GK]]))
            nc.vector.tensor_mul(
                _ap(t2v, 0, [[3 * GK, 2], [GK, 3], [1, GK]]),
                _ap(rv, 2 * RVc, [[RVv, 2], [RVc, 3], [1, GK]]),
                _ap(rv, RVv + RVc, [[RVv, 2], [RVc, 3], [1, GK]]))
            nc.vector.tensor_sub(
                _ap(nn, 0, [[NVv, 2], [GK, 3], [1, GK]]),
                _ap(t1v, 0, [[3 * GK, 2], [GK, 3], [1, GK]]),
                _ap(t2v, 0, [[3 * GK, 2], [GK, 3], [1, GK]]))
            nc.vector.tensor_copy(
                _ap(nn, 3 * GK, [[NVv, 2], [GK, 2], [1, GK]]),
                _ap(nn, 0, [[NVv, 2], [GK, 2], [1, GK]]))
            # m = nIJK x rJK -> NN vec slot 2
            nc.vector.tensor_mul(
                _ap(t1v, 0, [[GK, 3], [1, GK]]),
                _ap(nn, GK, [[GK, 3], [1, GK]]),
                _ap(rv, RVv + 2 * RVc, [[RVc, 3], [1, GK]]))
            nc.vector.tensor_mul(
                _ap(t2v, 0, [[GK, 3], [1, GK]]),
                _ap(nn, 2 * GK, [[GK, 3], [1, GK]]),
                _ap(rv, RVv + RVc, [[RVc, 3], [1, GK]]))
            nc.vector.tensor_sub(
                _ap(nn, 2 * NVv, [[GK, 3], [1, GK]]),
                _ap(t1v, 0, [[GK, 3], [1, GK]]),
                _ap(t2v, 0, [[GK, 3], [1, GK]]))

            # pJ source: read straight out of pivP when available, else make
            # a compact copy
            if arange_quads:
                pj_b, pj_off, pj_cs = pivp, GK, 4 * GK
            else:
                PJC = pool.tile([P, 3, G, K], f32)
                pjc = PJC[:, :, :, :]
                nc.vector.tensor_copy(_ap(pjc, 0, [[GK, 3], [K, G], [1, K]]),
                                      _ap_cat3(pJ))
                pj_b, pj_off, pj_cs = pjc, 0, GK

            def rvec(v, c):
                return _ap(rv, v * RVv + c * RVc, [[K, G], [1, K]])

            def nvec(v, c):
                return _ap(nn, v * NVv + c * GK, [[K, G], [1, K]])

            rJK = [rvec(1, c) for c in range(3)]
            rjk_off = RVv                      # RV vec 1, comp stride RVc
            m_off = 2 * NVv                    # m lives in NN vec 2

            # paired dot products: one mul+reduce covers two dots that share
            # a left operand; results land adjacently for fused downstream ops
            DOTS = pool.tile([P, 4, GK], f32)  # rows: x0, l1^2, y0, lm^2
            DP4 = pool.tile([P, 2, GK, 3], f32)
            dots = DOTS[:, :, :]
            dp4 = DP4[:, :, :, :]
            # {x0, l1^2} = nIJK . (nJKL, nIJK)
            nc.vector.tensor_mul(
                dp4,
                _ap(nn, 0, [[0, 2], [1, GK], [GK, 3]]),
                _ap(nn, NVv, [[-NVv, 2], [1, GK], [GK, 3]]))
            nc.vector.tensor_reduce(
                _ap(dots, 0, [[GK, 2], [1, GK]]), dp4,
                mybir.AxisListType.X, Alu.add)
            # y0 = m . nJKL (single dot; reuse dp4's first GK*3 lane block)
            nc.vector.tensor_mul(
                _ap(dp4, 0, [[3, GK], [1, 3]]),
                _ap(nn, m_off, [[1, GK], [GK, 3]]),
                _ap(nn, NVv, [[1, GK], [GK, 3]]))
            nc.vector.tensor_reduce(
                _ap(dots, 2 * GK, [[1, GK]]),
                _ap(dp4, 0, [[3, GK], [1, 3]]),
                mybir.AxisListType.X, Alu.add)
            jks = dot3v(rv, rjk_off, RVc, rv, rjk_off, RVc)
            # lm^2 = l1^2 * |rJK|^2  (m = nIJK x rJK with nIJK _|_ rJK)
            nc.vector.tensor_mul(
                _ap(dots, 3 * GK, [[1, GK]]),
                _ap(dots, GK, [[1, GK]]),
                _ap(jks[:, :, :], 0, [[1, GK]]))
            L1LM = pool.tile([P, 2, GK], f32)  # (l1, lm)
            l1lm = L1LM[:, :, :]
            nc.scalar.activation(
                _ap(l1lm, 0, [[GK, 2], [1, GK]]),
                _ap(dots, GK, [[2 * GK, 2], [1, GK]]), Act.Sqrt)
            XY = pool.tile([P, 2, GK], f32)    # (x1, y1) = (x0*lm, y0*l1)
            xy = XY[:, :, :]
            nc.vector.tensor_mul(
                xy,
                _ap(dots, 0, [[2 * GK, 2], [1, GK]]),
                _ap(l1lm, GK, [[-GK, 2], [1, GK]]))
            SQ = pool.tile([P, 2, GK], f32)
            sq = SQ[:, :, :]
            nc.vector.tensor_mul(sq, xy, xy)
            hs = T()
            nc.vector.tensor_add(_ap(hs[:, :, :], 0, [[1, GK]]),
                                 _ap(sq, 0, [[1, GK]]),
                                 _ap(sq, GK, [[1, GK]]))
            hr = T(); nc.vector.reciprocal(hr, hs)
            rh = activ(hr, Act.Sqrt)            # 1/hypot
            CS = pool.tile([P, 2, GK], f32)     # (ccur, scur)
            cs_ = CS[:, :, :]
            nc.vector.tensor_mul(
                cs_, xy, _ap(rh[:, :, :], 0, [[0, 2], [1, GK]]))
            jkr = T(); nc.vector.reciprocal(jkr, jks)
            jrs = activ(jkr, Act.Sqrt)          # 1/|rJK|
            AXT = pool.tile([P, 3, G, K], f32)
            axt = AXT[:, :, :, :]
            nc.vector.tensor_mul(
                _ap(axt, 0, [[GK, 3], [1, GK]]),
                _ap(rv, rjk_off, [[RVc, 3], [1, GK]]),
                _ap(jrs[:, :, :], 0, [[0, 3], [1, GK]]),
            )
            ax = [_ap(axt, c * GK, [[K, G], [1, K]]) for c in range(3)]

            # sin/cos of targets with range reduction (Sin table ok |x|<~3.55)
            def reduced_sin(shift_quarter, extra):
                q = aff(vv, 1.0 / TWO_PI, 1024.0 + shift_quarter)
                qi = T(i32)
                nc.vector.tensor_copy(qi, q)     # f32->i32 rounds to nearest
                qf = T()
                nc.vector.tensor_copy(qf, qi)
                t = aff(qf, -TWO_PI, 1024.0 * TWO_PI + extra)
                return activ(add(vv, t), Act.Sin)

            sv = reduced_sin(0.0, 0.0)
            cv = reduced_sin(0.25, _HALF_PI)

            PC1 = pool.tile([P, 2, GK], f32)   # cv * (ccur, scur)
            PC2 = pool.tile([P, 2, GK], f32)   # sv * (ccur, scur)
            pc1 = PC1[:, :, :]
            pc2 = PC2[:, :, :]
            nc.vector.tensor_mul(pc1, cs_, _ap(cv[:, :, :], 0, [[0, 2], [1, GK]]))
            nc.vector.tensor_mul(pc2, cs_, _ap(sv[:, :, :], 0, [[0, 2], [1, GK]]))
            c_ = T()
            s_ = T()
            nc.vector.tensor_add(_ap(c_[:, :, :], 0, [[1, GK]]),
                                 _ap(pc1, 0, [[1, GK]]), _ap(pc2, GK, [[1, GK]]))
            nc.vector.tensor_sub(_ap(s_[:, :, :], 0, [[1, GK]]),
                                 _ap(pc2, 0, [[1, GK]]), _ap(pc1, GK, [[1, GK]]))
            t1_ = T()
            nc.vector.tensor_scalar(t1_, c_, -1.0, 1.0, Alu.mult, Alu.add)  # 1-cos

            TAX = pool.tile([P, 3, G, K], f32)
            SAX = pool.tile([P, 3, G, K], f32)
            UD = pool.tile([P, 3, G, K], f32)
            OD = pool.tile([P, 2, G, K], f32)
            taxv = TAX[:, :, :, :]
            saxv = SAX[:, :, :, :]
            udv = UD[:, :, :, :]
            odv = OD[:, :, :, :]
            d3 = [[GK, 3], [1, GK]]
            bc3 = [[0, 3], [1, GK]]
            nc.vector.tensor_mul(_ap(taxv, 0, d3), _ap(axt, 0, d3),
                                 _ap(t1_[:, :, :], 0, bc3))
            nc.vector.tensor_mul(_ap(saxv, 0, d3), _ap(axt, 0, d3),
                                 _ap(s_[:, :, :], 0, bc3))
            nc.vector.tensor_mul(_ap(udv, 0, d3), _ap(taxv, 0, d3),
                                 _ap(axt, 0, d3))

            def aq(q):
                return _ap(at, q * GK, [[K, G], [1, K]])

            # diagonal: q = 0,5,10 -> stride 5*GK
            nc.vector.tensor_add(
                _ap(at, 0, [[5 * GK, 3], [1, GK]]),
                _ap(udv, 0, d3),
                _ap(c_[:, :, :], 0, bc3),
            )
            # off-diagonal products: txy,txz = tax0*(ax1,ax2); tyz = tax1*ax2
            nc.vector.tensor_mul(
                _ap(odv, 0, [[GK, 2], [1, GK]]),
                _ap(axt, GK, [[GK, 2], [1, GK]]),
                _ap(taxv, 0, [[0, 2], [1, GK]]),
            )
            tyz = T()
            nc.vector.tensor_mul(tyz, _ap(taxv, GK, [[K, G], [1, K]]),
                                 _ap(axt, 2 * GK, [[K, G], [1, K]]))
            txy = _ap(odv, 0, [[K, G], [1, K]])
            txz = _ap(odv, GK, [[K, G], [1, K]])
            sax = [_ap(saxv, c * GK, [[K, G], [1, K]]) for c in range(3)]
            nc.vector.tensor_sub(aq(1), txy, sax[2])
            nc.vector.tensor_add(aq(4), txy, sax[2])
            nc.vector.tensor_add(aq(2), txz, sax[1])
            nc.vector.tensor_sub(aq(8), txz, sax[1])
            nc.vector.tensor_sub(aq(6), tyz, sax[0])
            nc.vector.tensor_add(aq(9), tyz, sax[0])

            # b = pJ - R @ pJ : batched products, reduce, sub
            BP = pool.tile([P, 3, GK, 3], f32)
            bp = BP[:, :, :, :]
            nc.vector.tensor_mul(
                bp,
                _ap(at, 0, [[4 * GK, 3], [1, GK], [GK, 3]]),
                _ap(pj_b, pj_off, [[0, 3], [1, GK], [pj_cs, 3]]),
            )
            RPJ = pool.tile([P, 3, G, K], f32)
            rpj = RPJ[:, :, :, :]
            nc.vector.tensor_reduce(
                _ap(rpj, 0, [[GK, 3], [1, GK]]), bp,
                mybir.AxisListType.X, Alu.add)
            nc.vector.tensor_sub(
                _ap(at, 3 * GK, [[4 * GK, 3], [1, GK]]),
                _ap(pj_b, pj_off, [[pj_cs, 3], [1, GK]]),
                _ap(rpj, 0, [[GK, 3], [1, GK]]),
            )

            # ---- stage B: blocked prefix composition ----
            at_flat = _ap(at, 0, [[GK, 12], [1, GK]])

            def compose(dst, dq, dbd, doff, left, lq, lbd, loff,
                        right, rq, rbd, roff):
                """dst[i,j,*] = sum_m left[i,m,*]*right[m,j,*]; dst[i,3,*] +=
                left[i,3,*].  *bd = batch [step,count] dims (equal counts)."""
                counts = [d[1] for d in dbd]
                assert [d[1] for d in lbd] == counts
                assert [d[1] for d in rbd] == counts
                nb = 1
                for cnt in counts:
                    nb *= cnt
                abd = []
                stp = 1
                for cnt in reversed(counts):
                    abd.insert(0, [stp, cnt])
                    stp *= cnt

                def accv(base):
                    return _ap(base, 0, [[4 * nb, 3], [nb, 4]] + abd)

                dstv = _ap(dst, doff, [[4 * dq, 3], [dq, 4]] + dbd)

                def dmul(tgt, mrow):
                    nc.vector.tensor_mul(
                        accv(tgt),
                        _ap(right, roff + 4 * mrow * rq,
                            [[0, 3], [rq, 4]] + rbd),
                        _ap(left, loff + mrow * lq,
                            [[4 * lq, 3], [0, 4]] + lbd),
                    )

                dmul(acc, 0)
                dmul(ac2, 1)
                nc.vector.tensor_add(accv(acc), accv(acc), accv(ac2))
                dmul(ac2, 2)
                nc.vector.tensor_add(dstv, accv(acc), accv(ac2))
                bias_d = _ap(dst, doff + 3 * dq, [[4 * dq, 3]] + dbd)
                nc.vector.tensor_add(
                    bias_d, bias_d,
                    _ap(left, loff + 3 * lq, [[4 * lq, 3]] + lbd),
                )

            # within-block scan, in place: A[:, t] <- A[:, t-1] o A[:, t]
            # (the 3 muls read the slot before the final add overwrites it)
            for t in range(1, L):
                compose(at_flat, GK, [[L, NB]], t,
                        at_flat, GK, [[L, NB]], t - 1,
                        at_flat, GK, [[L, NB]], t)
            # block products
            nc.vector.tensor_copy(
                _ap(pt, 0, [[NBP, 12], [1, NB]]),
                _ap(at_flat, L - 1, [[GK, 12], [L, NB]]),
            )
            # block-prefix scan: Hillis-Steele over the flattened (g,b) axis.
            # Lanes j%B < s read the neighbour's tail (garbage) and are
            # patched by the trailing copy before the buffers swap.
            src_pt, dst_pt = pt, pt2
            s = 1
            while s < B:
                compose(dst_pt, NBP, [[1, NB]], 0,
                        src_pt, NBP, [[1, NB]], -s,
                        src_pt, NBP, [[1, NB]], 0)
                nc.vector.tensor_copy(
                    _ap(dst_pt, 0, [[NBP, 12], [B, G], [1, s]]),
                    _ap(src_pt, 0, [[NBP, 12], [B, G], [1, s]]))
                src_pt, dst_pt = dst_pt, src_pt
                s *= 2
            ptf = src_pt

            # ---- stage C ----
            def dma_out_cols(a0, ln, ring):
                base, mloc, gs, cs = out_view(a0, ln)
                nc.scalar.dma_start(
                    out=_dram_ap(outT[:, :, :, :], a0,
                                 [[3 * G * M, P], [G * M, 3], [M, G], [1, ln]]),
                    in_=_ap(base, mloc, [[cs, 3], [gs, G], [1, ln]]),
                )

            def apply_single_from(coef, coefq, coefoff, m0, length):
                """out[:, :, m0:m0+length] = R@p + b with per-(partition,g)
                scalar coefficients from `coef` (q stride coefq, g stride
                coefoff).  Muls on ACT (per-partition scale), adds on DVE."""
                plbase, mloc, gs, cs = pl_view(m0, length, None)
                obase, omloc, ogs, ocs = out_view(m0, length)
                tmp_idx[0] += 1
                prod = [[pool.tile([P, G * length], f32,
                                   name=f"prod{tmp_idx[0]}_{i}_{cc}")[:, :]
                         for cc in range(3)] for i in range(3)]
                for i in range(3):
                    for cc in range(3):
                        for g in range(G):
                            nc.scalar.activation(
                                _ap(prod[i][cc], g * length, [[1, length]]),
                                _ap(plbase, cc * cs + g * gs + mloc,
                                    [[1, length]]),
                                Act.Identity,
                                scale=_ap(coef, (4 * i + cc) * coefq
                                          + g * coefoff, [[1, 1]]),
                            )
                for i in range(3):
                    d_t = [[length, G], [1, length]]
                    s1 = _ap(prod[i][0], 0, d_t)
                    nc.vector.tensor_add(s1, s1, _ap(prod[i][1], 0, d_t))
                    nc.vector.tensor_add(s1, s1, _ap(prod[i][2], 0, d_t))
                    for g in range(G):
                        # + translation via the ACT bias port (keeps DVE free)
                        nc.scalar.activation(
                            _ap(obase, i * ocs + g * ogs + omloc, [[1, length]]),
                            _ap(prod[i][0], g * length, [[1, length]]),
                            Act.Identity,
                            bias=_ap(coef, (4 * i + 3) * coefq + g * coefoff,
                                     [[1, 1]]),
                            scale=1.0,
                        )

            pt_last = bass.AP(tensor=ptf.tensor, offset=ptf.offset + (B - 1),
                              ap=list(ptf.ap))

            def apply_runs(starts, length, ks):
                nr = len(starts)
                if nr == 1 and ks[0] == K - 1:
                    # chain-last prefix == last block product: ready right
                    # after the block-prefix scan, before distribute.
                    apply_single_from(pt_last, NBP, B, starts[0], length)
                    return
                if nr == 1:
                    base = bass.AP(tensor=ct.tensor, offset=ct.offset + ks[0],
                                   ap=list(ct.ap))
                    apply_single_from(base, GK, K, starts[0], length)
                    return
                sm = starts[1] - starts[0]
                sk = ks[1] - ks[0]
                m0, k0 = starts[0], ks[0]
                span = max(starts) + length - m0
                plbase, mloc, gs, cs = pl_view(m0, span, None)
                obase, omloc, ogs, ocs = out_view(m0, span)
                d_pl = [[gs, G], [sm, nr], [1, length]]
                d_out = [[ogs, G], [sm, nr], [1, length]]
                d_c = [[K, G], [sk, nr], [0, length]]
                d_acc = [[nr * length, G], [length, nr], [1, length]]
                for i in range(3):
                    for cc in range(2):
                        tgt = acc if cc == 0 else ac2
                        nc.vector.tensor_mul(
                            _ap(tgt, 0, d_acc),
                            _ap(plbase, cc * cs + mloc, d_pl),
                            _ap(ct, (4 * i + cc) * GK + k0, d_c),
                        )
                    nc.vector.tensor_add(
                        _ap(acc, 0, d_acc), _ap(acc, 0, d_acc), _ap(ac2, 0, d_acc)
                    )
                    nc.vector.tensor_mul(
                        _ap(ac2, 0, d_acc),
                        _ap(plbase, 2 * cs + mloc, d_pl),
                        _ap(ct, (4 * i + 2) * GK + k0, d_c),
                    )
                    nc.vector.tensor_add(
                        _ap(acc, 0, d_acc), _ap(acc, 0, d_acc),
                        _ap(ac2, 0, d_acc),
                    )
                    nc.vector.tensor_add(
                        _ap(obase, i * ocs + omloc, d_out),
                        _ap(acc, 0, d_acc),
                        _ap(ct, (4 * i + 3) * GK + k0, d_c),
                    )

            def emit_distribute():
                # distribute: block 0 copies, blocks b>=1 get P[b-1] @ W
                nk = (B - 1) * L
                nc.vector.tensor_copy(
                    _ap(ct, 0, [[GK, 12], [K, G], [1, L]]),
                    _ap(at_flat, 0, [[GK, 12], [K, G], [1, L]]),
                )
                # broadcast block prefixes over t so g fuses into 3-dim APs:
                # PTB[g][q][jb*L + t] = ptf[q][g*B + jb]
                for g in range(G):
                    nc.vector.tensor_copy(
                        _ap(ptb, g * 12 * nk, [[nk, 12], [L, B - 1], [1, L]]),
                        _ap(ptf, g * B, [[NBP, 12], [1, B - 1], [0, L]]))
                d_w = [[GK, 4], [K, G], [1, nk]]
                d_a = [[G * nk, 4], [nk, G], [1, nk]]
                accs = (acc, ac2, ac3)
                # all 9 cross products first (pure reads of W and P), then
                # the combines
                for m in range(3):
                    for i in range(3):
                        nc.vector.tensor_mul(
                            _ap(accs[m], i * 4 * G * nk, d_a),
                            _ap(at_flat, 4 * m * GK + L, d_w),
                            _ap(ptb, (4 * i + m) * nk,
                                [[0, 4], [12 * nk, G], [1, nk]]),
                        )
                for i in range(3):
                    o = i * 4 * G * nk
                    nc.vector.tensor_add(
                        _ap(acc, o, d_a), _ap(acc, o, d_a), _ap(ac2, o, d_a))
                    nc.vector.tensor_add(
                        _ap(ct, 4 * i * GK + L, d_w),
                        _ap(acc, o, d_a), _ap(ac3, o, d_a))
                    nc.vector.tensor_add(
                        _ap(ct, (4 * i + 3) * GK + L, [[K, G], [1, nk]]),
                        _ap(ct, (4 * i + 3) * GK + L, [[K, G], [1, nk]]),
                        _ap(ptb, (4 * i + 3) * nk, [[12 * nk, G], [1, nk]]),
                    )

            # unmoved atoms: copy + DMA as soon as PL lands
            unmoved = [m for m in range(M) if km[m] < 0]
            u0 = 0
            while u0 < len(unmoved):
                u1 = u0
                while u1 + 1 < len(unmoved) and unmoved[u1 + 1] == unmoved[u1] + 1:
                    u1 += 1
                a0, ln = unmoved[u0], u1 - u0 + 1
                assert a0 + ln <= SP or a0 >= SPB
                ubase, umloc, ugs, ucs = pl_view(a0, ln, None)
                uobase, uomloc, uogs, uocs = out_view(a0, ln)
                nc.vector.tensor_copy(
                    _ap(uobase, uomloc, [[uocs, 3], [uogs, G], [1, ln]]),
                    _ap(ubase, umloc, [[ucs, 3], [ugs, G], [1, ln]]),
                )
                dma_out_cols(a0, ln, 0)
                u0 = u1 + 1

            # classes: chain-last single-run first (overlaps distribute)
            by_len = {}
            for (m0, ln, k) in runs:
                by_len.setdefault(ln, []).append((m0, k))
            classes = sorted(
                by_len.items(),
                key=lambda kv: 0 if (len(kv[1]) == 1 and kv[1][0][1] == K - 1)
                else 1)
            emitted_distribute = False
            ring = 1
            for ln, rs in classes:
                starts = [r[0] for r in rs]
                ks = [r[1] for r in rs]
                nr = len(rs)
                chain_last_single = nr == 1 and ks[0] == K - 1
                if not chain_last_single and not emitted_distribute:
                    emit_distribute()
                    emitted_distribute = True
                regular = nr <= 2 or (
                    all(starts[r] == starts[0] + r * (starts[1] - starts[0])
                        for r in range(nr))
                    and all(ks[r] == ks[0] + r * (ks[1] - ks[0])
                            for r in range(nr))
                )
                if regular:
                    apply_runs(starts, ln, ks)
                else:
                    for (m0, k) in rs:
                        apply_runs([m0], ln, [k])
                lo = min(starts)
                hi = max(s + ln for s in starts)
                dma_out_cols(lo, hi - lo, ring)
                ring ^= 1

    _split_multi_waits(nc)
    return nc


_BUILD_CACHE = {}


def make_in_maps(input, pos, angles, move_mask):
    input = np.asarray(input, dtype=np.float32)
    pos = np.asarray(pos, dtype=np.float32)
    angles = np.asarray(angles)
    N, K = input.shape
    M = pos.shape[1]
    NL = N // NCORES
    G = NL // P
    SP, SPB = _seg_bounds(angles, np.asarray(move_mask).astype(bool), M)
    arange_quads = bool((angles == np.arange(K * 4).reshape(K, 4)).all())
    in_maps = []
    for c in range(NCORES):
        sl = slice(c * NL, (c + 1) * NL)
        # (NL, M, 3) -> (P, 3, G, M): partition-major so each partition row
        # is one contiguous DMA descriptor
        pm = pos[sl].reshape(G, P, M, 3).transpose(1, 3, 0, 2)
        vrows = input[sl].reshape(G, P, K).transpose(1, 0, 2).reshape(P, G * K)
        im = {"vin": np.ascontiguousarray(vrows),
              "pivA": np.ascontiguousarray(
                  pm[:, :, :, :SP].reshape(P, 3 * G * SP))}
        if arange_quads:
            # pivP[p][c][q][g][k] = pm[p][c][g][4k+q]
            pp = pm[:, :, :, :4 * K].reshape(P, 3, G, K, 4)
            im["pivP"] = np.ascontiguousarray(
                pp.transpose(0, 1, 4, 2, 3).reshape(P, 12 * G * K))
        if SPB < M:
            im["posB"] = np.ascontiguousarray(pm[:, :, :, SPB:])
        in_maps.append(im)
    return in_maps


def kernel(input, pos, angles, move_mask):
    input = np.ascontiguousarray(np.asarray(input, dtype=np.float32))
    pos = np.ascontiguousarray(np.asarray(pos, dtype=np.float32))
    angles = np.asarray(angles)
    move_mask = np.asarray(move_mask).astype(bool)

    N, K = input.shape
    _, M, three = pos.shape
    assert three == 3
    assert N % (NCORES * P) == 0
    NL = N // NCORES

    key = (N, K, M, angles.tobytes(), move_mask.tobytes())
    nc = _BUILD_CACHE.get(key)
    if nc is None:
        nc = _build(angles, move_mask, NL, K, M)
        _BUILD_CACHE[key] = nc

    in_maps = make_in_maps(input, pos, angles, move_mask)

    # the axon-proxied NRT occasionally wedges transiently
    # (NRT_EXEC_UNIT_UNRECOVERABLE); one retry recovers it
    try:
        res = run_bass_kernel_spmd(nc, in_maps, list(range(NCORES)))
    except Exception:
        res = run_bass_kernel_spmd(nc, in_maps, list(range(NCORES)))

    out = np.empty((N, M, 3), dtype=np.float32)
    for c in range(NCORES):
        sl = slice(c * NL, (c + 1) * NL)
        o = res.results[c]["outT"]           # (P, 3, G, M)
        out[sl] = o.transpose(2, 0, 3, 1).reshape(NL, M, 3)
    return out



# revision 57
# speedup vs baseline: 1.0958x; 1.0460x over previous
"""Dihedral2Coord Trainium2 kernel.

Math: the reference applies K sequential dihedral-set steps; step k rotates
a suffix of the atom chain rigidly about the current J-K bond.  Every step's
transform is a proper rigid motion that moves all four pivot atoms of every
later step together, so the dihedral measured at application time equals the
dihedral of the ORIGINAL coordinates (dihedrals are invariant under rigid
motion).  Step k's rotation, expressed in original coordinates, is therefore
a fixed affine A_k computable from the original positions alone, and the
cumulative transform is the prefix product C_k = A_0 @ A_1 @ ... @ A_k.
The kernel:
  A) computes all K per-conformer Rodrigues affines in parallel,
  B) prefix-composes them with a blocked scan,
  C) applies C_{km(m)} to each atom run, where km(m) counts the steps whose
     mask includes atom m (verified prefix-structured on host).

Sharding: pure data parallelism over conformers N across 8 cores (SPMD).
"""

import sys

import numpy as np

try:
    import concourse.bass as bass
except ImportError:  # path in the grading container
    sys.path.insert(0, "/opt/trn_rl_repo")
    import concourse.bass as bass

import concourse.tile as tile
from concourse import mybir
from concourse.bass_utils import run_bass_kernel_spmd

f32 = mybir.dt.float32
i32 = mybir.dt.int32
Alu = mybir.AluOpType
Act = mybir.ActivationFunctionType

NCORES = 8
P = 128
TWO_PI = float(2.0 * np.pi)
_HALF_PI = float(np.pi / 2)

_WAIT_CAP = 1  # this walrus build rejects >1 sync-wait per instruction


def _register_const(nc, value, dtype=f32):
    """Register an activation-bias constant.  Written on the Activation
    engine from the framework's const-1.0 AP (ordered by Bass.__init__'s
    barrier); later ACT reads are same-engine program-ordered, so no extra
    barrier is needed."""
    if (dtype, value) in nc.const_aps.aps:
        return
    t = nc.alloc_sbuf_tensor(f"const-{dtype.name}-{value}", [128, 1], dtype)
    one = nc.const_aps.aps[(f32, 1.0)]
    nc.scalar.activation(t.ap(), one, Act.Identity, bias=0.0, scale=float(value))
    nc.const_aps.aps[(dtype, value)] = t.ap()


def _split_multi_waits(nc):
    """Split every instruction carrying >cap sync-waits into single-wait
    NoOps (same engine, immediately before, same block).  Waits are monotone
    semaphore conditions so this preserves semantics exactly."""
    n = 0
    for func in nc.m.functions:
        for bb in func.blocks:
            old = list(bb.instructions)
            if not any(
                i.sync_info is not None and len(i.sync_info.on_wait) > _WAIT_CAP
                for i in old
            ):
                continue
            new = []
            for inst in old:
                si = inst.sync_info
                if si is not None and len(si.on_wait) > _WAIT_CAP:
                    waits = list(si.on_wait)
                    head, tail = waits[:-_WAIT_CAP], waits[-_WAIT_CAP:]
                    for j in range(0, len(head), _WAIT_CAP):
                        n += 1
                        new.append(
                            mybir.InstNoOp(
                                name=f"{inst.name}_ws{j}",
                                engine=inst.engine,
                                sync_info=mybir.SyncInfo(
                                    on_wait=list(head[j : j + _WAIT_CAP]), on_update=[]
                                ),
                                bass_nofuse=True,
                            )
                        )
                    try:
                        si.on_wait[:] = tail
                    except TypeError:
                        inst.sync_info = mybir.SyncInfo(
                            on_wait=tail, on_update=list(si.on_update)
                        )
                new.append(inst)
            try:
                bb.instructions[:] = new
            except TypeError:
                bb.instructions = new
    return n


def _ap(base, offset_elems, dims):
    """Free-dim AP view into an SBUF tile AP `base` (partition dim kept).
    dims: list of [step, count] in elements of the tile's free space."""
    return bass.AP(
        tensor=base.tensor,
        offset=base.offset + offset_elems,
        ap=[list(base.ap[0])] + [list(d) for d in dims],
    )


def _dram_ap(t, offset, dims):
    return bass.AP(tensor=t.tensor, offset=offset, ap=[list(d) for d in dims])


def _analyse_mask(angles, move_mask):
    """Host-side structural analysis. Returns (km, runs): km[m] is the last
    step applied to atom m (-1 = never moved); runs are (start, len, k)."""
    K, M = move_mask.shape
    km = move_mask.astype(np.int64).sum(0) - 1
    kk = np.arange(K)[:, None]
    if not (move_mask == (kk <= km[None, :])).all():
        raise NotImplementedError("move_mask is not prefix-structured per atom")
    for k in range(K):
        for a in angles[k]:
            if not move_mask[:k, a].all():
                raise NotImplementedError("pivot atoms not rigidly co-moved")
    runs = []
    m = 0
    while m < M:
        j = m
        while j + 1 < M and km[j + 1] == km[m]:
            j += 1
        if km[m] >= 0:
            runs.append((m, j - m + 1, int(km[m])))
        m = j + 1
    return km, runs


def _seg_bounds(angles, move_mask, M):
    """(SP, SPB): pivot region [0, SP); B-tile starts at SPB <= SP so no
    run/unmoved segment crosses a tile boundary (columns [SPB, SP) are
    duplicated in both tiles)."""
    km, runs = _analyse_mask(angles, move_mask)
    SP = min(int(np.asarray(angles).max()) + 1, M)
    segs = [(m0, ln) for (m0, ln, _k) in runs]
    m = 0
    while m < M:
        if km[m] < 0:
            j = m
            while j + 1 < M and km[j + 1] < 0:
                j += 1
            segs.append((m, j - m + 1))
            m = j + 1
        else:
            m += 1
    SPB = SP
    for (m0, ln) in segs:
        if m0 < SP < m0 + ln:
            SPB = min(SPB, m0)
    return SP, SPB


def _build(angles, move_mask, NL, K, M):
    """Build the Bass module for one core handling NL conformers."""
    G = NL // P
    assert NL == G * P
    GK = G * K
    L = 8               # within-block scan length
    assert K % L == 0
    B = K // L          # blocks per conformer-group
    NB = G * B          # blocks over the flattened (g,k) axis

    angles = np.asarray(angles)
    arange_quads = bool((angles == np.arange(K * 4).reshape(K, 4)).all())
    km, runs = _analyse_mask(angles, move_mask)

    nc = bass.Bass()
    for cval in (1024.0, 1024.25, 1024.0 * TWO_PI, 1024.0 * TWO_PI + _HALF_PI):
        _register_const(nc, float(cval))
    SP, SPB = _seg_bounds(angles, move_mask, M)
    MB = M - SPB        # B-tile width
    vinD = nc.declare_dram_parameter("vin", [P, G * K], f32, isOutput=False)
    pivA = nc.declare_dram_parameter("pivA", [P, 3 * G * SP], f32,
                                     isOutput=False)
    # quad-permuted pivot planes: pivP[c][q][g][k] = pos[., 4k+q, c] so the
    # r-vector subtraction reads contiguously (innermost (g,k))
    pivPd = (nc.declare_dram_parameter("pivP", [P, 12 * G * K], f32,
                                       isOutput=False) if arange_quads
             else None)
    posB = (nc.declare_dram_parameter("posB", [P, 3, G, MB], f32,
                                      isOutput=False) if SPB < M else None)
    # outputs as whole tiles: one contiguous DMA descriptor per partition
    outAd = nc.declare_dram_parameter("outA", [P, 3 * G * SP], f32,
                                      isOutput=True)
    outBd = (nc.declare_dram_parameter("outB", [P, 3 * G * MB], f32,
                                       isOutput=True) if SPB < M else None)

    with tile.TileContext(nc) as tc:
        with tc.tile_pool(name="main", bufs=1) as pool:
            # ---- SBUF tensors ----
            # separate tiles per DMA so consumers wait only on what they need
            VIN = pool.tile([P, G * K], f32)
            PLA = pool.tile([P, 3 * G * SP], f32)
            PIVP = (pool.tile([P, 3, 4, GK], f32, name="PIVP")
                    if arange_quads else None)
            PLB = pool.tile([P, 3, G, MB], f32, name="PLB") if SPB < M else None
            OUTA = pool.tile([P, 3, G, SP], f32)
            OUTB = pool.tile([P, 3, G, MB], f32, name="OUTB") if SPB < M else None
            # packed r-vectors / normals with duplicated xy components so a
            # +1/+2 component rotation is a plain offset (cross-product trick)
            RV = pool.tile([P, 3, 5, G, K], f32)  # (rIJ,rJK,rKL) x (x,y,z,x,y)
            NN = pool.tile([P, 3, 5, G, K], f32)  # (nIJK,nJKL,m) x (x,y,z,x,y)
            TA = pool.tile([P, 2, 3, G, K], f32)
            TB = pool.tile([P, 2, 3, G, K], f32)
            AT = pool.tile([P, 12, G, K], f32)   # A_k; q=4i+j, strides q:GK, g:K, k:1
            CT = pool.tile([P, 12, GK], f32)     # full prefixes
            NBP = NB + 4   # 4 pad columns so Hillis j<s lanes read in-bounds
            PT = pool.tile([P, 12, NBP], f32)    # block products / prefixes
            PT2 = pool.tile([P, 12, NBP], f32)   # Hillis ping-pong buffer
            PTB = pool.tile([P, G, 12, (K // 8) * 7], f32)  # prefixes bcast over t
            ACC = pool.tile([P, 12 * max(GK, 64)], f32)
            AC2 = pool.tile([P, 12 * max(GK, 64)], f32)
            AC3 = pool.tile([P, 12 * max(GK, 64)], f32)

            vv = _ap(VIN[:, :], 0, [[K, G], [1, K]])
            pla = _ap(PLA[:, :], 0, [])
            pivp = PIVP[:, :, :, :] if PIVP is not None else None
            plb = PLB[:, :, :, :] if PLB is not None else None
            outa = OUTA[:, :, :, :]
            outb = OUTB[:, :, :, :] if OUTB is not None else None

            def pl_view(m0, ln, _unused=None):
                """(base_ap, local column offset, group stride, comp stride)
                for columns [m0, m0+ln): B tile from SPB, else A tile."""
                if m0 >= SPB:
                    return plb, m0 - SPB, MB, G * MB
                assert m0 + ln <= SP
                return pla, m0, SP, G * SP

            def out_view(m0, ln):
                if m0 >= SPB:
                    return outb, m0 - SPB, MB, G * MB
                assert m0 + ln <= SP
                return outa, m0, SP, G * SP
            rv = RV[:, :, :, :, :]
            nn = NN[:, :, :, :, :]
            t1v = TA[:, :, :, :, :]
            t2v = TB[:, :, :, :, :]
            at = AT[:, :, :, :]
            ct = CT[:, :, :]
            pt = _ap(PT[:, :, :], 4, [[NBP, 12], [1, NB]])
            pt2 = _ap(PT2[:, :, :], 4, [[NBP, 12], [1, NB]])
            ptb = PTB[:, :, :, :]
            acc = ACC[:, :]
            ac2 = AC2[:, :]
            ac3 = AC3[:, :]

            RVv, RVc = 5 * GK, GK   # RV strides: vec, comp
            NVv = 5 * GK

            # ---- DMA in ----
            # All on the sync ring, in priority order: pivP (gates stage A),
            # vin (gates the ACT sin chain), pivA (gates pJ copy + A-apply),
            # posB (gates the B-tile apply, late).  Host arrays are
            # partition-major so each row is one contiguous descriptor.
            if pivp is not None:
                nc.sync.dma_start(
                    out=_ap(pivp, 0, [[1, 12 * GK]]),
                    in_=_dram_ap(pivPd[:, :], 0, [[12 * GK, P], [1, 12 * GK]]),
                )
            nc.sync.dma_start(
                out=_ap(vv, 0, [[1, GK]]),
                in_=_dram_ap(vinD[:, :], 0, [[GK, P], [1, GK]]),
            )
            nc.sync.dma_start(
                out=_ap(pla, 0, [[1, 3 * G * SP]]),
                in_=_dram_ap(pivA[:, :], 0, [[3 * G * SP, P], [1, 3 * G * SP]]),
            )
            if PLB is not None:
                nc.sync.dma_start(
                    out=_ap(plb, 0, [[1, 3 * G * MB]]),
                    in_=_dram_ap(posB[:, :, :, :], 0,
                                 [[3 * G * MB, P], [1, 3 * G * MB]]),
                )
            # Hillis pad columns must hold finite values (they feed the
            # patched lanes); zero them before the block-prefix scan
            nc.gpsimd.memset(_ap(PT[:, :, :], 0, [[NBP, 12], [1, 4]]), 0.0)
            nc.gpsimd.memset(_ap(PT2[:, :, :], 0, [[NBP, 12], [1, 4]]), 0.0)

            # ---- helpers ----
            tmp_idx = [0]

            def T(dt=f32):
                tmp_idx[0] += 1
                return pool.tile([P, G, K], dt, name=f"tmp{tmp_idx[0]}")

            def mul(a, b):
                o = T(); nc.vector.tensor_mul(o, a, b); return o

            def add(a, b):
                o = T(); nc.vector.tensor_add(o, a, b); return o

            def sub(a, b):
                o = T(); nc.vector.tensor_sub(o, a, b); return o

            def aff(a, scale, bias):
                o = T()
                nc.scalar.activation(o, a, Act.Identity, bias=bias, scale=scale)
                return o

            def activ(a, fn):
                o = T(); nc.scalar.activation(o, a, fn); return o

            def dot3v(a_base, a_off, a_cs, b_base, b_off, b_cs, eng=None):
                """dot over xyz comps via one mul + one innermost-reduce.
                a/b given as (tile_ap, elem offset, comp stride); both must
                have gk contiguous (stride 1)."""
                tmp_idx[0] += 1
                dp = pool.tile([P, GK, 3], f32, name=f"dp{tmp_idx[0]}")[:, :, :]
                (eng or nc.vector).tensor_mul(
                    dp,
                    _ap(a_base, a_off, [[1, GK], [a_cs, 3]]),
                    _ap(b_base, b_off, [[1, GK], [b_cs, 3]]),
                )
                o = T()
                nc.vector.tensor_reduce(
                    _ap(o, 0, [[1, GK]]), dp, mybir.AxisListType.X, Alu.add)
                return o

            # ---- pivot sources ----
            if not arange_quads:
                PIV = pool.tile([P, 3, G, 4, K], f32)
                pv = PIV[:, :, :, :, :]
                for k in range(K):
                    for q in range(4):
                        nc.vector.tensor_copy(
                            _ap(pv, q * K + k, [[G * 4 * K, 3], [4 * K, G]]),
                            _ap(pla, int(angles[k, q]),
                                [[G * SP, 3], [SP, G]]),
                        )

            def piv_ap(c, q):
                if arange_quads:
                    return _ap(pivp, c * 4 * GK + q * GK, [[K, G], [1, K]])
                return _ap(pv, c * G * 4 * K + q * K, [[4 * K, G], [1, K]])

            pJ = [piv_ap(c, 1) for c in range(3)]

            def _ap_cat3(_pj):
                # the three pJ views share a regular comp stride; rebuild as
                # one 3-dim AP [c][g][k]
                if arange_quads:
                    return _ap(pivp, GK, [[4 * GK, 3], [1, GK]])
                return _ap(pv, K, [[G * 4 * K, 3], [4 * K, G], [1, K]])

            # ---- stage A: packed r-vectors and cross products ----
            if arange_quads:
                # quad-permuted pivots: v-dim is the q axis, (g,k) contiguous
                nc.vector.tensor_sub(
                    _ap(rv, 0, [[RVv, 3], [RVc, 3], [1, GK]]),
                    _ap(pivp, GK, [[GK, 3], [4 * GK, 3], [1, GK]]),
                    _ap(pivp, 0, [[GK, 3], [4 * GK, 3], [1, GK]]))
            else:
                for g in range(G):
                    in1 = _ap(pv, g * 4 * K + K,
                              [[K, 3], [G * 4 * K, 3], [1, K]])
                    in0 = _ap(pv, g * 4 * K + 0,
                              [[K, 3], [G * 4 * K, 3], [1, K]])
                    nc.vector.tensor_sub(
                        _ap(rv, g * K, [[RVv, 3], [RVc, 3], [1, K]]), in1, in0)
            # duplicate comps x,y into slots 3,4
            nc.vector.tensor_copy(
                _ap(rv, 3 * RVc, [[RVv, 3], [RVc, 2], [1, GK]]),
                _ap(rv, 0, [[RVv, 3], [RVc, 2], [1, GK]]))
            # nIJK, nJKL = cross(A=[rIJ,rJK], B=[rJK,rKL]) via comp offsets
            nc.vector.tensor_mul(
                _ap(t1v, 0, [[3 * GK, 2], [GK, 3], [1, GK]]),
                _ap(rv, RVc, [[RVv, 2], [RVc, 3], [1, GK]]),
                _ap(rv, RVv + 2 * RVc, [[RVv, 2], [RVc, 3], [1, GK]]))
            nc.vector.tensor_mul(
                _ap(t2v, 0, [[3 * GK, 2], [GK, 3], [1, GK]]),
                _ap(rv, 2 * RVc, [[RVv, 2], [RVc, 3], [1, GK]]),
                _ap(rv, RVv + RVc, [[RVv, 2], [RVc, 3], [1, GK]]))
            nc.vector.tensor_sub(
                _ap(nn, 0, [[NVv, 2], [GK, 3], [1, GK]]),
                _ap(t1v, 0, [[3 * GK, 2], [GK, 3], [1, GK]]),
                _ap(t2v, 0, [[3 * GK, 2], [GK, 3], [1, GK]]))
            nc.vector.tensor_copy(
                _ap(nn, 3 * GK, [[NVv, 2], [GK, 2], [1, GK]]),
                _ap(nn, 0, [[NVv, 2], [GK, 2], [1, GK]]))
            # m = nIJK x rJK -> NN vec slot 2
            nc.vector.tensor_mul(
                _ap(t1v, 0, [[GK, 3], [1, GK]]),
                _ap(nn, GK, [[GK, 3], [1, GK]]),
                _ap(rv, RVv + 2 * RVc, [[RVc, 3], [1, GK]]))
            nc.vector.tensor_mul(
                _ap(t2v, 0, [[GK, 3], [1, GK]]),
                _ap(nn, 2 * GK, [[GK, 3], [1, GK]]),
                _ap(rv, RVv + RVc, [[RVc, 3], [1, GK]]))
            nc.vector.tensor_sub(
                _ap(nn, 2 * NVv, [[GK, 3], [1, GK]]),
                _ap(t1v, 0, [[GK, 3], [1, GK]]),
                _ap(t2v, 0, [[GK, 3], [1, GK]]))

            # pJ source: read straight out of pivP when available, else make
            # a compact copy
            if arange_quads:
                pj_b, pj_off, pj_cs = pivp, GK, 4 * GK
            else:
                PJC = pool.tile([P, 3, G, K], f32)
                pjc = PJC[:, :, :, :]
                nc.vector.tensor_copy(_ap(pjc, 0, [[GK, 3], [K, G], [1, K]]),
                                      _ap_cat3(pJ))
                pj_b, pj_off, pj_cs = pjc, 0, GK

            def rvec(v, c):
                return _ap(rv, v * RVv + c * RVc, [[K, G], [1, K]])

            def nvec(v, c):
                return _ap(nn, v * NVv + c * GK, [[K, G], [1, K]])

            rJK = [rvec(1, c) for c in range(3)]
            rjk_off = RVv                      # RV vec 1, comp stride RVc
            m_off = 2 * NVv                    # m lives in NN vec 2

            # paired dot products: one mul+reduce covers two dots that share
            # a left operand; results land adjacently for fused downstream ops
            DOTS = pool.tile([P, 4, GK], f32)  # rows: x0, l1^2, y0, lm^2
            DP4 = pool.tile([P, 2, GK, 3], f32)
            dots = DOTS[:, :, :]
            dp4 = DP4[:, :, :, :]
            # {x0, l1^2} = nIJK . (nJKL, nIJK)
            nc.vector.tensor_mul(
                dp4,
                _ap(nn, 0, [[0, 2], [1, GK], [GK, 3]]),
                _ap(nn, NVv, [[-NVv, 2], [1, GK], [GK, 3]]))
            nc.vector.tensor_reduce(
                _ap(dots, 0, [[GK, 2], [1, GK]]), dp4,
                mybir.AxisListType.X, Alu.add)
            # y0 = m . nJKL (single dot; reuse dp4's first GK*3 lane block)
            nc.vector.tensor_mul(
                _ap(dp4, 0, [[3, GK], [1, 3]]),
                _ap(nn, m_off, [[1, GK], [GK, 3]]),
                _ap(nn, NVv, [[1, GK], [GK, 3]]))
            nc.vector.tensor_reduce(
                _ap(dots, 2 * GK, [[1, GK]]),
                _ap(dp4, 0, [[3, GK], [1, 3]]),
                mybir.AxisListType.X, Alu.add)
            jks = dot3v(rv, rjk_off, RVc, rv, rjk_off, RVc)
            # lm^2 = l1^2 * |rJK|^2  (m = nIJK x rJK with nIJK _|_ rJK)
            nc.vector.tensor_mul(
                _ap(dots, 3 * GK, [[1, GK]]),
                _ap(dots, GK, [[1, GK]]),
                _ap(jks[:, :, :], 0, [[1, GK]]))
            L1LM = pool.tile([P, 2, GK], f32)  # (l1, lm)
            l1lm = L1LM[:, :, :]
            nc.scalar.activation(
                _ap(l1lm, 0, [[GK, 2], [1, GK]]),
                _ap(dots, GK, [[2 * GK, 2], [1, GK]]), Act.Sqrt)
            XY = pool.tile([P, 2, GK], f32)    # (x1, y1) = (x0*lm, y0*l1)
            xy = XY[:, :, :]
            nc.vector.tensor_mul(
                xy,
                _ap(dots, 0, [[2 * GK, 2], [1, GK]]),
                _ap(l1lm, GK, [[-GK, 2], [1, GK]]))
            SQ = pool.tile([P, 2, GK], f32)
            sq = SQ[:, :, :]
            nc.vector.tensor_mul(sq, xy, xy)
            hs = T()
            nc.vector.tensor_add(_ap(hs[:, :, :], 0, [[1, GK]]),
                                 _ap(sq, 0, [[1, GK]]),
                                 _ap(sq, GK, [[1, GK]]))
            hr = T(); nc.vector.reciprocal(hr, hs)
            rh = activ(hr, Act.Sqrt)            # 1/hypot
            CS = pool.tile([P, 2, GK], f32)     # (ccur, scur)
            cs_ = CS[:, :, :]
            nc.vector.tensor_mul(
                cs_, xy, _ap(rh[:, :, :], 0, [[0, 2], [1, GK]]))
            jkr = T(); nc.vector.reciprocal(jkr, jks)
            jrs = activ(jkr, Act.Sqrt)          # 1/|rJK|
            AXT = pool.tile([P, 3, G, K], f32)
            axt = AXT[:, :, :, :]
            nc.vector.tensor_mul(
                _ap(axt, 0, [[GK, 3], [1, GK]]),
                _ap(rv, rjk_off, [[RVc, 3], [1, GK]]),
                _ap(jrs[:, :, :], 0, [[0, 3], [1, GK]]),
            )
            ax = [_ap(axt, c * GK, [[K, G], [1, K]]) for c in range(3)]

            # sin/cos of targets with range reduction (Sin table ok |x|<~3.55)
            def reduced_sin(shift_quarter, extra):
                q = aff(vv, 1.0 / TWO_PI, 1024.0 + shift_quarter)
                qi = T(i32)
                nc.vector.tensor_copy(qi, q)     # f32->i32 rounds to nearest
                qf = T()
                nc.vector.tensor_copy(qf, qi)
                t = aff(qf, -TWO_PI, 1024.0 * TWO_PI + extra)
                return activ(add(vv, t), Act.Sin)

            sv = reduced_sin(0.0, 0.0)
            cv = reduced_sin(0.25, _HALF_PI)

            PC1 = pool.tile([P, 2, GK], f32)   # cv * (ccur, scur)
            PC2 = pool.tile([P, 2, GK], f32)   # sv * (ccur, scur)
            pc1 = PC1[:, :, :]
            pc2 = PC2[:, :, :]
            nc.vector.tensor_mul(pc1, cs_, _ap(cv[:, :, :], 0, [[0, 2], [1, GK]]))
            nc.vector.tensor_mul(pc2, cs_, _ap(sv[:, :, :], 0, [[0, 2], [1, GK]]))
            c_ = T()
            s_ = T()
            nc.vector.tensor_add(_ap(c_[:, :, :], 0, [[1, GK]]),
                                 _ap(pc1, 0, [[1, GK]]), _ap(pc2, GK, [[1, GK]]))
            nc.vector.tensor_sub(_ap(s_[:, :, :], 0, [[1, GK]]),
                                 _ap(pc2, 0, [[1, GK]]), _ap(pc1, GK, [[1, GK]]))
            t1_ = T()
            nc.vector.tensor_scalar(t1_, c_, -1.0, 1.0, Alu.mult, Alu.add)  # 1-cos

            TAX = pool.tile([P, 3, G, K], f32)
            SAX = pool.tile([P, 3, G, K], f32)
            UD = pool.tile([P, 3, G, K], f32)
            OD = pool.tile([P, 2, G, K], f32)
            taxv = TAX[:, :, :, :]
            saxv = SAX[:, :, :, :]
            udv = UD[:, :, :, :]
            odv = OD[:, :, :, :]
            d3 = [[GK, 3], [1, GK]]
            bc3 = [[0, 3], [1, GK]]
            nc.vector.tensor_mul(_ap(taxv, 0, d3), _ap(axt, 0, d3),
                                 _ap(t1_[:, :, :], 0, bc3))
            nc.vector.tensor_mul(_ap(saxv, 0, d3), _ap(axt, 0, d3),
                                 _ap(s_[:, :, :], 0, bc3))
            nc.vector.tensor_mul(_ap(udv, 0, d3), _ap(taxv, 0, d3),
                                 _ap(axt, 0, d3))

            def aq(q):
                return _ap(at, q * GK, [[K, G], [1, K]])

            # diagonal: q = 0,5,10 -> stride 5*GK
            nc.vector.tensor_add(
                _ap(at, 0, [[5 * GK, 3], [1, GK]]),
                _ap(udv, 0, d3),
                _ap(c_[:, :, :], 0, bc3),
            )
            # off-diagonal products: txy,txz = tax0*(ax1,ax2); tyz = tax1*ax2
            nc.vector.tensor_mul(
                _ap(odv, 0, [[GK, 2], [1, GK]]),
                _ap(axt, GK, [[GK, 2], [1, GK]]),
                _ap(taxv, 0, [[0, 2], [1, GK]]),
            )
            tyz = T()
            nc.vector.tensor_mul(tyz, _ap(taxv, GK, [[K, G], [1, K]]),
                                 _ap(axt, 2 * GK, [[K, G], [1, K]]))
            txy = _ap(odv, 0, [[K, G], [1, K]])
            txz = _ap(odv, GK, [[K, G], [1, K]])
            sax = [_ap(saxv, c * GK, [[K, G], [1, K]]) for c in range(3)]
            nc.vector.tensor_sub(aq(1), txy, sax[2])
            nc.vector.tensor_add(aq(4), txy, sax[2])
            nc.vector.tensor_add(aq(2), txz, sax[1])
            nc.vector.tensor_sub(aq(8), txz, sax[1])
            nc.vector.tensor_sub(aq(6), tyz, sax[0])
            nc.vector.tensor_add(aq(9), tyz, sax[0])

            # b = pJ - R @ pJ : batched products, reduce, sub
            BP = pool.tile([P, 3, GK, 3], f32)
            bp = BP[:, :, :, :]
            nc.vector.tensor_mul(
                bp,
                _ap(at, 0, [[4 * GK, 3], [1, GK], [GK, 3]]),
                _ap(pj_b, pj_off, [[0, 3], [1, GK], [pj_cs, 3]]),
            )
            RPJ = pool.tile([P, 3, G, K], f32)
            rpj = RPJ[:, :, :, :]
            nc.vector.tensor_reduce(
                _ap(rpj, 0, [[GK, 3], [1, GK]]), bp,
                mybir.AxisListType.X, Alu.add)
            nc.vector.tensor_sub(
                _ap(at, 3 * GK, [[4 * GK, 3], [1, GK]]),
                _ap(pj_b, pj_off, [[pj_cs, 3], [1, GK]]),
                _ap(rpj, 0, [[GK, 3], [1, GK]]),
            )

            # ---- stage B: blocked prefix composition ----
            at_flat = _ap(at, 0, [[GK, 12], [1, GK]])

            def compose(dst, dq, dbd, doff, left, lq, lbd, loff,
                        right, rq, rbd, roff):
                """dst[i,j,*] = sum_m left[i,m,*]*right[m,j,*]; dst[i,3,*] +=
                left[i,3,*].  *bd = batch [step,count] dims (equal counts)."""
                counts = [d[1] for d in dbd]
                assert [d[1] for d in lbd] == counts
                assert [d[1] for d in rbd] == counts
                nb = 1
                for cnt in counts:
                    nb *= cnt
                abd = []
                stp = 1
                for cnt in reversed(counts):
                    abd.insert(0, [stp, cnt])
                    stp *= cnt

                def accv(base):
                    return _ap(base, 0, [[4 * nb, 3], [nb, 4]] + abd)

                dstv = _ap(dst, doff, [[4 * dq, 3], [dq, 4]] + dbd)

                def dmul(tgt, mrow):
                    nc.vector.tensor_mul(
                        accv(tgt),
                        _ap(right, roff + 4 * mrow * rq,
                            [[0, 3], [rq, 4]] + rbd),
                        _ap(left, loff + mrow * lq,
                            [[4 * lq, 3], [0, 4]] + lbd),
                    )

                dmul(acc, 0)
                dmul(ac2, 1)
                nc.vector.tensor_add(accv(acc), accv(acc), accv(ac2))
                dmul(ac2, 2)
                nc.vector.tensor_add(dstv, accv(acc), accv(ac2))
                bias_d = _ap(dst, doff + 3 * dq, [[4 * dq, 3]] + dbd)
                nc.vector.tensor_add(
                    bias_d, bias_d,
                    _ap(left, loff + 3 * lq, [[4 * lq, 3]] + lbd),
                )

            # within-block scan, in place: A[:, t] <- A[:, t-1] o A[:, t]
            # (the 3 muls read the slot before the final add overwrites it)
            for t in range(1, L):
                compose(at_flat, GK, [[L, NB]], t,
                        at_flat, GK, [[L, NB]], t - 1,
                        at_flat, GK, [[L, NB]], t)
            # block products
            nc.vector.tensor_copy(
                _ap(pt, 0, [[NBP, 12], [1, NB]]),
                _ap(at_flat, L - 1, [[GK, 12], [L, NB]]),
            )
            # block-prefix scan: Hillis-Steele over the flattened (g,b) axis.
            # Lanes j%B < s read the neighbour's tail (garbage) and are
            # patched by the trailing copy before the buffers swap.
            src_pt, dst_pt = pt, pt2
            s = 1
            while s < B:
                compose(dst_pt, NBP, [[1, NB]], 0,
                        src_pt, NBP, [[1, NB]], -s,
                        src_pt, NBP, [[1, NB]], 0)
                nc.vector.tensor_copy(
                    _ap(dst_pt, 0, [[NBP, 12], [B, G], [1, s]]),
                    _ap(src_pt, 0, [[NBP, 12], [B, G], [1, s]]))
                src_pt, dst_pt = dst_pt, src_pt
                s *= 2
            ptf = src_pt

            # ---- stage C ----
            def dma_out_tile(which):
                """DMA a whole output tile; Tile's dependency tracking makes
                it wait for every writer of that tile."""
                if which == "A":
                    nc.scalar.dma_start(
                        out=_dram_ap(outAd[:, :], 0,
                                     [[3 * G * SP, P], [1, 3 * G * SP]]),
                        in_=_ap(outa, 0, [[1, 3 * G * SP]]),
                    )
                else:
                    nc.scalar.dma_start(
                        out=_dram_ap(outBd[:, :], 0,
                                     [[3 * G * MB, P], [1, 3 * G * MB]]),
                        in_=_ap(outb, 0, [[1, 3 * G * MB]]),
                    )

            def apply_single_from(coef, coefq, coefoff, m0, length):
                """out[:, :, m0:m0+length] = R@p + b with per-(partition,g)
                scalar coefficients from `coef` (q stride coefq, g stride
                coefoff).  Muls on ACT (per-partition scale), adds on DVE."""
                plbase, mloc, gs, cs = pl_view(m0, length, None)
                obase, omloc, ogs, ocs = out_view(m0, length)
                tmp_idx[0] += 1
                prod = [[pool.tile([P, G * length], f32,
                                   name=f"prod{tmp_idx[0]}_{i}_{cc}")[:, :]
                         for cc in range(3)] for i in range(3)]
                for i in range(3):
                    for cc in range(3):
                        for g in range(G):
                            nc.scalar.activation(
                                _ap(prod[i][cc], g * length, [[1, length]]),
                                _ap(plbase, cc * cs + g * gs + mloc,
                                    [[1, length]]),
                                Act.Identity,
                                scale=_ap(coef, (4 * i + cc) * coefq
                                          + g * coefoff, [[1, 1]]),
                            )
                for i in range(3):
                    d_t = [[length, G], [1, length]]
                    s1 = _ap(prod[i][0], 0, d_t)
                    nc.vector.tensor_add(s1, s1, _ap(prod[i][1], 0, d_t))
                    nc.vector.tensor_add(s1, s1, _ap(prod[i][2], 0, d_t))
                    for g in range(G):
                        # + translation via the ACT bias port (keeps DVE free)
                        nc.scalar.activation(
                            _ap(obase, i * ocs + g * ogs + omloc, [[1, length]]),
                            _ap(prod[i][0], g * length, [[1, length]]),
                            Act.Identity,
                            bias=_ap(coef, (4 * i + 3) * coefq + g * coefoff,
                                     [[1, 1]]),
                            scale=1.0,
                        )

            pt_last = bass.AP(tensor=ptf.tensor, offset=ptf.offset + (B - 1),
                              ap=list(ptf.ap))

            def apply_runs(starts, length, ks):
                nr = len(starts)
                if nr == 1 and ks[0] == K - 1:
                    # chain-last prefix == last block product: ready right
                    # after the block-prefix scan, before distribute.
                    apply_single_from(pt_last, NBP, B, starts[0], length)
                    return
                if nr == 1:
                    base = bass.AP(tensor=ct.tensor, offset=ct.offset + ks[0],
                                   ap=list(ct.ap))
                    apply_single_from(base, GK, K, starts[0], length)
                    return
                sm = starts[1] - starts[0]
                sk = ks[1] - ks[0]
                m0, k0 = starts[0], ks[0]
                span = max(starts) + length - m0
                plbase, mloc, gs, cs = pl_view(m0, span, None)
                obase, omloc, ogs, ocs = out_view(m0, span)
                d_pl = [[gs, G], [sm, nr], [1, length]]
                d_out = [[ogs, G], [sm, nr], [1, length]]
                d_c = [[K, G], [sk, nr], [0, length]]
                d_acc = [[nr * length, G], [length, nr], [1, length]]
                for i in range(3):
                    for cc in range(2):
                        tgt = acc if cc == 0 else ac2
                        nc.vector.tensor_mul(
                            _ap(tgt, 0, d_acc),
                            _ap(plbase, cc * cs + mloc, d_pl),
                            _ap(ct, (4 * i + cc) * GK + k0, d_c),
                        )
                    nc.vector.tensor_add(
                        _ap(acc, 0, d_acc), _ap(acc, 0, d_acc), _ap(ac2, 0, d_acc)
                    )
                    nc.vector.tensor_mul(
                        _ap(ac2, 0, d_acc),
                        _ap(plbase, 2 * cs + mloc, d_pl),
                        _ap(ct, (4 * i + 2) * GK + k0, d_c),
                    )
                    nc.vector.tensor_add(
                        _ap(acc, 0, d_acc), _ap(acc, 0, d_acc),
                        _ap(ac2, 0, d_acc),
                    )
                    nc.vector.tensor_add(
                        _ap(obase, i * ocs + omloc, d_out),
                        _ap(acc, 0, d_acc),
                        _ap(ct, (4 * i + 3) * GK + k0, d_c),
                    )

            def emit_distribute():
                # distribute: block 0 copies, blocks b>=1 get P[b-1] @ W
                nk = (B - 1) * L
                nc.vector.tensor_copy(
                    _ap(ct, 0, [[GK, 12], [K, G], [1, L]]),
                    _ap(at_flat, 0, [[GK, 12], [K, G], [1, L]]),
                )
                # broadcast block prefixes over t so g fuses into 3-dim APs:
                # PTB[g][q][jb*L + t] = ptf[q][g*B + jb]
                for g in range(G):
                    nc.vector.tensor_copy(
                        _ap(ptb, g * 12 * nk, [[nk, 12], [L, B - 1], [1, L]]),
                        _ap(ptf, g * B, [[NBP, 12], [1, B - 1], [0, L]]))
                d_w = [[GK, 4], [K, G], [1, nk]]
                d_a = [[G * nk, 4], [nk, G], [1, nk]]
                accs = (acc, ac2, ac3)
                # all 9 cross products first (pure reads of W and P), then
                # the combines
                for m in range(3):
                    for i in range(3):
                        nc.vector.tensor_mul(
                            _ap(accs[m], i * 4 * G * nk, d_a),
                            _ap(at_flat, 4 * m * GK + L, d_w),
                            _ap(ptb, (4 * i + m) * nk,
                                [[0, 4], [12 * nk, G], [1, nk]]),
                        )
                for i in range(3):
                    o = i * 4 * G * nk
                    nc.vector.tensor_add(
                        _ap(acc, o, d_a), _ap(acc, o, d_a), _ap(ac2, o, d_a))
                    nc.vector.tensor_add(
                        _ap(ct, 4 * i * GK + L, d_w),
                        _ap(acc, o, d_a), _ap(ac3, o, d_a))
                    nc.vector.tensor_add(
                        _ap(ct, (4 * i + 3) * GK + L, [[K, G], [1, nk]]),
                        _ap(ct, (4 * i + 3) * GK + L, [[K, G], [1, nk]]),
                        _ap(ptb, (4 * i + 3) * nk, [[12 * nk, G], [1, nk]]),
                    )

            # unmoved atoms: copy + DMA as soon as PL lands
            unmoved = [m for m in range(M) if km[m] < 0]
            u0 = 0
            while u0 < len(unmoved):
                u1 = u0
                while u1 + 1 < len(unmoved) and unmoved[u1 + 1] == unmoved[u1] + 1:
                    u1 += 1
                a0, ln = unmoved[u0], u1 - u0 + 1
                assert a0 + ln <= SP or a0 >= SPB
                ubase, umloc, ugs, ucs = pl_view(a0, ln, None)
                uobase, uomloc, uogs, uocs = out_view(a0, ln)
                nc.vector.tensor_copy(
                    _ap(uobase, uomloc, [[uocs, 3], [uogs, G], [1, ln]]),
                    _ap(ubase, umloc, [[ucs, 3], [ugs, G], [1, ln]]),
                )
                u0 = u1 + 1

            # classes: chain-last single-run first (overlaps distribute)
            by_len = {}
            for (m0, ln, k) in runs:
                by_len.setdefault(ln, []).append((m0, k))
            classes = sorted(
                by_len.items(),
                key=lambda kv: 0 if (len(kv[1]) == 1 and kv[1][0][1] == K - 1)
                else 1)
            emitted_distribute = False
            for ln, rs in classes:
                starts = [r[0] for r in rs]
                ks = [r[1] for r in rs]
                nr = len(rs)
                chain_last_single = nr == 1 and ks[0] == K - 1
                if not chain_last_single and not emitted_distribute:
                    emit_distribute()
                    emitted_distribute = True
                regular = nr <= 2 or (
                    all(starts[r] == starts[0] + r * (starts[1] - starts[0])
                        for r in range(nr))
                    and all(ks[r] == ks[0] + r * (ks[1] - ks[0])
                            for r in range(nr))
                )
                if regular:
                    apply_runs(starts, ln, ks)
                else:
                    for (m0, k) in rs:
                        apply_runs([m0], ln, [k])
            # B tile drains first (its writers finish early); HWDGE is FIFO
            # per engine so the early DMA must be queued first
            if OUTB is not None:
                dma_out_tile("B")
            dma_out_tile("A")

    _split_multi_waits(nc)
    return nc


_BUILD_CACHE = {}


def make_in_maps(input, pos, angles, move_mask):
    input = np.asarray(input, dtype=np.float32)
    pos = np.asarray(pos, dtype=np.float32)
    angles = np.asarray(angles)
    N, K = input.shape
    M = pos.shape[1]
    NL = N // NCORES
    G = NL // P
    SP, SPB = _seg_bounds(angles, np.asarray(move_mask).astype(bool), M)
    arange_quads = bool((angles == np.arange(K * 4).reshape(K, 4)).all())
    in_maps = []
    for c in range(NCORES):
        sl = slice(c * NL, (c + 1) * NL)
        # (NL, M, 3) -> (P, 3, G, M): partition-major so each partition row
        # is one contiguous DMA descriptor
        pm = pos[sl].reshape(G, P, M, 3).transpose(1, 3, 0, 2)
        vrows = input[sl].reshape(G, P, K).transpose(1, 0, 2).reshape(P, G * K)
        im = {"vin": np.ascontiguousarray(vrows),
              "pivA": np.ascontiguousarray(
                  pm[:, :, :, :SP].reshape(P, 3 * G * SP))}
        if arange_quads:
            # pivP[p][c][q][g][k] = pm[p][c][g][4k+q]
            pp = pm[:, :, :, :4 * K].reshape(P, 3, G, K, 4)
            im["pivP"] = np.ascontiguousarray(
                pp.transpose(0, 1, 4, 2, 3).reshape(P, 12 * G * K))
        if SPB < M:
            im["posB"] = np.ascontiguousarray(pm[:, :, :, SPB:])
        in_maps.append(im)
    return in_maps


def kernel(input, pos, angles, move_mask):
    input = np.ascontiguousarray(np.asarray(input, dtype=np.float32))
    pos = np.ascontiguousarray(np.asarray(pos, dtype=np.float32))
    angles = np.asarray(angles)
    move_mask = np.asarray(move_mask).astype(bool)

    N, K = input.shape
    _, M, three = pos.shape
    assert three == 3
    assert N % (NCORES * P) == 0
    NL = N // NCORES

    key = (N, K, M, angles.tobytes(), move_mask.tobytes())
    nc = _BUILD_CACHE.get(key)
    if nc is None:
        nc = _build(angles, move_mask, NL, K, M)
        _BUILD_CACHE[key] = nc

    in_maps = make_in_maps(input, pos, angles, move_mask)

    # the axon-proxied NRT occasionally wedges transiently
    # (NRT_EXEC_UNIT_UNRECOVERABLE); one retry recovers it
    try:
        res = run_bass_kernel_spmd(nc, in_maps, list(range(NCORES)))
    except Exception:
        res = run_bass_kernel_spmd(nc, in_maps, list(range(NCORES)))

    G = NL // P
    SP, SPB = _seg_bounds(angles, move_mask, M)
    out = np.empty((N, M, 3), dtype=np.float32)
    for c in range(NCORES):
        sl = slice(c * NL, (c + 1) * NL)
        # (P, 3, G, cols) -> (NL, cols, 3)
        oa = res.results[c]["outA"].reshape(P, 3, G, SP)
        out[sl, :SPB] = oa.transpose(2, 0, 3, 1).reshape(NL, SP, 3)[:, :SPB]
        if SPB < M:
            ob = res.results[c]["outB"].reshape(P, 3, G, M - SPB)
            out[sl, SPB:] = ob.transpose(2, 0, 3, 1).reshape(NL, M - SPB, 3)
    return out



# revision 58
# speedup vs baseline: 1.1005x; 1.0044x over previous
"""Dihedral2Coord Trainium2 kernel.

Math: the reference applies K sequential dihedral-set steps; step k rotates
a suffix of the atom chain rigidly about the current J-K bond.  Every step's
transform is a proper rigid motion that moves all four pivot atoms of every
later step together, so the dihedral measured at application time equals the
dihedral of the ORIGINAL coordinates (dihedrals are invariant under rigid
motion).  Step k's rotation, expressed in original coordinates, is therefore
a fixed affine A_k computable from the original positions alone, and the
cumulative transform is the prefix product C_k = A_0 @ A_1 @ ... @ A_k.
The kernel:
  A) computes all K per-conformer Rodrigues affines in parallel,
  B) prefix-composes them with a blocked scan,
  C) applies C_{km(m)} to each atom run, where km(m) counts the steps whose
     mask includes atom m (verified prefix-structured on host).

Sharding: pure data parallelism over conformers N across 8 cores (SPMD).
"""

import sys

import numpy as np

try:
    import concourse.bass as bass
except ImportError:  # path in the grading container
    sys.path.insert(0, "/opt/trn_rl_repo")
    import concourse.bass as bass

import concourse.tile as tile
from concourse import mybir
from concourse.bass_utils import run_bass_kernel_spmd

f32 = mybir.dt.float32
i32 = mybir.dt.int32
Alu = mybir.AluOpType
Act = mybir.ActivationFunctionType

NCORES = 8
P = 128
TWO_PI = float(2.0 * np.pi)
_HALF_PI = float(np.pi / 2)

_WAIT_CAP = 1  # this walrus build rejects >1 sync-wait per instruction


def _register_const(nc, value, dtype=f32):
    """Register an activation-bias constant.  Written on the Activation
    engine from the framework's const-1.0 AP (ordered by Bass.__init__'s
    barrier); later ACT reads are same-engine program-ordered, so no extra
    barrier is needed."""
    if (dtype, value) in nc.const_aps.aps:
        return
    t = nc.alloc_sbuf_tensor(f"const-{dtype.name}-{value}", [128, 1], dtype)
    one = nc.const_aps.aps[(f32, 1.0)]
    nc.scalar.activation(t.ap(), one, Act.Identity, bias=0.0, scale=float(value))
    nc.const_aps.aps[(dtype, value)] = t.ap()


def _split_multi_waits(nc):
    """Split every instruction carrying >cap sync-waits into single-wait
    NoOps (same engine, immediately before, same block).  Waits are monotone
    semaphore conditions so this preserves semantics exactly."""
    n = 0
    for func in nc.m.functions:
        for bb in func.blocks:
            old = list(bb.instructions)
            if not any(
                i.sync_info is not None and len(i.sync_info.on_wait) > _WAIT_CAP
                for i in old
            ):
                continue
            new = []
            for inst in old:
                si = inst.sync_info
                if si is not None and len(si.on_wait) > _WAIT_CAP:
                    waits = list(si.on_wait)
                    head, tail = waits[:-_WAIT_CAP], waits[-_WAIT_CAP:]
                    for j in range(0, len(head), _WAIT_CAP):
                        n += 1
                        new.append(
                            mybir.InstNoOp(
                                name=f"{inst.name}_ws{j}",
                                engine=inst.engine,
                                sync_info=mybir.SyncInfo(
                                    on_wait=list(head[j : j + _WAIT_CAP]), on_update=[]
                                ),
                                bass_nofuse=True,
                            )
                        )
                    try:
                        si.on_wait[:] = tail
                    except TypeError:
                        inst.sync_info = mybir.SyncInfo(
                            on_wait=tail, on_update=list(si.on_update)
                        )
                new.append(inst)
            try:
                bb.instructions[:] = new
            except TypeError:
                bb.instructions = new
    return n


def _ap(base, offset_elems, dims):
    """Free-dim AP view into an SBUF tile AP `base` (partition dim kept).
    dims: list of [step, count] in elements of the tile's free space."""
    return bass.AP(
        tensor=base.tensor,
        offset=base.offset + offset_elems,
        ap=[list(base.ap[0])] + [list(d) for d in dims],
    )


def _dram_ap(t, offset, dims):
    return bass.AP(tensor=t.tensor, offset=offset, ap=[list(d) for d in dims])


def _analyse_mask(angles, move_mask):
    """Host-side structural analysis. Returns (km, runs): km[m] is the last
    step applied to atom m (-1 = never moved); runs are (start, len, k)."""
    K, M = move_mask.shape
    km = move_mask.astype(np.int64).sum(0) - 1
    kk = np.arange(K)[:, None]
    if not (move_mask == (kk <= km[None, :])).all():
        raise NotImplementedError("move_mask is not prefix-structured per atom")
    for k in range(K):
        for a in angles[k]:
            if not move_mask[:k, a].all():
                raise NotImplementedError("pivot atoms not rigidly co-moved")
    runs = []
    m = 0
    while m < M:
        j = m
        while j + 1 < M and km[j + 1] == km[m]:
            j += 1
        if km[m] >= 0:
            runs.append((m, j - m + 1, int(km[m])))
        m = j + 1
    return km, runs


def _seg_bounds(angles, move_mask, M):
    """(SP, SPB): pivot region [0, SP); B-tile starts at SPB <= SP so no
    run/unmoved segment crosses a tile boundary (columns [SPB, SP) are
    duplicated in both tiles)."""
    km, runs = _analyse_mask(angles, move_mask)
    SP = min(int(np.asarray(angles).max()) + 1, M)
    segs = [(m0, ln) for (m0, ln, _k) in runs]
    m = 0
    while m < M:
        if km[m] < 0:
            j = m
            while j + 1 < M and km[j + 1] < 0:
                j += 1
            segs.append((m, j - m + 1))
            m = j + 1
        else:
            m += 1
    SPB = SP
    for (m0, ln) in segs:
        if m0 < SP < m0 + ln:
            SPB = min(SPB, m0)
    return SP, SPB


def _build(angles, move_mask, NL, K, M):
    """Build the Bass module for one core handling NL conformers."""
    G = NL // P
    assert NL == G * P
    GK = G * K
    L = 8               # within-block scan length
    assert K % L == 0
    B = K // L          # blocks per conformer-group
    NB = G * B          # blocks over the flattened (g,k) axis

    angles = np.asarray(angles)
    arange_quads = bool((angles == np.arange(K * 4).reshape(K, 4)).all())
    km, runs = _analyse_mask(angles, move_mask)

    nc = bass.Bass()
    for cval in (1024.0, 1024.25, 1024.0 * TWO_PI, 1024.0 * TWO_PI + _HALF_PI):
        _register_const(nc, float(cval))
    SP, SPB = _seg_bounds(angles, move_mask, M)
    MB = M - SPB        # B-tile width
    vinD = nc.declare_dram_parameter("vin", [P, G * K], f32, isOutput=False)
    pivA = nc.declare_dram_parameter("pivA", [P, 3 * G * SP], f32,
                                     isOutput=False)
    # quad-permuted pivot planes: pivP[c][q][g][k] = pos[., 4k+q, c] so the
    # r-vector subtraction reads contiguously (innermost (g,k))
    pivPd = (nc.declare_dram_parameter("pivP", [P, 12 * G * K], f32,
                                       isOutput=False) if arange_quads
             else None)
    posB = (nc.declare_dram_parameter("posB", [P, 3, G, MB], f32,
                                      isOutput=False) if SPB < M else None)
    # outputs as whole tiles: one contiguous DMA descriptor per partition
    outAd = nc.declare_dram_parameter("outA", [P, 3 * G * SP], f32,
                                      isOutput=True)
    outBd = (nc.declare_dram_parameter("outB", [P, 3 * G * MB], f32,
                                       isOutput=True) if SPB < M else None)

    with tile.TileContext(nc) as tc:
        with tc.tile_pool(name="main", bufs=1) as pool:
            # ---- SBUF tensors ----
            # separate tiles per DMA so consumers wait only on what they need
            VIN = pool.tile([P, G * K], f32)
            PLA = pool.tile([P, 3 * G * SP], f32)
            PIVP = (pool.tile([P, 3, 4, GK], f32, name="PIVP")
                    if arange_quads else None)
            PLB = pool.tile([P, 3, G, MB], f32, name="PLB") if SPB < M else None
            OUTA = pool.tile([P, 3, G, SP], f32)
            OUTB = pool.tile([P, 3, G, MB], f32, name="OUTB") if SPB < M else None
            # packed r-vectors / normals with duplicated xy components so a
            # +1/+2 component rotation is a plain offset (cross-product trick)
            RV = pool.tile([P, 3, 5, G, K], f32)  # (rIJ,rJK,rKL) x (x,y,z,x,y)
            NN = pool.tile([P, 3, 5, G, K], f32)  # (nIJK,nJKL,m) x (x,y,z,x,y)
            TA = pool.tile([P, 2, 3, G, K], f32)
            TB = pool.tile([P, 2, 3, G, K], f32)
            AT = pool.tile([P, 12, G, K], f32)   # A_k; q=4i+j, strides q:GK, g:K, k:1
            CT = pool.tile([P, 12, GK], f32)     # full prefixes
            NBP = NB + 4   # 4 pad columns so Hillis j<s lanes read in-bounds
            PT = pool.tile([P, 12, NBP], f32)    # block products / prefixes
            PT2 = pool.tile([P, 12, NBP], f32)   # Hillis ping-pong buffer
            PTB = pool.tile([P, G, 12, (K // 8) * 7], f32)  # prefixes bcast over t
            ACC = pool.tile([P, 12 * max(GK, 64)], f32)
            AC2 = pool.tile([P, 12 * max(GK, 64)], f32)
            AC3 = pool.tile([P, 12 * max(GK, 64)], f32)

            vv = _ap(VIN[:, :], 0, [[K, G], [1, K]])
            pla = _ap(PLA[:, :], 0, [])
            pivp = PIVP[:, :, :, :] if PIVP is not None else None
            plb = PLB[:, :, :, :] if PLB is not None else None
            outa = OUTA[:, :, :, :]
            outb = OUTB[:, :, :, :] if OUTB is not None else None

            def pl_view(m0, ln, _unused=None):
                """(base_ap, local column offset, group stride, comp stride)
                for columns [m0, m0+ln): B tile from SPB, else A tile."""
                if m0 >= SPB:
                    return plb, m0 - SPB, MB, G * MB
                assert m0 + ln <= SP
                return pla, m0, SP, G * SP

            def out_view(m0, ln):
                if m0 >= SPB:
                    return outb, m0 - SPB, MB, G * MB
                assert m0 + ln <= SP
                return outa, m0, SP, G * SP
            rv = RV[:, :, :, :, :]
            nn = NN[:, :, :, :, :]
            t1v = TA[:, :, :, :, :]
            t2v = TB[:, :, :, :, :]
            at = AT[:, :, :, :]
            ct = CT[:, :, :]
            pt = _ap(PT[:, :, :], 4, [[NBP, 12], [1, NB]])
            pt2 = _ap(PT2[:, :, :], 4, [[NBP, 12], [1, NB]])
            ptb = PTB[:, :, :, :]
            acc = ACC[:, :]
            ac2 = AC2[:, :]
            ac3 = AC3[:, :]

            RVv, RVc = 5 * GK, GK   # RV strides: vec, comp
            NVv = 5 * GK

            # ---- DMA in ----
            # All on the sync ring, in priority order: pivP (gates stage A),
            # vin (gates the ACT sin chain), pivA (gates pJ copy + A-apply),
            # posB (gates the B-tile apply, late).  Host arrays are
            # partition-major so each row is one contiguous descriptor.
            if pivp is not None:
                nc.sync.dma_start(
                    out=_ap(pivp, 0, [[1, 12 * GK]]),
                    in_=_dram_ap(pivPd[:, :], 0, [[12 * GK, P], [1, 12 * GK]]),
                )
            nc.sync.dma_start(
                out=_ap(vv, 0, [[1, GK]]),
                in_=_dram_ap(vinD[:, :], 0, [[GK, P], [1, GK]]),
            )
            nc.sync.dma_start(
                out=_ap(pla, 0, [[1, 3 * G * SP]]),
                in_=_dram_ap(pivA[:, :], 0, [[3 * G * SP, P], [1, 3 * G * SP]]),
            )
            if PLB is not None:
                nc.sync.dma_start(
                    out=_ap(plb, 0, [[1, 3 * G * MB]]),
                    in_=_dram_ap(posB[:, :, :, :], 0,
                                 [[3 * G * MB, P], [1, 3 * G * MB]]),
                )
            # Hillis pad columns must hold finite values (they feed the
            # patched lanes); zero them before the block-prefix scan
            nc.gpsimd.memset(_ap(PT[:, :, :], 0, [[NBP, 12], [1, 4]]), 0.0)
            nc.gpsimd.memset(_ap(PT2[:, :, :], 0, [[NBP, 12], [1, 4]]), 0.0)

            # ---- helpers ----
            tmp_idx = [0]

            def T(dt=f32):
                tmp_idx[0] += 1
                return pool.tile([P, G, K], dt, name=f"tmp{tmp_idx[0]}")

            def mul(a, b):
                o = T(); nc.vector.tensor_mul(o, a, b); return o

            def add(a, b):
                o = T(); nc.vector.tensor_add(o, a, b); return o

            def sub(a, b):
                o = T(); nc.vector.tensor_sub(o, a, b); return o

            def aff(a, scale, bias):
                o = T()
                nc.scalar.activation(o, a, Act.Identity, bias=bias, scale=scale)
                return o

            def activ(a, fn):
                o = T(); nc.scalar.activation(o, a, fn); return o

            def dot3v(a_base, a_off, a_cs, b_base, b_off, b_cs, eng=None):
                """dot over xyz comps via one mul + one innermost-reduce.
                a/b given as (tile_ap, elem offset, comp stride); both must
                have gk contiguous (stride 1)."""
                tmp_idx[0] += 1
                dp = pool.tile([P, GK, 3], f32, name=f"dp{tmp_idx[0]}")[:, :, :]
                (eng or nc.vector).tensor_mul(
                    dp,
                    _ap(a_base, a_off, [[1, GK], [a_cs, 3]]),
                    _ap(b_base, b_off, [[1, GK], [b_cs, 3]]),
                )
                o = T()
                nc.vector.tensor_reduce(
                    _ap(o, 0, [[1, GK]]), dp, mybir.AxisListType.X, Alu.add)
                return o

            # ---- pivot sources ----
            if not arange_quads:
                PIV = pool.tile([P, 3, G, 4, K], f32)
                pv = PIV[:, :, :, :, :]
                for k in range(K):
                    for q in range(4):
                        nc.vector.tensor_copy(
                            _ap(pv, q * K + k, [[G * 4 * K, 3], [4 * K, G]]),
                            _ap(pla, int(angles[k, q]),
                                [[G * SP, 3], [SP, G]]),
                        )

            def piv_ap(c, q):
                if arange_quads:
                    return _ap(pivp, c * 4 * GK + q * GK, [[K, G], [1, K]])
                return _ap(pv, c * G * 4 * K + q * K, [[4 * K, G], [1, K]])

            pJ = [piv_ap(c, 1) for c in range(3)]

            def _ap_cat3(_pj):
                # the three pJ views share a regular comp stride; rebuild as
                # one 3-dim AP [c][g][k]
                if arange_quads:
                    return _ap(pivp, GK, [[4 * GK, 3], [1, GK]])
                return _ap(pv, K, [[G * 4 * K, 3], [4 * K, G], [1, K]])

            # ---- stage A: packed r-vectors and cross products ----
            if arange_quads:
                # quad-permuted pivots: v-dim is the q axis, (g,k) contiguous
                nc.vector.tensor_sub(
                    _ap(rv, 0, [[RVv, 3], [RVc, 3], [1, GK]]),
                    _ap(pivp, GK, [[GK, 3], [4 * GK, 3], [1, GK]]),
                    _ap(pivp, 0, [[GK, 3], [4 * GK, 3], [1, GK]]))
            else:
                for g in range(G):
                    in1 = _ap(pv, g * 4 * K + K,
                              [[K, 3], [G * 4 * K, 3], [1, K]])
                    in0 = _ap(pv, g * 4 * K + 0,
                              [[K, 3], [G * 4 * K, 3], [1, K]])
                    nc.vector.tensor_sub(
                        _ap(rv, g * K, [[RVv, 3], [RVc, 3], [1, K]]), in1, in0)
            # duplicate comps x,y into slots 3,4
            nc.vector.tensor_copy(
                _ap(rv, 3 * RVc, [[RVv, 3], [RVc, 2], [1, GK]]),
                _ap(rv, 0, [[RVv, 3], [RVc, 2], [1, GK]]))
            # nIJK, nJKL = cross(A=[rIJ,rJK], B=[rJK,rKL]) via comp offsets
            nc.vector.tensor_mul(
                _ap(t1v, 0, [[3 * GK, 2], [GK, 3], [1, GK]]),
                _ap(rv, RVc, [[RVv, 2], [RVc, 3], [1, GK]]),
                _ap(rv, RVv + 2 * RVc, [[RVv, 2], [RVc, 3], [1, GK]]))
            nc.vector.tensor_mul(
                _ap(t2v, 0, [[3 * GK, 2], [GK, 3], [1, GK]]),
                _ap(rv, 2 * RVc, [[RVv, 2], [RVc, 3], [1, GK]]),
                _ap(rv, RVv + RVc, [[RVv, 2], [RVc, 3], [1, GK]]))
            nc.vector.tensor_sub(
                _ap(nn, 0, [[NVv, 2], [GK, 3], [1, GK]]),
                _ap(t1v, 0, [[3 * GK, 2], [GK, 3], [1, GK]]),
                _ap(t2v, 0, [[3 * GK, 2], [GK, 3], [1, GK]]))
            nc.vector.tensor_copy(
                _ap(nn, 3 * GK, [[NVv, 2], [GK, 2], [1, GK]]),
                _ap(nn, 0, [[NVv, 2], [GK, 2], [1, GK]]))
            # m = nIJK x rJK -> NN vec slot 2
            nc.vector.tensor_mul(
                _ap(t1v, 0, [[GK, 3], [1, GK]]),
                _ap(nn, GK, [[GK, 3], [1, GK]]),
                _ap(rv, RVv + 2 * RVc, [[RVc, 3], [1, GK]]))
            nc.vector.tensor_mul(
                _ap(t2v, 0, [[GK, 3], [1, GK]]),
                _ap(nn, 2 * GK, [[GK, 3], [1, GK]]),
                _ap(rv, RVv + RVc, [[RVc, 3], [1, GK]]))
            nc.vector.tensor_sub(
                _ap(nn, 2 * NVv, [[GK, 3], [1, GK]]),
                _ap(t1v, 0, [[GK, 3], [1, GK]]),
                _ap(t2v, 0, [[GK, 3], [1, GK]]))

            # pJ source: read straight out of pivP when available, else make
            # a compact copy
            if arange_quads:
                pj_b, pj_off, pj_cs = pivp, GK, 4 * GK
            else:
                PJC = pool.tile([P, 3, G, K], f32)
                pjc = PJC[:, :, :, :]
                nc.vector.tensor_copy(_ap(pjc, 0, [[GK, 3], [K, G], [1, K]]),
                                      _ap_cat3(pJ))
                pj_b, pj_off, pj_cs = pjc, 0, GK

            def rvec(v, c):
                return _ap(rv, v * RVv + c * RVc, [[K, G], [1, K]])

            def nvec(v, c):
                return _ap(nn, v * NVv + c * GK, [[K, G], [1, K]])

            rJK = [rvec(1, c) for c in range(3)]
            rjk_off = RVv                      # RV vec 1, comp stride RVc
            m_off = 2 * NVv                    # m lives in NN vec 2

            # paired dot products: one mul+reduce covers two dots that share
            # a left operand; results land adjacently for fused downstream ops
            DOTS = pool.tile([P, 4, GK], f32)  # rows: x0, l1^2, y0, lm^2
            DP4 = pool.tile([P, 2, GK, 3], f32)
            dots = DOTS[:, :, :]
            dp4 = DP4[:, :, :, :]
            # {x0, l1^2} = nIJK . (nJKL, nIJK)
            nc.vector.tensor_mul(
                dp4,
                _ap(nn, 0, [[0, 2], [1, GK], [GK, 3]]),
                _ap(nn, NVv, [[-NVv, 2], [1, GK], [GK, 3]]))
            nc.vector.tensor_reduce(
                _ap(dots, 0, [[GK, 2], [1, GK]]), dp4,
                mybir.AxisListType.X, Alu.add)
            # y0 = m . nJKL (single dot; reuse dp4's first GK*3 lane block)
            nc.vector.tensor_mul(
                _ap(dp4, 0, [[3, GK], [1, 3]]),
                _ap(nn, m_off, [[1, GK], [GK, 3]]),
                _ap(nn, NVv, [[1, GK], [GK, 3]]))
            nc.vector.tensor_reduce(
                _ap(dots, 2 * GK, [[1, GK]]),
                _ap(dp4, 0, [[3, GK], [1, 3]]),
                mybir.AxisListType.X, Alu.add)
            jks = dot3v(rv, rjk_off, RVc, rv, rjk_off, RVc)
            # lm^2 = l1^2 * |rJK|^2  (m = nIJK x rJK with nIJK _|_ rJK)
            nc.vector.tensor_mul(
                _ap(dots, 3 * GK, [[1, GK]]),
                _ap(dots, GK, [[1, GK]]),
                _ap(jks[:, :, :], 0, [[1, GK]]))
            L1LM = pool.tile([P, 2, GK], f32)  # (l1, lm)
            l1lm = L1LM[:, :, :]
            nc.scalar.activation(
                _ap(l1lm, 0, [[GK, 2], [1, GK]]),
                _ap(dots, GK, [[2 * GK, 2], [1, GK]]), Act.Sqrt)
            XY = pool.tile([P, 2, GK], f32)    # (x1, y1) = (x0*lm, y0*l1)
            xy = XY[:, :, :]
            nc.vector.tensor_mul(
                xy,
                _ap(dots, 0, [[2 * GK, 2], [1, GK]]),
                _ap(l1lm, GK, [[-GK, 2], [1, GK]]))
            SQ = pool.tile([P, 2, GK], f32)
            sq = SQ[:, :, :]
            nc.vector.tensor_mul(sq, xy, xy)
            hs = T()
            nc.vector.tensor_add(_ap(hs[:, :, :], 0, [[1, GK]]),
                                 _ap(sq, 0, [[1, GK]]),
                                 _ap(sq, GK, [[1, GK]]))
            hr = T(); nc.vector.reciprocal(hr, hs)
            rh = activ(hr, Act.Sqrt)            # 1/hypot
            CS = pool.tile([P, 2, GK], f32)     # (ccur, scur)
            cs_ = CS[:, :, :]
            nc.vector.tensor_mul(
                cs_, xy, _ap(rh[:, :, :], 0, [[0, 2], [1, GK]]))
            jkr = T(); nc.vector.reciprocal(jkr, jks)
            jrs = activ(jkr, Act.Sqrt)          # 1/|rJK|
            AXT = pool.tile([P, 3, G, K], f32)
            axt = AXT[:, :, :, :]
            nc.vector.tensor_mul(
                _ap(axt, 0, [[GK, 3], [1, GK]]),
                _ap(rv, rjk_off, [[RVc, 3], [1, GK]]),
                _ap(jrs[:, :, :], 0, [[0, 3], [1, GK]]),
            )
            ax = [_ap(axt, c * GK, [[K, G], [1, K]]) for c in range(3)]

            # sin/cos of targets with range reduction (Sin table ok |x|<~3.55)
            def reduced_sin(shift_quarter, extra):
                q = aff(vv, 1.0 / TWO_PI, 1024.0 + shift_quarter)
                qi = T(i32)
                nc.vector.tensor_copy(qi, q)     # f32->i32 rounds to nearest
                qf = T()
                nc.vector.tensor_copy(qf, qi)
                t = aff(qf, -TWO_PI, 1024.0 * TWO_PI + extra)
                return activ(add(vv, t), Act.Sin)

            sv = reduced_sin(0.0, 0.0)
            cv = reduced_sin(0.25, _HALF_PI)

            PC1 = pool.tile([P, 2, GK], f32)   # cv * (ccur, scur)
            PC2 = pool.tile([P, 2, GK], f32)   # sv * (ccur, scur)
            pc1 = PC1[:, :, :]
            pc2 = PC2[:, :, :]
            nc.vector.tensor_mul(pc1, cs_, _ap(cv[:, :, :], 0, [[0, 2], [1, GK]]))
            nc.vector.tensor_mul(pc2, cs_, _ap(sv[:, :, :], 0, [[0, 2], [1, GK]]))
            c_ = T()
            s_ = T()
            nc.vector.tensor_add(_ap(c_[:, :, :], 0, [[1, GK]]),
                                 _ap(pc1, 0, [[1, GK]]), _ap(pc2, GK, [[1, GK]]))
            nc.vector.tensor_sub(_ap(s_[:, :, :], 0, [[1, GK]]),
                                 _ap(pc2, 0, [[1, GK]]), _ap(pc1, GK, [[1, GK]]))
            t1_ = T()
            nc.vector.tensor_scalar(t1_, c_, -1.0, 1.0, Alu.mult, Alu.add)  # 1-cos

            TAX = pool.tile([P, 3, G, K], f32)
            SAX = pool.tile([P, 3, G, K], f32)
            UD = pool.tile([P, 3, G, K], f32)
            OD = pool.tile([P, 2, G, K], f32)
            taxv = TAX[:, :, :, :]
            saxv = SAX[:, :, :, :]
            udv = UD[:, :, :, :]
            odv = OD[:, :, :, :]
            d3 = [[GK, 3], [1, GK]]
            bc3 = [[0, 3], [1, GK]]
            nc.vector.tensor_mul(_ap(taxv, 0, d3), _ap(axt, 0, d3),
                                 _ap(t1_[:, :, :], 0, bc3))
            nc.vector.tensor_mul(_ap(saxv, 0, d3), _ap(axt, 0, d3),
                                 _ap(s_[:, :, :], 0, bc3))
            nc.vector.tensor_mul(_ap(udv, 0, d3), _ap(taxv, 0, d3),
                                 _ap(axt, 0, d3))

            def aq(q):
                return _ap(at, q * GK, [[K, G], [1, K]])

            # diagonal: q = 0,5,10 -> stride 5*GK
            nc.vector.tensor_add(
                _ap(at, 0, [[5 * GK, 3], [1, GK]]),
                _ap(udv, 0, d3),
                _ap(c_[:, :, :], 0, bc3),
            )
            # off-diagonal products: txy,txz = tax0*(ax1,ax2); tyz = tax1*ax2
            nc.vector.tensor_mul(
                _ap(odv, 0, [[GK, 2], [1, GK]]),
                _ap(axt, GK, [[GK, 2], [1, GK]]),
                _ap(taxv, 0, [[0, 2], [1, GK]]),
            )
            tyz = T()
            nc.vector.tensor_mul(tyz, _ap(taxv, GK, [[K, G], [1, K]]),
                                 _ap(axt, 2 * GK, [[K, G], [1, K]]))
            txy = _ap(odv, 0, [[K, G], [1, K]])
            txz = _ap(odv, GK, [[K, G], [1, K]])
            sax = [_ap(saxv, c * GK, [[K, G], [1, K]]) for c in range(3)]
            nc.vector.tensor_sub(aq(1), txy, sax[2])
            nc.vector.tensor_add(aq(4), txy, sax[2])
            nc.vector.tensor_add(aq(2), txz, sax[1])
            nc.vector.tensor_sub(aq(8), txz, sax[1])
            nc.vector.tensor_sub(aq(6), tyz, sax[0])
            nc.vector.tensor_add(aq(9), tyz, sax[0])

            # b = pJ - R @ pJ : batched products, reduce, sub
            BP = pool.tile([P, 3, GK, 3], f32)
            bp = BP[:, :, :, :]
            nc.vector.tensor_mul(
                bp,
                _ap(at, 0, [[4 * GK, 3], [1, GK], [GK, 3]]),
                _ap(pj_b, pj_off, [[0, 3], [1, GK], [pj_cs, 3]]),
            )
            RPJ = pool.tile([P, 3, G, K], f32)
            rpj = RPJ[:, :, :, :]
            nc.vector.tensor_reduce(
                _ap(rpj, 0, [[GK, 3], [1, GK]]), bp,
                mybir.AxisListType.X, Alu.add)
            nc.vector.tensor_sub(
                _ap(at, 3 * GK, [[4 * GK, 3], [1, GK]]),
                _ap(pj_b, pj_off, [[pj_cs, 3], [1, GK]]),
                _ap(rpj, 0, [[GK, 3], [1, GK]]),
            )

            # ---- stage B: blocked prefix composition ----
            at_flat = _ap(at, 0, [[GK, 12], [1, GK]])

            def compose(dst, dq, dbd, doff, left, lq, lbd, loff,
                        right, rq, rbd, roff):
                """dst[i,j,*] = sum_m left[i,m,*]*right[m,j,*]; dst[i,3,*] +=
                left[i,3,*].  *bd = batch [step,count] dims (equal counts)."""
                counts = [d[1] for d in dbd]
                assert [d[1] for d in lbd] == counts
                assert [d[1] for d in rbd] == counts
                nb = 1
                for cnt in counts:
                    nb *= cnt
                abd = []
                stp = 1
                for cnt in reversed(counts):
                    abd.insert(0, [stp, cnt])
                    stp *= cnt

                def accv(base):
                    return _ap(base, 0, [[4 * nb, 3], [nb, 4]] + abd)

                dstv = _ap(dst, doff, [[4 * dq, 3], [dq, 4]] + dbd)

                def dmul(tgt, mrow):
                    nc.vector.tensor_mul(
                        accv(tgt),
                        _ap(right, roff + 4 * mrow * rq,
                            [[0, 3], [rq, 4]] + rbd),
                        _ap(left, loff + mrow * lq,
                            [[4 * lq, 3], [0, 4]] + lbd),
                    )

                dmul(acc, 0)
                dmul(ac2, 1)
                nc.vector.tensor_add(accv(acc), accv(acc), accv(ac2))
                dmul(ac2, 2)
                nc.vector.tensor_add(dstv, accv(acc), accv(ac2))
                bias_d = _ap(dst, doff + 3 * dq, [[4 * dq, 3]] + dbd)
                nc.vector.tensor_add(
                    bias_d, bias_d,
                    _ap(left, loff + 3 * lq, [[4 * lq, 3]] + lbd),
                )

            # within-block scan, in place: A[:, t] <- A[:, t-1] o A[:, t]
            # (the 3 muls read the slot before the final add overwrites it)
            for t in range(1, L):
                compose(at_flat, GK, [[L, NB]], t,
                        at_flat, GK, [[L, NB]], t - 1,
                        at_flat, GK, [[L, NB]], t)
            # block products
            nc.vector.tensor_copy(
                _ap(pt, 0, [[NBP, 12], [1, NB]]),
                _ap(at_flat, L - 1, [[GK, 12], [L, NB]]),
            )
            # block-prefix scan: Hillis-Steele over the flattened (g,b) axis.
            # Lanes j%B < s read the neighbour's tail (garbage) and are
            # patched by the trailing copy before the buffers swap.
            src_pt, dst_pt = pt, pt2
            s = 1
            while s < B:
                compose(dst_pt, NBP, [[1, NB]], 0,
                        src_pt, NBP, [[1, NB]], -s,
                        src_pt, NBP, [[1, NB]], 0)
                nc.vector.tensor_copy(
                    _ap(dst_pt, 0, [[NBP, 12], [B, G], [1, s]]),
                    _ap(src_pt, 0, [[NBP, 12], [B, G], [1, s]]))
                src_pt, dst_pt = dst_pt, src_pt
                s *= 2
            ptf = src_pt

            # ---- stage C ----
            def dma_out_tile(which):
                """DMA an output tile, one component plane at a time, so the
                early planes drain while compute finishes the last one.
                Tile's subtile deps make each wait only on its own writers."""
                if which == "A":
                    for i in range(3):
                        nc.scalar.dma_start(
                            out=_dram_ap(outAd[:, :], i * G * SP,
                                         [[3 * G * SP, P], [1, G * SP]]),
                            in_=_ap(outa, i * G * SP, [[1, G * SP]]),
                        )
                else:
                    for i in range(3):
                        nc.scalar.dma_start(
                            out=_dram_ap(outBd[:, :], i * G * MB,
                                         [[3 * G * MB, P], [1, G * MB]]),
                            in_=_ap(outb, i * G * MB, [[1, G * MB]]),
                        )

            def apply_single_from(coef, coefq, coefoff, m0, length):
                """out[:, :, m0:m0+length] = R@p + b with per-(partition,g)
                scalar coefficients from `coef` (q stride coefq, g stride
                coefoff).  Muls on ACT (per-partition scale), adds on DVE."""
                plbase, mloc, gs, cs = pl_view(m0, length, None)
                obase, omloc, ogs, ocs = out_view(m0, length)
                tmp_idx[0] += 1
                prod = [[pool.tile([P, G * length], f32,
                                   name=f"prod{tmp_idx[0]}_{i}_{cc}")[:, :]
                         for cc in range(3)] for i in range(3)]
                for i in range(3):
                    for cc in range(3):
                        for g in range(G):
                            nc.scalar.activation(
                                _ap(prod[i][cc], g * length, [[1, length]]),
                                _ap(plbase, cc * cs + g * gs + mloc,
                                    [[1, length]]),
                                Act.Identity,
                                scale=_ap(coef, (4 * i + cc) * coefq
                                          + g * coefoff, [[1, 1]]),
                            )
                for i in range(3):
                    d_t = [[length, G], [1, length]]
                    s1 = _ap(prod[i][0], 0, d_t)
                    nc.vector.tensor_add(s1, s1, _ap(prod[i][1], 0, d_t))
                    nc.vector.tensor_add(s1, s1, _ap(prod[i][2], 0, d_t))
                    for g in range(G):
                        # + translation via the ACT bias port (keeps DVE free)
                        nc.scalar.activation(
                            _ap(obase, i * ocs + g * ogs + omloc, [[1, length]]),
                            _ap(prod[i][0], g * length, [[1, length]]),
                            Act.Identity,
                            bias=_ap(coef, (4 * i + 3) * coefq + g * coefoff,
                                     [[1, 1]]),
                            scale=1.0,
                        )

            pt_last = bass.AP(tensor=ptf.tensor, offset=ptf.offset + (B - 1),
                              ap=list(ptf.ap))

            def apply_runs(starts, length, ks):
                nr = len(starts)
                if nr == 1 and ks[0] == K - 1:
                    # chain-last prefix == last block product: ready right
                    # after the block-prefix scan, before distribute.
                    apply_single_from(pt_last, NBP, B, starts[0], length)
                    return
                if nr == 1:
                    base = bass.AP(tensor=ct.tensor, offset=ct.offset + ks[0],
                                   ap=list(ct.ap))
                    apply_single_from(base, GK, K, starts[0], length)
                    return
                sm = starts[1] - starts[0]
                sk = ks[1] - ks[0]
                m0, k0 = starts[0], ks[0]
                span = max(starts) + length - m0
                plbase, mloc, gs, cs = pl_view(m0, span, None)
                obase, omloc, ogs, ocs = out_view(m0, span)
                d_pl = [[gs, G], [sm, nr], [1, length]]
                d_out = [[ogs, G], [sm, nr], [1, length]]
                d_c = [[K, G], [sk, nr], [0, length]]
                d_acc = [[nr * length, G], [length, nr], [1, length]]
                for i in range(3):
                    for cc in range(2):
                        tgt = acc if cc == 0 else ac2
                        nc.vector.tensor_mul(
                            _ap(tgt, 0, d_acc),
                            _ap(plbase, cc * cs + mloc, d_pl),
                            _ap(ct, (4 * i + cc) * GK + k0, d_c),
                        )
                    nc.vector.tensor_add(
                        _ap(acc, 0, d_acc), _ap(acc, 0, d_acc), _ap(ac2, 0, d_acc)
                    )
                    nc.vector.tensor_mul(
                        _ap(ac2, 0, d_acc),
                        _ap(plbase, 2 * cs + mloc, d_pl),
                        _ap(ct, (4 * i + 2) * GK + k0, d_c),
                    )
                    nc.vector.tensor_add(
                        _ap(acc, 0, d_acc), _ap(acc, 0, d_acc),
                        _ap(ac2, 0, d_acc),
                    )
                    nc.vector.tensor_add(
                        _ap(obase, i * ocs + omloc, d_out),
                        _ap(acc, 0, d_acc),
                        _ap(ct, (4 * i + 3) * GK + k0, d_c),
                    )

            def emit_distribute():
                # distribute: block 0 copies, blocks b>=1 get P[b-1] @ W
                nk = (B - 1) * L
                nc.vector.tensor_copy(
                    _ap(ct, 0, [[GK, 12], [K, G], [1, L]]),
                    _ap(at_flat, 0, [[GK, 12], [K, G], [1, L]]),
                )
                # broadcast block prefixes over t so g fuses into 3-dim APs:
                # PTB[g][q][jb*L + t] = ptf[q][g*B + jb]
                for g in range(G):
                    nc.vector.tensor_copy(
                        _ap(ptb, g * 12 * nk, [[nk, 12], [L, B - 1], [1, L]]),
                        _ap(ptf, g * B, [[NBP, 12], [1, B - 1], [0, L]]))
                d_w = [[GK, 4], [K, G], [1, nk]]
                d_a = [[G * nk, 4], [nk, G], [1, nk]]
                accs = (acc, ac2, ac3)
                # all 9 cross products first (pure reads of W and P), then
                # the combines
                for m in range(3):
                    for i in range(3):
                        nc.vector.tensor_mul(
                            _ap(accs[m], i * 4 * G * nk, d_a),
                            _ap(at_flat, 4 * m * GK + L, d_w),
                            _ap(ptb, (4 * i + m) * nk,
                                [[0, 4], [12 * nk, G], [1, nk]]),
                        )
                for i in range(3):
                    o = i * 4 * G * nk
                    nc.vector.tensor_add(
                        _ap(acc, o, d_a), _ap(acc, o, d_a), _ap(ac2, o, d_a))
                    nc.vector.tensor_add(
                        _ap(ct, 4 * i * GK + L, d_w),
                        _ap(acc, o, d_a), _ap(ac3, o, d_a))
                    nc.vector.tensor_add(
                        _ap(ct, (4 * i + 3) * GK + L, [[K, G], [1, nk]]),
                        _ap(ct, (4 * i + 3) * GK + L, [[K, G], [1, nk]]),
                        _ap(ptb, (4 * i + 3) * nk, [[12 * nk, G], [1, nk]]),
                    )

            # unmoved atoms: copy + DMA as soon as PL lands
            unmoved = [m for m in range(M) if km[m] < 0]
            u0 = 0
            while u0 < len(unmoved):
                u1 = u0
                while u1 + 1 < len(unmoved) and unmoved[u1 + 1] == unmoved[u1] + 1:
                    u1 += 1
                a0, ln = unmoved[u0], u1 - u0 + 1
                assert a0 + ln <= SP or a0 >= SPB
                ubase, umloc, ugs, ucs = pl_view(a0, ln, None)
                uobase, uomloc, uogs, uocs = out_view(a0, ln)
                nc.vector.tensor_copy(
                    _ap(uobase, uomloc, [[uocs, 3], [uogs, G], [1, ln]]),
                    _ap(ubase, umloc, [[ucs, 3], [ugs, G], [1, ln]]),
                )
                u0 = u1 + 1

            # classes: chain-last single-run first (overlaps distribute)
            by_len = {}
            for (m0, ln, k) in runs:
                by_len.setdefault(ln, []).append((m0, k))
            classes = sorted(
                by_len.items(),
                key=lambda kv: 0 if (len(kv[1]) == 1 and kv[1][0][1] == K - 1)
                else 1)
            emitted_distribute = False
            for ln, rs in classes:
                starts = [r[0] for r in rs]
                ks = [r[1] for r in rs]
                nr = len(rs)
                chain_last_single = nr == 1 and ks[0] == K - 1
                if not chain_last_single and not emitted_distribute:
                    emit_distribute()
                    emitted_distribute = True
                regular = nr <= 2 or (
                    all(starts[r] == starts[0] + r * (starts[1] - starts[0])
                        for r in range(nr))
                    and all(ks[r] == ks[0] + r * (ks[1] - ks[0])
                            for r in range(nr))
                )
                if regular:
                    apply_runs(starts, ln, ks)
                else:
                    for (m0, k) in rs:
                        apply_runs([m0], ln, [k])
            # B tile drains first (its writers finish early); HWDGE is FIFO
            # per engine so the early DMA must be queued first
            if OUTB is not None:
                dma_out_tile("B")
            dma_out_tile("A")

    _split_multi_waits(nc)
    return nc


_BUILD_CACHE = {}


def make_in_maps(input, pos, angles, move_mask):
    input = np.asarray(input, dtype=np.float32)
    pos = np.asarray(pos, dtype=np.float32)
    angles = np.asarray(angles)
    N, K = input.shape
    M = pos.shape[1]
    NL = N // NCORES
    G = NL // P
    SP, SPB = _seg_bounds(angles, np.asarray(move_mask).astype(bool), M)
    arange_quads = bool((angles == np.arange(K * 4).reshape(K, 4)).all())
    in_maps = []
    for c in range(NCORES):
        sl = slice(c * NL, (c + 1) * NL)
        # (NL, M, 3) -> (P, 3, G, M): partition-major so each partition row
        # is one contiguous DMA descriptor
        pm = pos[sl].reshape(G, P, M, 3).transpose(1, 3, 0, 2)
        vrows = input[sl].reshape(G, P, K).transpose(1, 0, 2).reshape(P, G * K)
        im = {"vin": np.ascontiguousarray(vrows),
              "pivA": np.ascontiguousarray(
                  pm[:, :, :, :SP].reshape(P, 3 * G * SP))}
        if arange_quads:
            # pivP[p][c][q][g][k] = pm[p][c][g][4k+q]
            pp = pm[:, :, :, :4 * K].reshape(P, 3, G, K, 4)
            im["pivP"] = np.ascontiguousarray(
                pp.transpose(0, 1, 4, 2, 3).reshape(P, 12 * G * K))
        if SPB < M:
            im["posB"] = np.ascontiguousarray(pm[:, :, :, SPB:])
        in_maps.append(im)
    return in_maps


def kernel(input, pos, angles, move_mask):
    input = np.ascontiguousarray(np.asarray(input, dtype=np.float32))
    pos = np.ascontiguousarray(np.asarray(pos, dtype=np.float32))
    angles = np.asarray(angles)
    move_mask = np.asarray(move_mask).astype(bool)

    N, K = input.shape
    _, M, three = pos.shape
    assert three == 3
    assert N % (NCORES * P) == 0
    NL = N // NCORES

    key = (N, K, M, angles.tobytes(), move_mask.tobytes())
    nc = _BUILD_CACHE.get(key)
    if nc is None:
        nc = _build(angles, move_mask, NL, K, M)
        _BUILD_CACHE[key] = nc

    in_maps = make_in_maps(input, pos, angles, move_mask)

    # the axon-proxied NRT occasionally wedges transiently
    # (NRT_EXEC_UNIT_UNRECOVERABLE); one retry recovers it
    try:
        res = run_bass_kernel_spmd(nc, in_maps, list(range(NCORES)))
    except Exception:
        res = run_bass_kernel_spmd(nc, in_maps, list(range(NCORES)))

    G = NL // P
    SP, SPB = _seg_bounds(angles, move_mask, M)
    out = np.empty((N, M, 3), dtype=np.float32)
    for c in range(NCORES):
        sl = slice(c * NL, (c + 1) * NL)
        # (P, 3, G, cols) -> (NL, cols, 3)
        oa = res.results[c]["outA"].reshape(P, 3, G, SP)
        out[sl, :SPB] = oa.transpose(2, 0, 3, 1).reshape(NL, SP, 3)[:, :SPB]
        if SPB < M:
            ob = res.results[c]["outB"].reshape(P, 3, G, M - SPB)
            out[sl, SPB:] = ob.transpose(2, 0, 3, 1).reshape(NL, M - SPB, 3)
    return out



# revision 61
# speedup vs baseline: 1.1249x; 1.0221x over previous
"""Dihedral2Coord Trainium2 kernel.

Math: the reference applies K sequential dihedral-set steps; step k rotates
a suffix of the atom chain rigidly about the current J-K bond.  Every step's
transform is a proper rigid motion that moves all four pivot atoms of every
later step together, so the dihedral measured at application time equals the
dihedral of the ORIGINAL coordinates (dihedrals are invariant under rigid
motion).  Step k's rotation, expressed in original coordinates, is therefore
a fixed affine A_k computable from the original positions alone, and the
cumulative transform is the prefix product C_k = A_0 @ A_1 @ ... @ A_k.
The kernel:
  A) computes all K per-conformer Rodrigues affines in parallel,
  B) prefix-composes them with a blocked scan,
  C) applies C_{km(m)} to each atom run, where km(m) counts the steps whose
     mask includes atom m (verified prefix-structured on host).

Sharding: pure data parallelism over conformers N across 8 cores (SPMD).
"""

import sys

import numpy as np

try:
    import concourse.bass as bass
except ImportError:  # path in the grading container
    sys.path.insert(0, "/opt/trn_rl_repo")
    import concourse.bass as bass

import concourse.tile as tile
from concourse import mybir
from concourse.bass_utils import run_bass_kernel_spmd

f32 = mybir.dt.float32
i32 = mybir.dt.int32
Alu = mybir.AluOpType
Act = mybir.ActivationFunctionType

NCORES = 8
P = 128
TWO_PI = float(2.0 * np.pi)
_HALF_PI = float(np.pi / 2)

_WAIT_CAP = 1  # this walrus build rejects >1 sync-wait per instruction


def _register_const(nc, value, dtype=f32):
    """Register an activation-bias constant.  Written on the Activation
    engine from the framework's const-1.0 AP (ordered by Bass.__init__'s
    barrier); later ACT reads are same-engine program-ordered, so no extra
    barrier is needed."""
    if (dtype, value) in nc.const_aps.aps:
        return
    t = nc.alloc_sbuf_tensor(f"const-{dtype.name}-{value}", [128, 1], dtype)
    one = nc.const_aps.aps[(f32, 1.0)]
    nc.scalar.activation(t.ap(), one, Act.Identity, bias=0.0, scale=float(value))
    nc.const_aps.aps[(dtype, value)] = t.ap()


def _split_multi_waits(nc):
    """Split every instruction carrying >cap sync-waits into single-wait
    NoOps (same engine, immediately before, same block).  Waits are monotone
    semaphore conditions so this preserves semantics exactly."""
    n = 0
    for func in nc.m.functions:
        for bb in func.blocks:
            old = list(bb.instructions)
            if not any(
                i.sync_info is not None and len(i.sync_info.on_wait) > _WAIT_CAP
                for i in old
            ):
                continue
            new = []
            for inst in old:
                si = inst.sync_info
                if si is not None and len(si.on_wait) > _WAIT_CAP:
                    waits = list(si.on_wait)
                    head, tail = waits[:-_WAIT_CAP], waits[-_WAIT_CAP:]
                    for j in range(0, len(head), _WAIT_CAP):
                        n += 1
                        new.append(
                            mybir.InstNoOp(
                                name=f"{inst.name}_ws{j}",
                                engine=inst.engine,
                                sync_info=mybir.SyncInfo(
                                    on_wait=list(head[j : j + _WAIT_CAP]), on_update=[]
                                ),
                                bass_nofuse=True,
                            )
                        )
                    try:
                        si.on_wait[:] = tail
                    except TypeError:
                        inst.sync_info = mybir.SyncInfo(
                            on_wait=tail, on_update=list(si.on_update)
                        )
                new.append(inst)
            try:
                bb.instructions[:] = new
            except TypeError:
                bb.instructions = new
    return n


def _ap(base, offset_elems, dims):
    """Free-dim AP view into an SBUF tile AP `base` (partition dim kept).
    dims: list of [step, count] in elements of the tile's free space."""
    return bass.AP(
        tensor=base.tensor,
        offset=base.offset + offset_elems,
        ap=[list(base.ap[0])] + [list(d) for d in dims],
    )


def _dram_ap(t, offset, dims):
    return bass.AP(tensor=t.tensor, offset=offset, ap=[list(d) for d in dims])


def _analyse_mask(angles, move_mask):
    """Host-side structural analysis. Returns (km, runs): km[m] is the last
    step applied to atom m (-1 = never moved); runs are (start, len, k)."""
    K, M = move_mask.shape
    km = move_mask.astype(np.int64).sum(0) - 1
    kk = np.arange(K)[:, None]
    if not (move_mask == (kk <= km[None, :])).all():
        raise NotImplementedError("move_mask is not prefix-structured per atom")
    for k in range(K):
        for a in angles[k]:
            if not move_mask[:k, a].all():
                raise NotImplementedError("pivot atoms not rigidly co-moved")
    runs = []
    m = 0
    while m < M:
        j = m
        while j + 1 < M and km[j + 1] == km[m]:
            j += 1
        if km[m] >= 0:
            runs.append((m, j - m + 1, int(km[m])))
        m = j + 1
    return km, runs


def _seg_bounds(angles, move_mask, M):
    """(SP, SPB): pivot region [0, SP); B-tile starts at SPB <= SP so no
    run/unmoved segment crosses a tile boundary (columns [SPB, SP) are
    duplicated in both tiles)."""
    km, runs = _analyse_mask(angles, move_mask)
    SP = min(int(np.asarray(angles).max()) + 1, M)
    segs = [(m0, ln) for (m0, ln, _k) in runs]
    m = 0
    while m < M:
        if km[m] < 0:
            j = m
            while j + 1 < M and km[j + 1] < 0:
                j += 1
            segs.append((m, j - m + 1))
            m = j + 1
        else:
            m += 1
    SPB = SP
    for (m0, ln) in segs:
        if m0 < SP < m0 + ln:
            SPB = min(SPB, m0)
    return SP, SPB


def _build(angles, move_mask, NL, K, M):
    """Build the Bass module for one core handling NL conformers."""
    G = NL // P
    assert NL == G * P
    GK = G * K
    L = 8               # within-block scan length
    assert K % L == 0
    B = K // L          # blocks per conformer-group
    NB = G * B          # blocks over the flattened (g,k) axis

    angles = np.asarray(angles)
    arange_quads = bool((angles == np.arange(K * 4).reshape(K, 4)).all())
    km, runs = _analyse_mask(angles, move_mask)

    nc = bass.Bass()
    for cval in (1024.0, 1024.25, 1024.0 * TWO_PI, 1024.0 * TWO_PI + _HALF_PI):
        _register_const(nc, float(cval))
    SP, SPB = _seg_bounds(angles, move_mask, M)
    MB = M - SPB        # B-tile width
    vinD = nc.declare_dram_parameter("vin", [P, G * K], f32, isOutput=False)
    pivA = nc.declare_dram_parameter("pivA", [P, 3 * G * SP], f32,
                                     isOutput=False)
    # quad-permuted pivot planes: pivP[c][q][g][k] = pos[., 4k+q, c] so the
    # r-vector subtraction reads contiguously (innermost (g,k))
    pivPd = (nc.declare_dram_parameter("pivP", [P, 12 * G * K], f32,
                                       isOutput=False) if arange_quads
             else None)
    posB = (nc.declare_dram_parameter("posB", [P, 3, G, MB], f32,
                                      isOutput=False) if SPB < M else None)
    # outputs as whole tiles: one contiguous DMA descriptor per partition
    outAd = nc.declare_dram_parameter("outA", [P, 3 * G * SP], f32,
                                      isOutput=True)
    outBd = (nc.declare_dram_parameter("outB", [P, 3 * G * MB], f32,
                                       isOutput=True) if SPB < M else None)

    with tile.TileContext(nc) as tc:
        with tc.tile_pool(name="main", bufs=1) as pool:
            # ---- SBUF tensors ----
            # separate tiles per DMA so consumers wait only on what they need
            VIN = pool.tile([P, G * K], f32)
            PLA = pool.tile([P, 3 * G * SP], f32)
            PIVP = (pool.tile([P, 3, 4, GK], f32, name="PIVP")
                    if arange_quads else None)
            PLB = pool.tile([P, 3, G, MB], f32, name="PLB") if SPB < M else None
            OUTA = pool.tile([P, 3, G, SP], f32)
            OUTB = pool.tile([P, 3, G, MB], f32, name="OUTB") if SPB < M else None
            # packed r-vectors / normals with duplicated xy components so a
            # +1/+2 component rotation is a plain offset (cross-product trick)
            RV = pool.tile([P, 3, 5, G, K], f32)  # (rIJ,rJK,rKL) x (x,y,z,x,y)
            NN = pool.tile([P, 3, 5, G, K], f32)  # (nIJK,nJKL,m) x (x,y,z,x,y)
            TA = pool.tile([P, 2, 3, G, K], f32)
            TB = pool.tile([P, 2, 3, G, K], f32)
            AT = pool.tile([P, 12, G, K], f32)   # A_k; q=4i+j, strides q:GK, g:K, k:1
            CT = pool.tile([P, 12, GK], f32)     # full prefixes
            NBP = NB + 4   # 4 pad columns so Hillis j<s lanes read in-bounds
            PT = pool.tile([P, 12, NBP], f32)    # block products / prefixes
            PT2 = pool.tile([P, 12, NBP], f32)   # Hillis ping-pong buffer
            PTB = pool.tile([P, G, 12, (K // 8) * 7], f32)  # prefixes bcast over t
            ACC = pool.tile([P, 12 * max(GK, 64)], f32)
            AC2 = pool.tile([P, 12 * max(GK, 64)], f32)
            AC3 = pool.tile([P, 12 * max(GK, 64)], f32)

            vv = _ap(VIN[:, :], 0, [[K, G], [1, K]])
            pla = _ap(PLA[:, :], 0, [])
            pivp = PIVP[:, :, :, :] if PIVP is not None else None
            plb = PLB[:, :, :, :] if PLB is not None else None
            outa = OUTA[:, :, :, :]
            outb = OUTB[:, :, :, :] if OUTB is not None else None

            def pl_view(m0, ln, _unused=None):
                """(base_ap, local column offset, group stride, comp stride)
                for columns [m0, m0+ln): B tile from SPB, else A tile."""
                if m0 >= SPB:
                    return plb, m0 - SPB, MB, G * MB
                assert m0 + ln <= SP
                return pla, m0, SP, G * SP

            def out_view(m0, ln):
                if m0 >= SPB:
                    return outb, m0 - SPB, MB, G * MB
                assert m0 + ln <= SP
                return outa, m0, SP, G * SP
            rv = RV[:, :, :, :, :]
            nn = NN[:, :, :, :, :]
            t1v = TA[:, :, :, :, :]
            t2v = TB[:, :, :, :, :]
            at = AT[:, :, :, :]
            ct = CT[:, :, :]
            pt = _ap(PT[:, :, :], 4, [[NBP, 12], [1, NB]])
            pt2 = _ap(PT2[:, :, :], 4, [[NBP, 12], [1, NB]])
            ptb = PTB[:, :, :, :]
            acc = ACC[:, :]
            ac2 = AC2[:, :]
            ac3 = AC3[:, :]

            RVv, RVc = 5 * GK, GK   # RV strides: vec, comp
            NVv = 5 * GK

            # ---- DMA in ----
            # All on the sync ring, in priority order: pivP (gates stage A),
            # vin (gates the ACT sin chain), pivA (gates pJ copy + A-apply),
            # posB (gates the B-tile apply, late).  Host arrays are
            # partition-major so each row is one contiguous descriptor.
            nc.sync.dma_start(
                out=_ap(vv, 0, [[1, GK]]),
                in_=_dram_ap(vinD[:, :], 0, [[GK, P], [1, GK]]),
            )
            if pivp is not None:
                nc.sync.dma_start(
                    out=_ap(pivp, 0, [[1, 12 * GK]]),
                    in_=_dram_ap(pivPd[:, :], 0, [[12 * GK, P], [1, 12 * GK]]),
                )
            nc.sync.dma_start(
                out=_ap(pla, 0, [[1, 3 * G * SP]]),
                in_=_dram_ap(pivA[:, :], 0, [[3 * G * SP, P], [1, 3 * G * SP]]),
            )
            if PLB is not None:
                nc.sync.dma_start(
                    out=_ap(plb, 0, [[1, 3 * G * MB]]),
                    in_=_dram_ap(posB[:, :, :, :], 0,
                                 [[3 * G * MB, P], [1, 3 * G * MB]]),
                )
            # Hillis pad columns must hold finite values (they feed the
            # patched lanes); zero them before the block-prefix scan
            nc.gpsimd.memset(_ap(PT[:, :, :], 0, [[NBP, 12], [1, 4]]), 0.0)
            nc.gpsimd.memset(_ap(PT2[:, :, :], 0, [[NBP, 12], [1, 4]]), 0.0)

            # ---- helpers ----
            tmp_idx = [0]

            def T(dt=f32):
                tmp_idx[0] += 1
                return pool.tile([P, G, K], dt, name=f"tmp{tmp_idx[0]}")

            def mul(a, b):
                o = T(); nc.vector.tensor_mul(o, a, b); return o

            def add(a, b):
                o = T(); nc.vector.tensor_add(o, a, b); return o

            def sub(a, b):
                o = T(); nc.vector.tensor_sub(o, a, b); return o

            def aff(a, scale, bias):
                o = T()
                nc.scalar.activation(o, a, Act.Identity, bias=bias, scale=scale)
                return o

            def activ(a, fn):
                o = T(); nc.scalar.activation(o, a, fn); return o

            def dot3v(a_base, a_off, a_cs, b_base, b_off, b_cs, eng=None):
                """dot over xyz comps via one mul + one innermost-reduce.
                a/b given as (tile_ap, elem offset, comp stride); both must
                have gk contiguous (stride 1)."""
                tmp_idx[0] += 1
                dp = pool.tile([P, GK, 3], f32, name=f"dp{tmp_idx[0]}")[:, :, :]
                (eng or nc.vector).tensor_mul(
                    dp,
                    _ap(a_base, a_off, [[1, GK], [a_cs, 3]]),
                    _ap(b_base, b_off, [[1, GK], [b_cs, 3]]),
                )
                o = T()
                nc.vector.tensor_reduce(
                    _ap(o, 0, [[1, GK]]), dp, mybir.AxisListType.X, Alu.add)
                return o

            # ---- pivot sources ----
            if not arange_quads:
                PIV = pool.tile([P, 3, G, 4, K], f32)
                pv = PIV[:, :, :, :, :]
                for k in range(K):
                    for q in range(4):
                        nc.vector.tensor_copy(
                            _ap(pv, q * K + k, [[G * 4 * K, 3], [4 * K, G]]),
                            _ap(pla, int(angles[k, q]),
                                [[G * SP, 3], [SP, G]]),
                        )

            def piv_ap(c, q):
                if arange_quads:
                    return _ap(pivp, c * 4 * GK + q * GK, [[K, G], [1, K]])
                return _ap(pv, c * G * 4 * K + q * K, [[4 * K, G], [1, K]])

            pJ = [piv_ap(c, 1) for c in range(3)]

            def _ap_cat3(_pj):
                # the three pJ views share a regular comp stride; rebuild as
                # one 3-dim AP [c][g][k]
                if arange_quads:
                    return _ap(pivp, GK, [[4 * GK, 3], [1, GK]])
                return _ap(pv, K, [[G * 4 * K, 3], [4 * K, G], [1, K]])

            # ---- stage A: packed r-vectors and cross products ----
            if arange_quads:
                # quad-permuted pivots: v-dim is the q axis, (g,k) contiguous
                nc.vector.tensor_sub(
                    _ap(rv, 0, [[RVv, 3], [RVc, 3], [1, GK]]),
                    _ap(pivp, GK, [[GK, 3], [4 * GK, 3], [1, GK]]),
                    _ap(pivp, 0, [[GK, 3], [4 * GK, 3], [1, GK]]))
            else:
                for g in range(G):
                    in1 = _ap(pv, g * 4 * K + K,
                              [[K, 3], [G * 4 * K, 3], [1, K]])
                    in0 = _ap(pv, g * 4 * K + 0,
                              [[K, 3], [G * 4 * K, 3], [1, K]])
                    nc.vector.tensor_sub(
                        _ap(rv, g * K, [[RVv, 3], [RVc, 3], [1, K]]), in1, in0)
            # duplicate comps x,y into slots 3,4
            nc.vector.tensor_copy(
                _ap(rv, 3 * RVc, [[RVv, 3], [RVc, 2], [1, GK]]),
                _ap(rv, 0, [[RVv, 3], [RVc, 2], [1, GK]]))
            # nIJK, nJKL = cross(A=[rIJ,rJK], B=[rJK,rKL]) via comp offsets
            nc.vector.tensor_mul(
                _ap(t1v, 0, [[3 * GK, 2], [GK, 3], [1, GK]]),
                _ap(rv, RVc, [[RVv, 2], [RVc, 3], [1, GK]]),
                _ap(rv, RVv + 2 * RVc, [[RVv, 2], [RVc, 3], [1, GK]]))
            nc.vector.tensor_mul(
                _ap(t2v, 0, [[3 * GK, 2], [GK, 3], [1, GK]]),
                _ap(rv, 2 * RVc, [[RVv, 2], [RVc, 3], [1, GK]]),
                _ap(rv, RVv + RVc, [[RVv, 2], [RVc, 3], [1, GK]]))
            nc.vector.tensor_sub(
                _ap(nn, 0, [[NVv, 2], [GK, 3], [1, GK]]),
                _ap(t1v, 0, [[3 * GK, 2], [GK, 3], [1, GK]]),
                _ap(t2v, 0, [[3 * GK, 2], [GK, 3], [1, GK]]))
            nc.vector.tensor_copy(
                _ap(nn, 3 * GK, [[NVv, 2], [GK, 2], [1, GK]]),
                _ap(nn, 0, [[NVv, 2], [GK, 2], [1, GK]]))
            # m = nIJK x rJK -> NN vec slot 2
            nc.vector.tensor_mul(
                _ap(t1v, 0, [[GK, 3], [1, GK]]),
                _ap(nn, GK, [[GK, 3], [1, GK]]),
                _ap(rv, RVv + 2 * RVc, [[RVc, 3], [1, GK]]))
            nc.vector.tensor_mul(
                _ap(t2v, 0, [[GK, 3], [1, GK]]),
                _ap(nn, 2 * GK, [[GK, 3], [1, GK]]),
                _ap(rv, RVv + RVc, [[RVc, 3], [1, GK]]))
            nc.vector.tensor_sub(
                _ap(nn, 2 * NVv, [[GK, 3], [1, GK]]),
                _ap(t1v, 0, [[GK, 3], [1, GK]]),
                _ap(t2v, 0, [[GK, 3], [1, GK]]))

            # sin/cos of targets with range reduction (Sin table ok |x|<~3.55)
            # — emitted here, early, so the ACT hops overlap the cross
            # products instead of stalling the affine assembly later
            def reduced_sin(shift_quarter, extra):
                q = aff(vv, 1.0 / TWO_PI, 1024.0 + shift_quarter)
                qi = T(i32)
                nc.vector.tensor_copy(qi, q)     # f32->i32 rounds to nearest
                qf = T()
                nc.vector.tensor_copy(qf, qi)
                t = aff(qf, -TWO_PI, 1024.0 * TWO_PI + extra)
                return activ(add(vv, t), Act.Sin)

            sv = reduced_sin(0.0, 0.0)
            cv = reduced_sin(0.25, _HALF_PI)

            # pJ source: read straight out of pivP when available, else make
            # a compact copy
            if arange_quads:
                pj_b, pj_off, pj_cs = pivp, GK, 4 * GK
            else:
                PJC = pool.tile([P, 3, G, K], f32)
                pjc = PJC[:, :, :, :]
                nc.vector.tensor_copy(_ap(pjc, 0, [[GK, 3], [K, G], [1, K]]),
                                      _ap_cat3(pJ))
                pj_b, pj_off, pj_cs = pjc, 0, GK

            def rvec(v, c):
                return _ap(rv, v * RVv + c * RVc, [[K, G], [1, K]])

            def nvec(v, c):
                return _ap(nn, v * NVv + c * GK, [[K, G], [1, K]])

            rJK = [rvec(1, c) for c in range(3)]
            rjk_off = RVv                      # RV vec 1, comp stride RVc
            m_off = 2 * NVv                    # m lives in NN vec 2

            # paired dot products: one mul+reduce covers two dots that share
            # a left operand; results land adjacently for fused downstream ops
            DOTS = pool.tile([P, 4, GK], f32)  # rows: x0, l1^2, y0, lm^2
            DP4 = pool.tile([P, 2, GK, 3], f32)
            dots = DOTS[:, :, :]
            dp4 = DP4[:, :, :, :]
            # {x0, l1^2} = nIJK . (nJKL, nIJK)
            nc.vector.tensor_mul(
                dp4,
                _ap(nn, 0, [[0, 2], [1, GK], [GK, 3]]),
                _ap(nn, NVv, [[-NVv, 2], [1, GK], [GK, 3]]))
            nc.vector.tensor_reduce(
                _ap(dots, 0, [[GK, 2], [1, GK]]), dp4,
                mybir.AxisListType.X, Alu.add)
            # y0 = m . nJKL (single dot; reuse dp4's first GK*3 lane block)
            nc.vector.tensor_mul(
                _ap(dp4, 0, [[3, GK], [1, 3]]),
                _ap(nn, m_off, [[1, GK], [GK, 3]]),
                _ap(nn, NVv, [[1, GK], [GK, 3]]))
            nc.vector.tensor_reduce(
                _ap(dots, 2 * GK, [[1, GK]]),
                _ap(dp4, 0, [[3, GK], [1, 3]]),
                mybir.AxisListType.X, Alu.add)
            jks = dot3v(rv, rjk_off, RVc, rv, rjk_off, RVc)
            # lm^2 = l1^2 * |rJK|^2  (m = nIJK x rJK with nIJK _|_ rJK)
            nc.vector.tensor_mul(
                _ap(dots, 3 * GK, [[1, GK]]),
                _ap(dots, GK, [[1, GK]]),
                _ap(jks[:, :, :], 0, [[1, GK]]))
            L1LM = pool.tile([P, 2, GK], f32)  # (l1, lm)
            l1lm = L1LM[:, :, :]
            nc.scalar.activation(
                _ap(l1lm, 0, [[GK, 2], [1, GK]]),
                _ap(dots, GK, [[2 * GK, 2], [1, GK]]), Act.Sqrt)
            XY = pool.tile([P, 2, GK], f32)    # (x1, y1) = (x0*lm, y0*l1)
            xy = XY[:, :, :]
            nc.vector.tensor_mul(
                xy,
                _ap(dots, 0, [[2 * GK, 2], [1, GK]]),
                _ap(l1lm, GK, [[-GK, 2], [1, GK]]))
            SQ = pool.tile([P, 2, GK], f32)
            sq = SQ[:, :, :]
            nc.vector.tensor_mul(sq, xy, xy)
            hs = T()
            nc.vector.tensor_add(_ap(hs[:, :, :], 0, [[1, GK]]),
                                 _ap(sq, 0, [[1, GK]]),
                                 _ap(sq, GK, [[1, GK]]))
            hr = T(); nc.vector.reciprocal(hr, hs)
            rh = activ(hr, Act.Sqrt)            # 1/hypot
            CS = pool.tile([P, 2, GK], f32)     # (ccur, scur)
            cs_ = CS[:, :, :]
            nc.vector.tensor_mul(
                cs_, xy, _ap(rh[:, :, :], 0, [[0, 2], [1, GK]]))
            jkr = T(); nc.vector.reciprocal(jkr, jks)
            jrs = activ(jkr, Act.Sqrt)          # 1/|rJK|
            AXT = pool.tile([P, 3, G, K], f32)
            axt = AXT[:, :, :, :]
            nc.vector.tensor_mul(
                _ap(axt, 0, [[GK, 3], [1, GK]]),
                _ap(rv, rjk_off, [[RVc, 3], [1, GK]]),
                _ap(jrs[:, :, :], 0, [[0, 3], [1, GK]]),
            )
            ax = [_ap(axt, c * GK, [[K, G], [1, K]]) for c in range(3)]

            PC1 = pool.tile([P, 2, GK], f32)   # cv * (ccur, scur)
            PC2 = pool.tile([P, 2, GK], f32)   # sv * (ccur, scur)
            pc1 = PC1[:, :, :]
            pc2 = PC2[:, :, :]
            nc.vector.tensor_mul(pc1, cs_, _ap(cv[:, :, :], 0, [[0, 2], [1, GK]]))
            nc.vector.tensor_mul(pc2, cs_, _ap(sv[:, :, :], 0, [[0, 2], [1, GK]]))
            c_ = T()
            s_ = T()
            nc.vector.tensor_add(_ap(c_[:, :, :], 0, [[1, GK]]),
                                 _ap(pc1, 0, [[1, GK]]), _ap(pc2, GK, [[1, GK]]))
            nc.vector.tensor_sub(_ap(s_[:, :, :], 0, [[1, GK]]),
                                 _ap(pc2, 0, [[1, GK]]), _ap(pc1, GK, [[1, GK]]))
            t1_ = T()
            nc.vector.tensor_scalar(t1_, c_, -1.0, 1.0, Alu.mult, Alu.add)  # 1-cos

            TAX = pool.tile([P, 3, G, K], f32)
            SAX = pool.tile([P, 3, G, K], f32)
            UD = pool.tile([P, 3, G, K], f32)
            OD = pool.tile([P, 2, G, K], f32)
            taxv = TAX[:, :, :, :]
            saxv = SAX[:, :, :, :]
            udv = UD[:, :, :, :]
            odv = OD[:, :, :, :]
            d3 = [[GK, 3], [1, GK]]
            bc3 = [[0, 3], [1, GK]]
            nc.vector.tensor_mul(_ap(taxv, 0, d3), _ap(axt, 0, d3),
                                 _ap(t1_[:, :, :], 0, bc3))
            nc.vector.tensor_mul(_ap(saxv, 0, d3), _ap(axt, 0, d3),
                                 _ap(s_[:, :, :], 0, bc3))
            nc.vector.tensor_mul(_ap(udv, 0, d3), _ap(taxv, 0, d3),
                                 _ap(axt, 0, d3))

            def aq(q):
                return _ap(at, q * GK, [[K, G], [1, K]])

            # diagonal: q = 0,5,10 -> stride 5*GK
            nc.vector.tensor_add(
                _ap(at, 0, [[5 * GK, 3], [1, GK]]),
                _ap(udv, 0, d3),
                _ap(c_[:, :, :], 0, bc3),
            )
            # off-diagonal products: txy,txz = tax0*(ax1,ax2); tyz = tax1*ax2
            nc.vector.tensor_mul(
                _ap(odv, 0, [[GK, 2], [1, GK]]),
                _ap(axt, GK, [[GK, 2], [1, GK]]),
                _ap(taxv, 0, [[0, 2], [1, GK]]),
            )
            tyz = T()
            nc.vector.tensor_mul(tyz, _ap(taxv, GK, [[K, G], [1, K]]),
                                 _ap(axt, 2 * GK, [[K, G], [1, K]]))
            txy = _ap(odv, 0, [[K, G], [1, K]])
            txz = _ap(odv, GK, [[K, G], [1, K]])
            sax = [_ap(saxv, c * GK, [[K, G], [1, K]]) for c in range(3)]
            nc.vector.tensor_sub(aq(1), txy, sax[2])
            nc.vector.tensor_add(aq(4), txy, sax[2])
            nc.vector.tensor_add(aq(2), txz, sax[1])
            nc.vector.tensor_sub(aq(8), txz, sax[1])
            nc.vector.tensor_sub(aq(6), tyz, sax[0])
            nc.vector.tensor_add(aq(9), tyz, sax[0])

            # b = pJ - R @ pJ : batched products, reduce, sub
            BP = pool.tile([P, 3, GK, 3], f32)
            bp = BP[:, :, :, :]
            nc.vector.tensor_mul(
                bp,
                _ap(at, 0, [[4 * GK, 3], [1, GK], [GK, 3]]),
                _ap(pj_b, pj_off, [[0, 3], [1, GK], [pj_cs, 3]]),
            )
            RPJ = pool.tile([P, 3, G, K], f32)
            rpj = RPJ[:, :, :, :]
            nc.vector.tensor_reduce(
                _ap(rpj, 0, [[GK, 3], [1, GK]]), bp,
                mybir.AxisListType.X, Alu.add)
            nc.vector.tensor_sub(
                _ap(at, 3 * GK, [[4 * GK, 3], [1, GK]]),
                _ap(pj_b, pj_off, [[pj_cs, 3], [1, GK]]),
                _ap(rpj, 0, [[GK, 3], [1, GK]]),
            )

            # ---- stage B: blocked prefix composition ----
            at_flat = _ap(at, 0, [[GK, 12], [1, GK]])

            def compose(dst, dq, dbd, doff, left, lq, lbd, loff,
                        right, rq, rbd, roff):
                """dst[i,j,*] = sum_m left[i,m,*]*right[m,j,*]; dst[i,3,*] +=
                left[i,3,*].  *bd = batch [step,count] dims (equal counts)."""
                counts = [d[1] for d in dbd]
                assert [d[1] for d in lbd] == counts
                assert [d[1] for d in rbd] == counts
                nb = 1
                for cnt in counts:
                    nb *= cnt
                abd = []
                stp = 1
                for cnt in reversed(counts):
                    abd.insert(0, [stp, cnt])
                    stp *= cnt

                def accv(base):
                    return _ap(base, 0, [[4 * nb, 3], [nb, 4]] + abd)

                dstv = _ap(dst, doff, [[4 * dq, 3], [dq, 4]] + dbd)

                def dmul(tgt, mrow):
                    nc.vector.tensor_mul(
                        accv(tgt),
                        _ap(right, roff + 4 * mrow * rq,
                            [[0, 3], [rq, 4]] + rbd),
                        _ap(left, loff + mrow * lq,
                            [[4 * lq, 3], [0, 4]] + lbd),
                    )

                dmul(acc, 0)
                dmul(ac2, 1)
                nc.vector.tensor_add(accv(acc), accv(acc), accv(ac2))
                dmul(ac2, 2)
                nc.vector.tensor_add(dstv, accv(acc), accv(ac2))
                bias_d = _ap(dst, doff + 3 * dq, [[4 * dq, 3]] + dbd)
                nc.vector.tensor_add(
                    bias_d, bias_d,
                    _ap(left, loff + 3 * lq, [[4 * lq, 3]] + lbd),
                )

            # within-block scan, in place: A[:, t] <- A[:, t-1] o A[:, t]
            # (the 3 muls read the slot before the final add overwrites it)
            for t in range(1, L):
                compose(at_flat, GK, [[L, NB]], t,
                        at_flat, GK, [[L, NB]], t - 1,
                        at_flat, GK, [[L, NB]], t)
            # block products
            nc.vector.tensor_copy(
                _ap(pt, 0, [[NBP, 12], [1, NB]]),
                _ap(at_flat, L - 1, [[GK, 12], [L, NB]]),
            )
            # block-prefix scan: Hillis-Steele over the flattened (g,b) axis.
            # Lanes j%B < s read the neighbour's tail (garbage) and are
            # patched by the trailing copy before the buffers swap.
            src_pt, dst_pt = pt, pt2
            s = 1
            while s < B:
                compose(dst_pt, NBP, [[1, NB]], 0,
                        src_pt, NBP, [[1, NB]], -s,
                        src_pt, NBP, [[1, NB]], 0)
                nc.vector.tensor_copy(
                    _ap(dst_pt, 0, [[NBP, 12], [B, G], [1, s]]),
                    _ap(src_pt, 0, [[NBP, 12], [B, G], [1, s]]))
                src_pt, dst_pt = dst_pt, src_pt
                s *= 2
            ptf = src_pt

            # ---- stage C ----
            def dma_out_tile(which):
                """DMA an output tile, one component plane at a time, so the
                early planes drain while compute finishes the last one.
                Tile's subtile deps make each wait only on its own writers."""
                if which == "A":
                    for i in range(3):
                        nc.scalar.dma_start(
                            out=_dram_ap(outAd[:, :], i * G * SP,
                                         [[3 * G * SP, P], [1, G * SP]]),
                            in_=_ap(outa, i * G * SP, [[1, G * SP]]),
                        )
                else:
                    for i in range(3):
                        nc.scalar.dma_start(
                            out=_dram_ap(outBd[:, :], i * G * MB,
                                         [[3 * G * MB, P], [1, G * MB]]),
                            in_=_ap(outb, i * G * MB, [[1, G * MB]]),
                        )

            def apply_single_from(coef, coefq, coefoff, m0, length):
                """out[:, :, m0:m0+length] = R@p + b with per-(partition,g)
                scalar coefficients from `coef` (q stride coefq, g stride
                coefoff).  Muls on ACT (per-partition scale), adds on DVE."""
                plbase, mloc, gs, cs = pl_view(m0, length, None)
                obase, omloc, ogs, ocs = out_view(m0, length)
                tmp_idx[0] += 1
                prod = [[pool.tile([P, G * length], f32,
                                   name=f"prod{tmp_idx[0]}_{i}_{cc}")[:, :]
                         for cc in range(3)] for i in range(3)]
                for i in range(3):
                    for cc in range(3):
                        for g in range(G):
                            nc.scalar.activation(
                                _ap(prod[i][cc], g * length, [[1, length]]),
                                _ap(plbase, cc * cs + g * gs + mloc,
                                    [[1, length]]),
                                Act.Identity,
                                scale=_ap(coef, (4 * i + cc) * coefq
                                          + g * coefoff, [[1, 1]]),
                            )
                for i in range(3):
                    d_t = [[length, G], [1, length]]
                    s1 = _ap(prod[i][0], 0, d_t)
                    nc.vector.tensor_add(s1, s1, _ap(prod[i][1], 0, d_t))
                    nc.vector.tensor_add(s1, s1, _ap(prod[i][2], 0, d_t))
                    for g in range(G):
                        # + translation via the ACT bias port (keeps DVE free)
                        nc.scalar.activation(
                            _ap(obase, i * ocs + g * ogs + omloc, [[1, length]]),
                            _ap(prod[i][0], g * length, [[1, length]]),
                            Act.Identity,
                            bias=_ap(coef, (4 * i + 3) * coefq + g * coefoff,
                                     [[1, 1]]),
                            scale=1.0,
                        )

            pt_last = bass.AP(tensor=ptf.tensor, offset=ptf.offset + (B - 1),
                              ap=list(ptf.ap))

            def apply_runs(starts, length, ks):
                nr = len(starts)
                if nr == 1 and ks[0] == K - 1:
                    # chain-last prefix == last block product: ready right
                    # after the block-prefix scan, before distribute.
                    apply_single_from(pt_last, NBP, B, starts[0], length)
                    return
                if nr == 1:
                    base = bass.AP(tensor=ct.tensor, offset=ct.offset + ks[0],
                                   ap=list(ct.ap))
                    apply_single_from(base, GK, K, starts[0], length)
                    return
                sm = starts[1] - starts[0]
                sk = ks[1] - ks[0]
                m0, k0 = starts[0], ks[0]
                span = max(starts) + length - m0
                plbase, mloc, gs, cs = pl_view(m0, span, None)
                obase, omloc, ogs, ocs = out_view(m0, span)
                d_pl = [[gs, G], [sm, nr], [1, length]]
                d_out = [[ogs, G], [sm, nr], [1, length]]
                d_c = [[K, G], [sk, nr], [0, length]]
                d_acc = [[nr * length, G], [length, nr], [1, length]]
                for i in range(3):
                    for cc in range(2):
                        tgt = acc if cc == 0 else ac2
                        nc.vector.tensor_mul(
                            _ap(tgt, 0, d_acc),
                            _ap(plbase, cc * cs + mloc, d_pl),
                            _ap(ct, (4 * i + cc) * GK + k0, d_c),
                        )
                    nc.vector.tensor_add(
                        _ap(acc, 0, d_acc), _ap(acc, 0, d_acc), _ap(ac2, 0, d_acc)
                    )
                    nc.vector.tensor_mul(
                        _ap(ac2, 0, d_acc),
                        _ap(plbase, 2 * cs + mloc, d_pl),
                        _ap(ct, (4 * i + 2) * GK + k0, d_c),
                    )
                    nc.vector.tensor_add(
                        _ap(acc, 0, d_acc), _ap(acc, 0, d_acc),
                        _ap(ac2, 0, d_acc),
                    )
                    nc.vector.tensor_add(
                        _ap(obase, i * ocs + omloc, d_out),
                        _ap(acc, 0, d_acc),
                        _ap(ct, (4 * i + 3) * GK + k0, d_c),
                    )

            def emit_distribute():
                # distribute: block 0 copies, blocks b>=1 get P[b-1] @ W
                nk = (B - 1) * L
                nc.vector.tensor_copy(
                    _ap(ct, 0, [[GK, 12], [K, G], [1, L]]),
                    _ap(at_flat, 0, [[GK, 12], [K, G], [1, L]]),
                )
                # broadcast block prefixes over t so g fuses into 3-dim APs:
                # PTB[g][q][jb*L + t] = ptf[q][g*B + jb]
                for g in range(G):
                    nc.vector.tensor_copy(
                        _ap(ptb, g * 12 * nk, [[nk, 12], [L, B - 1], [1, L]]),
                        _ap(ptf, g * B, [[NBP, 12], [1, B - 1], [0, L]]))
                d_w = [[GK, 4], [K, G], [1, nk]]
                d_a = [[G * nk, 4], [nk, G], [1, nk]]
                accs = (acc, ac2, ac3)
                # all 9 cross products first (pure reads of W and P), then
                # the combines
                for m in range(3):
                    for i in range(3):
                        nc.vector.tensor_mul(
                            _ap(accs[m], i * 4 * G * nk, d_a),
                            _ap(at_flat, 4 * m * GK + L, d_w),
                            _ap(ptb, (4 * i + m) * nk,
                                [[0, 4], [12 * nk, G], [1, nk]]),
                        )
                for i in range(3):
                    o = i * 4 * G * nk
                    nc.vector.tensor_add(
                        _ap(acc, o, d_a), _ap(acc, o, d_a), _ap(ac2, o, d_a))
                    nc.vector.tensor_add(
                        _ap(ct, 4 * i * GK + L, d_w),
                        _ap(acc, o, d_a), _ap(ac3, o, d_a))
                    nc.vector.tensor_add(
                        _ap(ct, (4 * i + 3) * GK + L, [[K, G], [1, nk]]),
                        _ap(ct, (4 * i + 3) * GK + L, [[K, G], [1, nk]]),
                        _ap(ptb, (4 * i + 3) * nk, [[12 * nk, G], [1, nk]]),
                    )

            # unmoved atoms: copy + DMA as soon as PL lands
            unmoved = [m for m in range(M) if km[m] < 0]
            u0 = 0
            while u0 < len(unmoved):
                u1 = u0
                while u1 + 1 < len(unmoved) and unmoved[u1 + 1] == unmoved[u1] + 1:
                    u1 += 1
                a0, ln = unmoved[u0], u1 - u0 + 1
                assert a0 + ln <= SP or a0 >= SPB
                ubase, umloc, ugs, ucs = pl_view(a0, ln, None)
                uobase, uomloc, uogs, uocs = out_view(a0, ln)
                nc.vector.tensor_copy(
                    _ap(uobase, uomloc, [[uocs, 3], [uogs, G], [1, ln]]),
                    _ap(ubase, umloc, [[ucs, 3], [ugs, G], [1, ln]]),
                )
                u0 = u1 + 1

            # classes: chain-last single-run first (overlaps distribute)
            by_len = {}
            for (m0, ln, k) in runs:
                by_len.setdefault(ln, []).append((m0, k))
            classes = sorted(
                by_len.items(),
                key=lambda kv: 0 if (len(kv[1]) == 1 and kv[1][0][1] == K - 1)
                else 1)
            emitted_distribute = False
            for ln, rs in classes:
                starts = [r[0] for r in rs]
                ks = [r[1] for r in rs]
                nr = len(rs)
                chain_last_single = nr == 1 and ks[0] == K - 1
                if not chain_last_single and not emitted_distribute:
                    emit_distribute()
                    emitted_distribute = True
                regular = nr <= 2 or (
                    all(starts[r] == starts[0] + r * (starts[1] - starts[0])
                        for r in range(nr))
                    and all(ks[r] == ks[0] + r * (ks[1] - ks[0])
                            for r in range(nr))
                )
                if regular:
                    apply_runs(starts, ln, ks)
                else:
                    for (m0, k) in rs:
                        apply_runs([m0], ln, [k])
            # B tile drains first (its writers finish early); HWDGE is FIFO
            # per engine so the early DMA must be queued first
            if OUTB is not None:
                dma_out_tile("B")
            dma_out_tile("A")

    _split_multi_waits(nc)
    return nc


_BUILD_CACHE = {}


def make_in_maps(input, pos, angles, move_mask):
    input = np.asarray(input, dtype=np.float32)
    pos = np.asarray(pos, dtype=np.float32)
    angles = np.asarray(angles)
    N, K = input.shape
    M = pos.shape[1]
    NL = N // NCORES
    G = NL // P
    SP, SPB = _seg_bounds(angles, np.asarray(move_mask).astype(bool), M)
    arange_quads = bool((angles == np.arange(K * 4).reshape(K, 4)).all())
    in_maps = []
    for c in range(NCORES):
        sl = slice(c * NL, (c + 1) * NL)
        # (NL, M, 3) -> (P, 3, G, M): partition-major so each partition row
        # is one contiguous DMA descriptor
        pm = pos[sl].reshape(G, P, M, 3).transpose(1, 3, 0, 2)
        vrows = input[sl].reshape(G, P, K).transpose(1, 0, 2).reshape(P, G * K)
        im = {"vin": np.ascontiguousarray(vrows),
              "pivA": np.ascontiguousarray(
                  pm[:, :, :, :SP].reshape(P, 3 * G * SP))}
        if arange_quads:
            # pivP[p][c][q][g][k] = pm[p][c][g][4k+q]
            pp = pm[:, :, :, :4 * K].reshape(P, 3, G, K, 4)
            im["pivP"] = np.ascontiguousarray(
                pp.transpose(0, 1, 4, 2, 3).reshape(P, 12 * G * K))
        if SPB < M:
            im["posB"] = np.ascontiguousarray(pm[:, :, :, SPB:])
        in_maps.append(im)
    return in_maps


def kernel(input, pos, angles, move_mask):
    input = np.ascontiguousarray(np.asarray(input, dtype=np.float32))
    pos = np.ascontiguousarray(np.asarray(pos, dtype=np.float32))
    angles = np.asarray(angles)
    move_mask = np.asarray(move_mask).astype(bool)

    N, K = input.shape
    _, M, three = pos.shape
    assert three == 3
    assert N % (NCORES * P) == 0
    NL = N // NCORES

    key = (N, K, M, angles.tobytes(), move_mask.tobytes())
    nc = _BUILD_CACHE.get(key)
    if nc is None:
        nc = _build(angles, move_mask, NL, K, M)
        _BUILD_CACHE[key] = nc

    in_maps = make_in_maps(input, pos, angles, move_mask)

    # the axon-proxied NRT occasionally wedges transiently
    # (NRT_EXEC_UNIT_UNRECOVERABLE); one retry recovers it
    try:
        res = run_bass_kernel_spmd(nc, in_maps, list(range(NCORES)))
    except Exception:
        res = run_bass_kernel_spmd(nc, in_maps, list(range(NCORES)))

    G = NL // P
    SP, SPB = _seg_bounds(angles, move_mask, M)
    out = np.empty((N, M, 3), dtype=np.float32)
    for c in range(NCORES):
        sl = slice(c * NL, (c + 1) * NL)
        # (P, 3, G, cols) -> (NL, cols, 3)
        oa = res.results[c]["outA"].reshape(P, 3, G, SP)
        out[sl, :SPB] = oa.transpose(2, 0, 3, 1).reshape(NL, SP, 3)[:, :SPB]
        if SPB < M:
            ob = res.results[c]["outB"].reshape(P, 3, G, M - SPB)
            out[sl, SPB:] = ob.transpose(2, 0, 3, 1).reshape(NL, M - SPB, 3)
    return out



# revision 70
# speedup vs baseline: 1.1630x; 1.0339x over previous
"""Dihedral2Coord Trainium2 kernel.

Math: the reference applies K sequential dihedral-set steps; step k rotates
a suffix of the atom chain rigidly about the current J-K bond.  Every step's
transform is a proper rigid motion that moves all four pivot atoms of every
later step together, so the dihedral measured at application time equals the
dihedral of the ORIGINAL coordinates (dihedrals are invariant under rigid
motion).  Step k's rotation, expressed in original coordinates, is therefore
a fixed affine A_k computable from the original positions alone, and the
cumulative transform is the prefix product C_k = A_0 @ A_1 @ ... @ A_k.
The kernel:
  A) computes all K per-conformer Rodrigues affines in parallel,
  B) prefix-composes them with a blocked scan,
  C) applies C_{km(m)} to each atom run, where km(m) counts the steps whose
     mask includes atom m (verified prefix-structured on host).

Sharding: pure data parallelism over conformers N across 8 cores (SPMD).
"""

import sys

import numpy as np

try:
    import concourse.bass as bass
except ImportError:  # path in the grading container
    sys.path.insert(0, "/opt/trn_rl_repo")
    import concourse.bass as bass

import concourse.tile as tile
from concourse import mybir
from concourse.bass_utils import run_bass_kernel_spmd

f32 = mybir.dt.float32
i32 = mybir.dt.int32
Alu = mybir.AluOpType
Act = mybir.ActivationFunctionType

NCORES = 8
P = 128
TWO_PI = float(2.0 * np.pi)
_HALF_PI = float(np.pi / 2)

_WAIT_CAP = 1  # this walrus build rejects >1 sync-wait per instruction


def _register_const(nc, value, dtype=f32):
    """Register an activation-bias constant.  Written on the Activation
    engine from the framework's const-1.0 AP (ordered by Bass.__init__'s
    barrier); later ACT reads are same-engine program-ordered, so no extra
    barrier is needed."""
    if (dtype, value) in nc.const_aps.aps:
        return
    t = nc.alloc_sbuf_tensor(f"const-{dtype.name}-{value}", [128, 1], dtype)
    one = nc.const_aps.aps[(f32, 1.0)]
    nc.scalar.activation(t.ap(), one, Act.Identity, bias=0.0, scale=float(value))
    nc.const_aps.aps[(dtype, value)] = t.ap()


def _split_multi_waits(nc):
    """Split every instruction carrying >cap sync-waits into single-wait
    NoOps (same engine, immediately before, same block).  Waits are monotone
    semaphore conditions so this preserves semantics exactly."""
    n = 0
    for func in nc.m.functions:
        for bb in func.blocks:
            old = list(bb.instructions)
            if not any(
                i.sync_info is not None and len(i.sync_info.on_wait) > _WAIT_CAP
                for i in old
            ):
                continue
            new = []
            for inst in old:
                si = inst.sync_info
                if si is not None and len(si.on_wait) > _WAIT_CAP:
                    waits = list(si.on_wait)
                    head, tail = waits[:-_WAIT_CAP], waits[-_WAIT_CAP:]
                    for j in range(0, len(head), _WAIT_CAP):
                        n += 1
                        new.append(
                            mybir.InstNoOp(
                                name=f"{inst.name}_ws{j}",
                                engine=inst.engine,
                                sync_info=mybir.SyncInfo(
                                    on_wait=list(head[j : j + _WAIT_CAP]), on_update=[]
                                ),
                                bass_nofuse=True,
                            )
                        )
                    try:
                        si.on_wait[:] = tail
                    except TypeError:
                        inst.sync_info = mybir.SyncInfo(
                            on_wait=tail, on_update=list(si.on_update)
                        )
                new.append(inst)
            try:
                bb.instructions[:] = new
            except TypeError:
                bb.instructions = new
    return n


def _ap(base, offset_elems, dims):
    """Free-dim AP view into an SBUF tile AP `base` (partition dim kept).
    dims: list of [step, count] in elements of the tile's free space."""
    return bass.AP(
        tensor=base.tensor,
        offset=base.offset + offset_elems,
        ap=[list(base.ap[0])] + [list(d) for d in dims],
    )


def _dram_ap(t, offset, dims):
    return bass.AP(tensor=t.tensor, offset=offset, ap=[list(d) for d in dims])


def _analyse_mask(angles, move_mask):
    """Host-side structural analysis. Returns (km, runs): km[m] is the last
    step applied to atom m (-1 = never moved); runs are (start, len, k)."""
    K, M = move_mask.shape
    km = move_mask.astype(np.int64).sum(0) - 1
    kk = np.arange(K)[:, None]
    if not (move_mask == (kk <= km[None, :])).all():
        raise NotImplementedError("move_mask is not prefix-structured per atom")
    for k in range(K):
        for a in angles[k]:
            if not move_mask[:k, a].all():
                raise NotImplementedError("pivot atoms not rigidly co-moved")
    runs = []
    m = 0
    while m < M:
        j = m
        while j + 1 < M and km[j + 1] == km[m]:
            j += 1
        if km[m] >= 0:
            runs.append((m, j - m + 1, int(km[m])))
        m = j + 1
    return km, runs


def _seg_bounds(angles, move_mask, M):
    """(SP, SPB): pivot region [0, SP); B-tile starts at SPB <= SP so no
    run/unmoved segment crosses a tile boundary (columns [SPB, SP) are
    duplicated in both tiles)."""
    km, runs = _analyse_mask(angles, move_mask)
    SP = min(int(np.asarray(angles).max()) + 1, M)
    segs = [(m0, ln) for (m0, ln, _k) in runs]
    m = 0
    while m < M:
        if km[m] < 0:
            j = m
            while j + 1 < M and km[j + 1] < 0:
                j += 1
            segs.append((m, j - m + 1))
            m = j + 1
        else:
            m += 1
    SPB = SP
    for (m0, ln) in segs:
        if m0 < SP < m0 + ln:
            SPB = min(SPB, m0)
    return SP, SPB


def _build(angles, move_mask, NL, K, M):
    """Build the Bass module for one core handling NL conformers."""
    G = NL // P
    assert NL == G * P
    GK = G * K
    L = 8               # within-block scan length
    assert K % L == 0
    B = K // L          # blocks per conformer-group
    NB = G * B          # blocks over the flattened (g,k) axis

    angles = np.asarray(angles)
    arange_quads = bool((angles == np.arange(K * 4).reshape(K, 4)).all())
    km, runs = _analyse_mask(angles, move_mask)

    nc = bass.Bass()
    for cval in (1024.0, 1024.25, 1024.0 * TWO_PI, 1024.0 * TWO_PI + _HALF_PI):
        _register_const(nc, float(cval))
    SP, SPB = _seg_bounds(angles, move_mask, M)
    MB = M - SPB        # B-tile width
    vinD = nc.declare_dram_parameter("vin", [P, G * K], f32, isOutput=False)
    pivA = nc.declare_dram_parameter("pivA", [P, 3 * G * SP], f32,
                                     isOutput=False)
    # quad-permuted pivot planes: pivP[c][q][g][k] = pos[., 4k+q, c] so the
    # r-vector subtraction reads contiguously (innermost (g,k))
    pivPd = (nc.declare_dram_parameter("pivP", [P, 12 * G * K], f32,
                                       isOutput=False) if arange_quads
             else None)
    posB = (nc.declare_dram_parameter("posB", [P, 3, G, MB], f32,
                                      isOutput=False) if SPB < M else None)
    # outputs as whole tiles: one contiguous DMA descriptor per partition
    outAd = nc.declare_dram_parameter("outA", [P, 3 * G * SP], f32,
                                      isOutput=True)
    outBd = (nc.declare_dram_parameter("outB", [P, 3 * G * MB], f32,
                                       isOutput=True) if SPB < M else None)

    with tile.TileContext(nc) as tc:
        with tc.tile_pool(name="main", bufs=1) as pool:
            # ---- SBUF tensors ----
            # separate tiles per DMA so consumers wait only on what they need
            VIN = pool.tile([P, G * K], f32)
            PLA = pool.tile([P, 3 * G * SP], f32)
            PIVP = (pool.tile([P, 3, 4, GK], f32, name="PIVP")
                    if arange_quads else None)
            PLB = pool.tile([P, 3, G, MB], f32, name="PLB") if SPB < M else None
            OUTA = pool.tile([P, 3, G, SP], f32)
            OUTB = pool.tile([P, 3, G, MB], f32, name="OUTB") if SPB < M else None
            # packed r-vectors / normals with duplicated xy components so a
            # +1/+2 component rotation is a plain offset (cross-product trick)
            RV = pool.tile([P, 3, 5, G, K], f32)  # (rIJ,rJK,rKL) x (x,y,z,x,y)
            NN = pool.tile([P, 3, 5, G, K], f32)  # (nIJK,nJKL,m) x (x,y,z,x,y)
            TA = pool.tile([P, 2, 3, G, K], f32)
            TB = pool.tile([P, 2, 3, G, K], f32)
            AT = pool.tile([P, G, K, 12], f32)   # A_k; q=4i+j innermost (stride 1)
            CT = pool.tile([P, 12, GK], f32)     # full prefixes
            NBP = NB + 4   # 4 pad columns so Hillis j<s lanes read in-bounds
            PT = pool.tile([P, 12, NBP], f32)    # block products / prefixes
            PT2 = pool.tile([P, 12, NBP], f32)   # Hillis ping-pong buffer
            PTB = pool.tile([P, G, 12, (K // 8) * 7], f32)  # prefixes bcast over t
            ACC = pool.tile([P, 12 * max(GK, 64)], f32)
            AC2 = pool.tile([P, 12 * max(GK, 64)], f32)
            AC3 = pool.tile([P, 12 * max(GK, 64)], f32)

            vv = _ap(VIN[:, :], 0, [[K, G], [1, K]])
            pla = _ap(PLA[:, :], 0, [])
            pivp = PIVP[:, :, :, :] if PIVP is not None else None
            plb = PLB[:, :, :, :] if PLB is not None else None
            outa = OUTA[:, :, :, :]
            outb = OUTB[:, :, :, :] if OUTB is not None else None

            def pl_view(m0, ln, _unused=None):
                """(base_ap, local column offset, group stride, comp stride)
                for columns [m0, m0+ln): B tile from SPB, else A tile."""
                if m0 >= SPB:
                    return plb, m0 - SPB, MB, G * MB
                assert m0 + ln <= SP
                return pla, m0, SP, G * SP

            def out_view(m0, ln):
                if m0 >= SPB:
                    return outb, m0 - SPB, MB, G * MB
                assert m0 + ln <= SP
                return outa, m0, SP, G * SP
            rv = RV[:, :, :, :, :]
            nn = NN[:, :, :, :, :]
            t1v = TA[:, :, :, :, :]
            t2v = TB[:, :, :, :, :]
            at = AT[:, :, :, :]
            ct = CT[:, :, :]
            pt = _ap(PT[:, :, :], 4, [[NBP, 12], [1, NB]])
            pt2 = _ap(PT2[:, :, :], 4, [[NBP, 12], [1, NB]])
            ptb = PTB[:, :, :, :]
            acc = ACC[:, :]
            ac2 = AC2[:, :]
            ac3 = AC3[:, :]

            RVv, RVc = 5 * GK, GK   # RV strides: vec, comp
            NVv = 5 * GK

            # ---- DMA in ----
            # All on the sync ring, in priority order: pivP (gates stage A),
            # vin (gates the ACT sin chain), pivA (gates pJ copy + A-apply),
            # posB (gates the B-tile apply, late).  Host arrays are
            # partition-major so each row is one contiguous descriptor.
            nc.sync.dma_start(
                out=_ap(vv, 0, [[1, GK]]),
                in_=_dram_ap(vinD[:, :], 0, [[GK, P], [1, GK]]),
            )
            if pivp is not None:
                nc.sync.dma_start(
                    out=_ap(pivp, 0, [[1, 12 * GK]]),
                    in_=_dram_ap(pivPd[:, :], 0, [[12 * GK, P], [1, 12 * GK]]),
                )
            nc.sync.dma_start(
                out=_ap(pla, 0, [[1, 3 * G * SP]]),
                in_=_dram_ap(pivA[:, :], 0, [[3 * G * SP, P], [1, 3 * G * SP]]),
            )
            if PLB is not None:
                nc.sync.dma_start(
                    out=_ap(plb, 0, [[1, 3 * G * MB]]),
                    in_=_dram_ap(posB[:, :, :, :], 0,
                                 [[3 * G * MB, P], [1, 3 * G * MB]]),
                )
            # Hillis pad columns must hold finite values (they feed the
            # patched lanes); zero them before the block-prefix scan
            nc.gpsimd.memset(_ap(PT[:, :, :], 0, [[NBP, 12], [1, 4]]), 0.0)
            nc.gpsimd.memset(_ap(PT2[:, :, :], 0, [[NBP, 12], [1, 4]]), 0.0)

            # ---- helpers ----
            tmp_idx = [0]

            def T(dt=f32):
                tmp_idx[0] += 1
                return pool.tile([P, G, K], dt, name=f"tmp{tmp_idx[0]}")

            def mul(a, b):
                o = T(); nc.vector.tensor_mul(o, a, b); return o

            def add(a, b):
                o = T(); nc.vector.tensor_add(o, a, b); return o

            def sub(a, b):
                o = T(); nc.vector.tensor_sub(o, a, b); return o

            def aff(a, scale, bias):
                o = T()
                nc.scalar.activation(o, a, Act.Identity, bias=bias, scale=scale)
                return o

            def activ(a, fn):
                o = T(); nc.scalar.activation(o, a, fn); return o

            def dot3v(a_base, a_off, a_cs, b_base, b_off, b_cs, eng=None):
                """dot over xyz comps via one mul + one innermost-reduce.
                a/b given as (tile_ap, elem offset, comp stride); both must
                have gk contiguous (stride 1)."""
                tmp_idx[0] += 1
                dp = pool.tile([P, GK, 3], f32, name=f"dp{tmp_idx[0]}")[:, :, :]
                (eng or nc.vector).tensor_mul(
                    dp,
                    _ap(a_base, a_off, [[1, GK], [a_cs, 3]]),
                    _ap(b_base, b_off, [[1, GK], [b_cs, 3]]),
                )
                o = T()
                nc.vector.tensor_reduce(
                    _ap(o, 0, [[1, GK]]), dp, mybir.AxisListType.X, Alu.add)
                return o

            # ---- pivot sources ----
            if not arange_quads:
                PIV = pool.tile([P, 3, G, 4, K], f32)
                pv = PIV[:, :, :, :, :]
                for k in range(K):
                    for q in range(4):
                        nc.vector.tensor_copy(
                            _ap(pv, q * K + k, [[G * 4 * K, 3], [4 * K, G]]),
                            _ap(pla, int(angles[k, q]),
                                [[G * SP, 3], [SP, G]]),
                        )

            def piv_ap(c, q):
                if arange_quads:
                    return _ap(pivp, c * 4 * GK + q * GK, [[K, G], [1, K]])
                return _ap(pv, c * G * 4 * K + q * K, [[4 * K, G], [1, K]])

            pJ = [piv_ap(c, 1) for c in range(3)]

            def _ap_cat3(_pj):
                # the three pJ views share a regular comp stride; rebuild as
                # one 3-dim AP [c][g][k]
                if arange_quads:
                    return _ap(pivp, GK, [[4 * GK, 3], [1, GK]])
                return _ap(pv, K, [[G * 4 * K, 3], [4 * K, G], [1, K]])

            # ---- stage A: packed r-vectors and cross products ----
            if arange_quads:
                # quad-permuted pivots: v-dim is the q axis, (g,k) contiguous
                nc.vector.tensor_sub(
                    _ap(rv, 0, [[RVv, 3], [RVc, 3], [1, GK]]),
                    _ap(pivp, GK, [[GK, 3], [4 * GK, 3], [1, GK]]),
                    _ap(pivp, 0, [[GK, 3], [4 * GK, 3], [1, GK]]))
            else:
                for g in range(G):
                    in1 = _ap(pv, g * 4 * K + K,
                              [[K, 3], [G * 4 * K, 3], [1, K]])
                    in0 = _ap(pv, g * 4 * K + 0,
                              [[K, 3], [G * 4 * K, 3], [1, K]])
                    nc.vector.tensor_sub(
                        _ap(rv, g * K, [[RVv, 3], [RVc, 3], [1, K]]), in1, in0)
            # duplicate comps x,y into slots 3,4
            nc.vector.tensor_copy(
                _ap(rv, 3 * RVc, [[RVv, 3], [RVc, 2], [1, GK]]),
                _ap(rv, 0, [[RVv, 3], [RVc, 2], [1, GK]]))
            # nIJK, nJKL = cross(A=[rIJ,rJK], B=[rJK,rKL]) via comp offsets
            nc.vector.tensor_mul(
                _ap(t1v, 0, [[3 * GK, 2], [GK, 3], [1, GK]]),
                _ap(rv, RVc, [[RVv, 2], [RVc, 3], [1, GK]]),
                _ap(rv, RVv + 2 * RVc, [[RVv, 2], [RVc, 3], [1, GK]]))
            nc.vector.tensor_mul(
                _ap(t2v, 0, [[3 * GK, 2], [GK, 3], [1, GK]]),
                _ap(rv, 2 * RVc, [[RVv, 2], [RVc, 3], [1, GK]]),
                _ap(rv, RVv + RVc, [[RVv, 2], [RVc, 3], [1, GK]]))
            nc.vector.tensor_sub(
                _ap(nn, 0, [[NVv, 2], [GK, 3], [1, GK]]),
                _ap(t1v, 0, [[3 * GK, 2], [GK, 3], [1, GK]]),
                _ap(t2v, 0, [[3 * GK, 2], [GK, 3], [1, GK]]))
            nc.vector.tensor_copy(
                _ap(nn, 3 * GK, [[NVv, 2], [GK, 2], [1, GK]]),
                _ap(nn, 0, [[NVv, 2], [GK, 2], [1, GK]]))
            # m = nIJK x rJK -> NN vec slot 2
            nc.vector.tensor_mul(
                _ap(t1v, 0, [[GK, 3], [1, GK]]),
                _ap(nn, GK, [[GK, 3], [1, GK]]),
                _ap(rv, RVv + 2 * RVc, [[RVc, 3], [1, GK]]))
            nc.vector.tensor_mul(
                _ap(t2v, 0, [[GK, 3], [1, GK]]),
                _ap(nn, 2 * GK, [[GK, 3], [1, GK]]),
                _ap(rv, RVv + RVc, [[RVc, 3], [1, GK]]))
            nc.vector.tensor_sub(
                _ap(nn, 2 * NVv, [[GK, 3], [1, GK]]),
                _ap(t1v, 0, [[GK, 3], [1, GK]]),
                _ap(t2v, 0, [[GK, 3], [1, GK]]))

            # sin/cos of targets with range reduction (Sin table ok |x|<~3.55)
            # — emitted here, early, so the ACT hops overlap the cross
            # products instead of stalling the affine assembly later
            def reduced_sin(shift_quarter, extra):
                q = aff(vv, 1.0 / TWO_PI, 1024.0 + shift_quarter)
                qi = T(i32)
                nc.vector.tensor_copy(qi, q)     # f32->i32 rounds to nearest
                qf = T()
                nc.vector.tensor_copy(qf, qi)
                t = aff(qf, -TWO_PI, 1024.0 * TWO_PI + extra)
                return activ(add(vv, t), Act.Sin)

            sv = reduced_sin(0.0, 0.0)
            cv = reduced_sin(0.25, _HALF_PI)

            # pJ source: read straight out of pivP when available, else make
            # a compact copy
            if arange_quads:
                pj_b, pj_off, pj_cs = pivp, GK, 4 * GK
            else:
                PJC = pool.tile([P, 3, G, K], f32)
                pjc = PJC[:, :, :, :]
                nc.vector.tensor_copy(_ap(pjc, 0, [[GK, 3], [K, G], [1, K]]),
                                      _ap_cat3(pJ))
                pj_b, pj_off, pj_cs = pjc, 0, GK

            def rvec(v, c):
                return _ap(rv, v * RVv + c * RVc, [[K, G], [1, K]])

            def nvec(v, c):
                return _ap(nn, v * NVv + c * GK, [[K, G], [1, K]])

            rJK = [rvec(1, c) for c in range(3)]
            rjk_off = RVv                      # RV vec 1, comp stride RVc
            m_off = 2 * NVv                    # m lives in NN vec 2

            # paired dot products: one mul+reduce covers two dots that share
            # a left operand; results land adjacently for fused downstream ops
            DOTS = pool.tile([P, 4, GK], f32)  # rows: x0, l1^2, y0, lm^2
            DP4 = pool.tile([P, 2, GK, 3], f32)
            dots = DOTS[:, :, :]
            dp4 = DP4[:, :, :, :]
            # {x0, l1^2} = nIJK . (nJKL, nIJK)
            nc.vector.tensor_mul(
                dp4,
                _ap(nn, 0, [[0, 2], [1, GK], [GK, 3]]),
                _ap(nn, NVv, [[-NVv, 2], [1, GK], [GK, 3]]))
            nc.vector.tensor_reduce(
                _ap(dots, 0, [[GK, 2], [1, GK]]), dp4,
                mybir.AxisListType.X, Alu.add)
            # y0 = m . nJKL (single dot; reuse dp4's first GK*3 lane block)
            nc.vector.tensor_mul(
                _ap(dp4, 0, [[3, GK], [1, 3]]),
                _ap(nn, m_off, [[1, GK], [GK, 3]]),
                _ap(nn, NVv, [[1, GK], [GK, 3]]))
            nc.vector.tensor_reduce(
                _ap(dots, 2 * GK, [[1, GK]]),
                _ap(dp4, 0, [[3, GK], [1, 3]]),
                mybir.AxisListType.X, Alu.add)
            jks = dot3v(rv, rjk_off, RVc, rv, rjk_off, RVc)
            # lm^2 = l1^2 * |rJK|^2  (m = nIJK x rJK with nIJK _|_ rJK)
            nc.vector.tensor_mul(
                _ap(dots, 3 * GK, [[1, GK]]),
                _ap(dots, GK, [[1, GK]]),
                _ap(jks[:, :, :], 0, [[1, GK]]))
            L1LM = pool.tile([P, 2, GK], f32)  # (l1, lm)
            l1lm = L1LM[:, :, :]
            nc.scalar.activation(
                _ap(l1lm, 0, [[GK, 2], [1, GK]]),
                _ap(dots, GK, [[2 * GK, 2], [1, GK]]), Act.Sqrt)
            XY = pool.tile([P, 2, GK], f32)    # (x1, y1) = (x0*lm, y0*l1)
            xy = XY[:, :, :]
            nc.vector.tensor_mul(
                xy,
                _ap(dots, 0, [[2 * GK, 2], [1, GK]]),
                _ap(l1lm, GK, [[-GK, 2], [1, GK]]))
            SQ = pool.tile([P, 2, GK], f32)
            sq = SQ[:, :, :]
            nc.vector.tensor_mul(sq, xy, xy)
            hs = T()
            nc.vector.tensor_add(_ap(hs[:, :, :], 0, [[1, GK]]),
                                 _ap(sq, 0, [[1, GK]]),
                                 _ap(sq, GK, [[1, GK]]))
            hr = T(); nc.vector.reciprocal(hr, hs)
            rh = activ(hr, Act.Sqrt)            # 1/hypot
            CS = pool.tile([P, 2, GK], f32)     # (ccur, scur)
            cs_ = CS[:, :, :]
            nc.vector.tensor_mul(
                cs_, xy, _ap(rh[:, :, :], 0, [[0, 2], [1, GK]]))
            jkr = T(); nc.vector.reciprocal(jkr, jks)
            jrs = activ(jkr, Act.Sqrt)          # 1/|rJK|
            AXT = pool.tile([P, 3, G, K], f32)
            axt = AXT[:, :, :, :]
            nc.vector.tensor_mul(
                _ap(axt, 0, [[GK, 3], [1, GK]]),
                _ap(rv, rjk_off, [[RVc, 3], [1, GK]]),
                _ap(jrs[:, :, :], 0, [[0, 3], [1, GK]]),
            )
            ax = [_ap(axt, c * GK, [[K, G], [1, K]]) for c in range(3)]

            PC1 = pool.tile([P, 2, GK], f32)   # cv * (ccur, scur)
            PC2 = pool.tile([P, 2, GK], f32)   # sv * (ccur, scur)
            pc1 = PC1[:, :, :]
            pc2 = PC2[:, :, :]
            nc.vector.tensor_mul(pc1, cs_, _ap(cv[:, :, :], 0, [[0, 2], [1, GK]]))
            nc.vector.tensor_mul(pc2, cs_, _ap(sv[:, :, :], 0, [[0, 2], [1, GK]]))
            c_ = T()
            s_ = T()
            nc.vector.tensor_add(_ap(c_[:, :, :], 0, [[1, GK]]),
                                 _ap(pc1, 0, [[1, GK]]), _ap(pc2, GK, [[1, GK]]))
            nc.vector.tensor_sub(_ap(s_[:, :, :], 0, [[1, GK]]),
                                 _ap(pc2, 0, [[1, GK]]), _ap(pc1, GK, [[1, GK]]))
            t1_ = T()
            nc.vector.tensor_scalar(t1_, c_, -1.0, 1.0, Alu.mult, Alu.add)  # 1-cos

            TAX = pool.tile([P, 3, G, K], f32)
            SAX = pool.tile([P, 3, G, K], f32)
            UD = pool.tile([P, 3, G, K], f32)
            OD = pool.tile([P, 2, G, K], f32)
            taxv = TAX[:, :, :, :]
            saxv = SAX[:, :, :, :]
            udv = UD[:, :, :, :]
            odv = OD[:, :, :, :]
            d3 = [[GK, 3], [1, GK]]
            bc3 = [[0, 3], [1, GK]]
            nc.vector.tensor_mul(_ap(taxv, 0, d3), _ap(axt, 0, d3),
                                 _ap(t1_[:, :, :], 0, bc3))
            nc.vector.tensor_mul(_ap(saxv, 0, d3), _ap(axt, 0, d3),
                                 _ap(s_[:, :, :], 0, bc3))
            nc.vector.tensor_mul(_ap(udv, 0, d3), _ap(taxv, 0, d3),
                                 _ap(axt, 0, d3))

            def aq(q):
                return _ap(at, q, [[12 * K, G], [12, K]])

            # diagonal: q = 0,5,10 -> stride 5*GK
            nc.vector.tensor_add(
                _ap(at, 0, [[5, 3], [12, GK]]),
                _ap(udv, 0, d3),
                _ap(c_[:, :, :], 0, bc3),
            )
            # off-diagonal products: txy,txz = tax0*(ax1,ax2); tyz = tax1*ax2
            nc.vector.tensor_mul(
                _ap(odv, 0, [[GK, 2], [1, GK]]),
                _ap(axt, GK, [[GK, 2], [1, GK]]),
                _ap(taxv, 0, [[0, 2], [1, GK]]),
            )
            tyz = T()
            nc.vector.tensor_mul(tyz, _ap(taxv, GK, [[K, G], [1, K]]),
                                 _ap(axt, 2 * GK, [[K, G], [1, K]]))
            txy = _ap(odv, 0, [[K, G], [1, K]])
            txz = _ap(odv, GK, [[K, G], [1, K]])
            sax = [_ap(saxv, c * GK, [[K, G], [1, K]]) for c in range(3)]
            nc.vector.tensor_sub(aq(1), txy, sax[2])
            nc.vector.tensor_add(aq(4), txy, sax[2])
            nc.vector.tensor_add(aq(2), txz, sax[1])
            nc.vector.tensor_sub(aq(8), txz, sax[1])
            nc.vector.tensor_sub(aq(6), tyz, sax[0])
            nc.vector.tensor_add(aq(9), tyz, sax[0])

            # b = pJ - R @ pJ : batched products, reduce, sub
            BP = pool.tile([P, 3, GK, 3], f32)
            bp = BP[:, :, :, :]
            nc.vector.tensor_mul(
                bp,
                _ap(at, 0, [[4, 3], [12, GK], [1, 3]]),
                _ap(pj_b, pj_off, [[0, 3], [1, GK], [pj_cs, 3]]),
            )
            RPJ = pool.tile([P, 3, G, K], f32)
            rpj = RPJ[:, :, :, :]
            nc.vector.tensor_reduce(
                _ap(rpj, 0, [[GK, 3], [1, GK]]), bp,
                mybir.AxisListType.X, Alu.add)
            nc.vector.tensor_sub(
                _ap(at, 3, [[4, 3], [12, GK]]),
                _ap(pj_b, pj_off, [[pj_cs, 3], [1, GK]]),
                _ap(rpj, 0, [[GK, 3], [1, GK]]),
            )

            # ---- stage B: blocked prefix composition ----


            def compose(dst, dq, dbd, doff, left, lq, lbd, loff,
                        right, rq, rbd, roff):
                """dst[i,j,*] = sum_m left[i,m,*]*right[m,j,*]; dst[i,3,*] +=
                left[i,3,*].  *bd = batch [step,count] dims (equal counts)."""
                counts = [d[1] for d in dbd]
                assert [d[1] for d in lbd] == counts
                assert [d[1] for d in rbd] == counts
                nb = 1
                for cnt in counts:
                    nb *= cnt
                abd = []
                stp = 1
                for cnt in reversed(counts):
                    abd.insert(0, [stp, cnt])
                    stp *= cnt

                def accv(base):
                    return _ap(base, 0, [[4 * nb, 3], [nb, 4]] + abd)

                dstv = _ap(dst, doff, [[4 * dq, 3], [dq, 4]] + dbd)

                def dmul(tgt, mrow):
                    nc.vector.tensor_mul(
                        accv(tgt),
                        _ap(right, roff + 4 * mrow * rq,
                            [[0, 3], [rq, 4]] + rbd),
                        _ap(left, loff + mrow * lq,
                            [[4 * lq, 3], [0, 4]] + lbd),
                    )

                dmul(acc, 0)
                dmul(ac2, 1)
                nc.vector.tensor_add(accv(acc), accv(acc), accv(ac2))
                dmul(ac2, 2)
                nc.vector.tensor_add(dstv, accv(acc), accv(ac2))
                bias_d = _ap(dst, doff + 3 * dq, [[4 * dq, 3]] + dbd)
                nc.vector.tensor_add(
                    bias_d, bias_d,
                    _ap(left, loff + 3 * lq, [[4 * lq, 3]] + lbd),
                )

            # within-block scan, in place: A[:, t] <- A[:, t-1] o A[:, t]
            # (the 3 muls read the slot before the final add overwrites it).
            # q-innermost layout keeps every big operand stride-1.
            d_b = [[12 * L, NB]]

            def compose_w(t):
                def right_ap(m):   # R[m, j], broadcast over i
                    return _ap(at, 12 * t + 4 * m, [[0, 3]] + d_b + [[1, 4]])

                def left_ap(m):    # L[i, m], broadcast over j
                    return _ap(at, 12 * (t - 1) + m, [[4, 3]] + d_b + [[0, 4]])

                def accv(base):
                    return _ap(base, 0, [[4 * NB, 3], [4, NB], [1, 4]])

                dstv = _ap(at, 12 * t, [[4, 3]] + d_b + [[1, 4]])
                nc.vector.tensor_mul(accv(acc), right_ap(0), left_ap(0))
                nc.vector.tensor_mul(accv(ac2), right_ap(1), left_ap(1))
                nc.vector.tensor_add(accv(acc), accv(acc), accv(ac2))
                nc.vector.tensor_mul(accv(ac2), right_ap(2), left_ap(2))
                nc.vector.tensor_add(dstv, accv(acc), accv(ac2))
                bias_d = _ap(at, 12 * t + 3, [[4, 3]] + d_b)
                nc.vector.tensor_add(
                    bias_d, bias_d, _ap(at, 12 * (t - 1) + 3, [[4, 3]] + d_b))

            for t in range(1, L):
                compose_w(t)
            # block products
            nc.vector.tensor_copy(
                _ap(pt, 0, [[NBP, 12], [1, NB]]),
                _ap(at, 12 * (L - 1), [[1, 12], [12 * L, NB]]),
            )
            # block-prefix scan: Hillis-Steele over the flattened (g,b) axis.
            # Lanes j%B < s read the neighbour's tail (garbage) and are
            # patched by the trailing copy before the buffers swap.
            src_pt, dst_pt = pt, pt2
            s = 1
            while s < B:
                compose(dst_pt, NBP, [[1, NB]], 0,
                        src_pt, NBP, [[1, NB]], -s,
                        src_pt, NBP, [[1, NB]], 0)
                nc.vector.tensor_copy(
                    _ap(dst_pt, 0, [[NBP, 12], [B, G], [1, s]]),
                    _ap(src_pt, 0, [[NBP, 12], [B, G], [1, s]]))
                src_pt, dst_pt = dst_pt, src_pt
                s *= 2
            ptf = src_pt

            # ---- stage C ----
            def dma_out_tile(which):
                """DMA an output tile, one component plane at a time, so the
                early planes drain while compute finishes the last one.
                Tile's subtile deps make each wait only on its own writers."""
                if which == "A":
                    for i in range(3):
                        nc.scalar.dma_start(
                            out=_dram_ap(outAd[:, :], i * G * SP,
                                         [[3 * G * SP, P], [1, G * SP]]),
                            in_=_ap(outa, i * G * SP, [[1, G * SP]]),
                        )
                else:
                    for i in range(3):
                        nc.scalar.dma_start(
                            out=_dram_ap(outBd[:, :], i * G * MB,
                                         [[3 * G * MB, P], [1, G * MB]]),
                            in_=_ap(outb, i * G * MB, [[1, G * MB]]),
                        )

            def apply_single_from(coef, coefq, coefoff, m0, length):
                """out[:, :, m0:m0+length] = R@p + b with per-(partition,g)
                scalar coefficients from `coef` (q stride coefq, g stride
                coefoff).  Muls on ACT (per-partition scale), adds on DVE."""
                plbase, mloc, gs, cs = pl_view(m0, length, None)
                obase, omloc, ogs, ocs = out_view(m0, length)
                tmp_idx[0] += 1
                prod = [[pool.tile([P, G * length], f32,
                                   name=f"prod{tmp_idx[0]}_{i}_{cc}")[:, :]
                         for cc in range(3)] for i in range(3)]
                for i in range(3):
                    for cc in range(3):
                        for g in range(G):
                            nc.scalar.activation(
                                _ap(prod[i][cc], g * length, [[1, length]]),
                                _ap(plbase, cc * cs + g * gs + mloc,
                                    [[1, length]]),
                                Act.Identity,
                                scale=_ap(coef, (4 * i + cc) * coefq
                                          + g * coefoff, [[1, 1]]),
                            )
                for i in range(3):
                    d_t = [[length, G], [1, length]]
                    s1 = _ap(prod[i][0], 0, d_t)
                    nc.vector.tensor_add(s1, s1, _ap(prod[i][1], 0, d_t))
                    nc.vector.tensor_add(s1, s1, _ap(prod[i][2], 0, d_t))
                    for g in range(G):
                        # + translation via the ACT bias port (keeps DVE free)
                        nc.scalar.activation(
                            _ap(obase, i * ocs + g * ogs + omloc, [[1, length]]),
                            _ap(prod[i][0], g * length, [[1, length]]),
                            Act.Identity,
                            bias=_ap(coef, (4 * i + 3) * coefq + g * coefoff,
                                     [[1, 1]]),
                            scale=1.0,
                        )

            pt_last = bass.AP(tensor=ptf.tensor, offset=ptf.offset + (B - 1),
                              ap=list(ptf.ap))

            def apply_runs(starts, length, ks):
                nr = len(starts)
                if nr == 1 and ks[0] == K - 1:
                    # chain-last prefix == last block product: ready right
                    # after the block-prefix scan, before distribute.
                    apply_single_from(pt_last, NBP, B, starts[0], length)
                    return
                if nr == 1:
                    base = bass.AP(tensor=ct.tensor, offset=ct.offset + ks[0],
                                   ap=list(ct.ap))
                    apply_single_from(base, GK, K, starts[0], length)
                    return
                sm = starts[1] - starts[0]
                sk = ks[1] - ks[0]
                m0, k0 = starts[0], ks[0]
                span = max(starts) + length - m0
                plbase, mloc, gs, cs = pl_view(m0, span, None)
                obase, omloc, ogs, ocs = out_view(m0, span)
                d_pl = [[gs, G], [sm, nr], [1, length]]
                d_out = [[ogs, G], [sm, nr], [1, length]]
                d_c = [[K, G], [sk, nr], [0, length]]
                d_acc = [[nr * length, G], [length, nr], [1, length]]
                for i in range(3):
                    for cc in range(2):
                        tgt = acc if cc == 0 else ac2
                        nc.vector.tensor_mul(
                            _ap(tgt, 0, d_acc),
                            _ap(plbase, cc * cs + mloc, d_pl),
                            _ap(ct, (4 * i + cc) * GK + k0, d_c),
                        )
                    nc.vector.tensor_add(
                        _ap(acc, 0, d_acc), _ap(acc, 0, d_acc), _ap(ac2, 0, d_acc)
                    )
                    nc.vector.tensor_mul(
                        _ap(ac2, 0, d_acc),
                        _ap(plbase, 2 * cs + mloc, d_pl),
                        _ap(ct, (4 * i + 2) * GK + k0, d_c),
                    )
                    nc.vector.tensor_add(
                        _ap(acc, 0, d_acc), _ap(acc, 0, d_acc),
                        _ap(ac2, 0, d_acc),
                    )
                    nc.vector.tensor_add(
                        _ap(obase, i * ocs + omloc, d_out),
                        _ap(acc, 0, d_acc),
                        _ap(ct, (4 * i + 3) * GK + k0, d_c),
                    )

            def emit_distribute():
                # distribute: block 0 copies, blocks b>=1 get P[b-1] @ W
                nk = (B - 1) * L
                nc.vector.tensor_copy(
                    _ap(ct, 0, [[GK, 12], [K, G], [1, L]]),
                    _ap(at, 0, [[1, 12], [12 * K, G], [12, L]]),
                )
                # broadcast block prefixes over t so g fuses into 3-dim APs:
                # PTB[g][q][jb*L + t] = ptf[q][g*B + jb]
                for g in range(G):
                    nc.vector.tensor_copy(
                        _ap(ptb, g * 12 * nk, [[nk, 12], [L, B - 1], [1, L]]),
                        _ap(ptf, g * B, [[NBP, 12], [1, B - 1], [0, L]]))
                # iteration (g, kt, j): W reads stay stride-1 innermost
                d_w = [[12 * K, G], [12, nk], [1, 4]]
                d_a = [[4 * nk, G], [4, nk], [1, 4]]
                accs = (acc, ac2, ac3)
                # all 9 cross products first (pure reads of W and P), then
                # the combines
                for m in range(3):
                    for i in range(3):
                        nc.vector.tensor_mul(
                            _ap(accs[m], i * 4 * G * nk, d_a),
                            _ap(at, 12 * L + 4 * m, d_w),
                            _ap(ptb, (4 * i + m) * nk,
                                [[12 * nk, G], [1, nk], [0, 4]]),
                        )
                for i in range(3):
                    o = i * 4 * G * nk
                    nc.vector.tensor_add(
                        _ap(acc, o, d_a), _ap(acc, o, d_a), _ap(ac2, o, d_a))
                    nc.vector.tensor_add(
                        _ap(ct, 4 * i * GK + L, [[K, G], [1, nk], [GK, 4]]),
                        _ap(acc, o, d_a), _ap(ac3, o, d_a))
                    nc.vector.tensor_add(
                        _ap(ct, (4 * i + 3) * GK + L, [[K, G], [1, nk]]),
                        _ap(ct, (4 * i + 3) * GK + L, [[K, G], [1, nk]]),
                        _ap(ptb, (4 * i + 3) * nk, [[12 * nk, G], [1, nk]]),
                    )

            # unmoved atoms: copy + DMA as soon as PL lands
            unmoved = [m for m in range(M) if km[m] < 0]
            u0 = 0
            while u0 < len(unmoved):
                u1 = u0
                while u1 + 1 < len(unmoved) and unmoved[u1 + 1] == unmoved[u1] + 1:
                    u1 += 1
                a0, ln = unmoved[u0], u1 - u0 + 1
                assert a0 + ln <= SP or a0 >= SPB
                ubase, umloc, ugs, ucs = pl_view(a0, ln, None)
                uobase, uomloc, uogs, uocs = out_view(a0, ln)
                nc.vector.tensor_copy(
                    _ap(uobase, uomloc, [[uocs, 3], [uogs, G], [1, ln]]),
                    _ap(ubase, umloc, [[ucs, 3], [ugs, G], [1, ln]]),
                )
                u0 = u1 + 1

            # classes: chain-last single-run first (overlaps distribute)
            by_len = {}
            for (m0, ln, k) in runs:
                by_len.setdefault(ln, []).append((m0, k))
            classes = sorted(
                by_len.items(),
                key=lambda kv: 0 if (len(kv[1]) == 1 and kv[1][0][1] == K - 1)
                else 1)
            emitted_distribute = False
            for ln, rs in classes:
                starts = [r[0] for r in rs]
                ks = [r[1] for r in rs]
                nr = len(rs)
                chain_last_single = nr == 1 and ks[0] == K - 1
                if not chain_last_single and not emitted_distribute:
                    emit_distribute()
                    emitted_distribute = True
                regular = nr <= 2 or (
                    all(starts[r] == starts[0] + r * (starts[1] - starts[0])
                        for r in range(nr))
                    and all(ks[r] == ks[0] + r * (ks[1] - ks[0])
                            for r in range(nr))
                )
                if regular:
                    apply_runs(starts, ln, ks)
                else:
                    for (m0, k) in rs:
                        apply_runs([m0], ln, [k])
            # B tile drains first (its writers finish early); HWDGE is FIFO
            # per engine so the early DMA must be queued first
            if OUTB is not None:
                dma_out_tile("B")
            dma_out_tile("A")

    _split_multi_waits(nc)
    return nc


_BUILD_CACHE = {}


def make_in_maps(input, pos, angles, move_mask):
    input = np.asarray(input, dtype=np.float32)
    pos = np.asarray(pos, dtype=np.float32)
    angles = np.asarray(angles)
    N, K = input.shape
    M = pos.shape[1]
    NL = N // NCORES
    G = NL // P
    SP, SPB = _seg_bounds(angles, np.asarray(move_mask).astype(bool), M)
    arange_quads = bool((angles == np.arange(K * 4).reshape(K, 4)).all())
    in_maps = []
    for c in range(NCORES):
        sl = slice(c * NL, (c + 1) * NL)
        # (NL, M, 3) -> (P, 3, G, M): partition-major so each partition row
        # is one contiguous DMA descriptor
        pm = pos[sl].reshape(G, P, M, 3).transpose(1, 3, 0, 2)
        vrows = input[sl].reshape(G, P, K).transpose(1, 0, 2).reshape(P, G * K)
        im = {"vin": np.ascontiguousarray(vrows),
              "pivA": np.ascontiguousarray(
                  pm[:, :, :, :SP].reshape(P, 3 * G * SP))}
        if arange_quads:
            # pivP[p][c][q][g][k] = pm[p][c][g][4k+q]
            pp = pm[:, :, :, :4 * K].reshape(P, 3, G, K, 4)
            im["pivP"] = np.ascontiguousarray(
                pp.transpose(0, 1, 4, 2, 3).reshape(P, 12 * G * K))
        if SPB < M:
            im["posB"] = np.ascontiguousarray(pm[:, :, :, SPB:])
        in_maps.append(im)
    return in_maps


def kernel(input, pos, angles, move_mask):
    input = np.ascontiguousarray(np.asarray(input, dtype=np.float32))
    pos = np.ascontiguousarray(np.asarray(pos, dtype=np.float32))
    angles = np.asarray(angles)
    move_mask = np.asarray(move_mask).astype(bool)

    N, K = input.shape
    _, M, three = pos.shape
    assert three == 3
    assert N % (NCORES * P) == 0
    NL = N // NCORES

    key = (N, K, M, angles.tobytes(), move_mask.tobytes())
    nc = _BUILD_CACHE.get(key)
    if nc is None:
        nc = _build(angles, move_mask, NL, K, M)
        _BUILD_CACHE[key] = nc

    in_maps = make_in_maps(input, pos, angles, move_mask)

    # the axon-proxied NRT occasionally wedges transiently
    # (NRT_EXEC_UNIT_UNRECOVERABLE); one retry recovers it
    try:
        res = run_bass_kernel_spmd(nc, in_maps, list(range(NCORES)))
    except Exception:
        res = run_bass_kernel_spmd(nc, in_maps, list(range(NCORES)))

    G = NL // P
    SP, SPB = _seg_bounds(angles, move_mask, M)
    out = np.empty((N, M, 3), dtype=np.float32)
    for c in range(NCORES):
        sl = slice(c * NL, (c + 1) * NL)
        # (P, 3, G, cols) -> (NL, cols, 3)
        oa = res.results[c]["outA"].reshape(P, 3, G, SP)
        out[sl, :SPB] = oa.transpose(2, 0, 3, 1).reshape(NL, SP, 3)[:, :SPB]
        if SPB < M:
            ob = res.results[c]["outB"].reshape(P, 3, G, M - SPB)
            out[sl, SPB:] = ob.transpose(2, 0, 3, 1).reshape(NL, M - SPB, 3)
    return out

